# revision 15
# baseline (speedup 1.0000x reference)
import sys

sys.path.insert(0, "/opt/trn_rl_repo")
import math

import numpy as np
import ml_dtypes

from concourse import bass, bacc, mybir
from concourse import tile
from concourse.bass_utils import run_bass_kernel_spmd

BF = ml_dtypes.bfloat16
bf16 = mybir.dt.bfloat16
f32 = mybir.dt.float32
Alu = mybir.AluOpType
Act = mybir.ActivationFunctionType

B, L, CIN, COND, DM, DEPTH = 8, 256, 256, 2048, 768, 12
NST, DCONV, DI, DTR = 16, 4, 1536, 48
NH, HD, LT, FREQ = 8, 96, 35, 256
GM, GI = DM // 128, DI // 128          # 6, 12
SEG = L + 4                            # 260, 4 zero pad cols reset scan state
SCANW = GI * SEG                       # 3120
HP = 128                               # padded head dim
SQ = 1.0 / math.sqrt(HD)
N_CORES = 8


def _bf(a):
    return np.ascontiguousarray(a, dtype=np.float32).astype(BF)


def _f(a):
    return np.ascontiguousarray(a, dtype=np.float32)


def prep_shared(inp):
    """Host-side layout/dtype staging of the weights (shared by all cores)."""
    d = {}
    d["xwT"] = _bf(inp["xw"].T.reshape(2, 128, DM))
    d["tw1T"] = _bf(inp["tw1"].T.reshape(2, 128, DM))
    d["tw2T"] = _bf(inp["tw2"].T.reshape(6, 128, DM))
    d["fcw1T"] = _bf(inp["fcw1"].T.reshape(16, 128, DM))
    d["fcw2T"] = _bf(inp["fcw2"].T.reshape(6, 128, DM))
    d["flwT"] = _bf(inp["flw"].T.reshape(16, 128, DM))
    d["adaT"] = _bf(np.ascontiguousarray(inp["ada_w"].transpose(0, 2, 1)).reshape(DEPTH, 6, 128, 6 * DM))
    d["in_wT"] = _bf(np.ascontiguousarray(inp["in_w"].transpose(0, 2, 1)).reshape(DEPTH, 6, 128, 2 * DI))
    xpt = np.ascontiguousarray(inp["xproj_w"].transpose(0, 2, 1)).astype(np.float32)  # [12,1536,80]
    xpp = np.zeros((DEPTH, DI, 96), np.float32)
    xpp[:, :, 0:48] = xpt[:, :, 0:48]
    xpp[:, :, 64:96] = xpt[:, :, 48:80]
    d["xprojT"] = _bf(xpp.reshape(DEPTH, 12, 128, 96))
    d["dt_wT"] = _bf(np.ascontiguousarray(inp["dt_w"].transpose(0, 2, 1)))          # [12,48,1536]
    d["out_wT"] = _bf(np.ascontiguousarray(inp["out_w"].transpose(0, 2, 1)).reshape(DEPTH, 12, 128, DM))
    qkv = inp["qkv_w"]
    wq, wk, wv = qkv[:, :DM], qkv[:, DM:2 * DM], qkv[:, 2 * DM:]
    for nm, w in (("wqT", wq), ("wkT", wk), ("wvT", wv)):
        wt = np.ascontiguousarray(w.transpose(0, 2, 1))                              # [12,768,768]
        pad = np.zeros((DEPTH, DM, NH * HP), np.float32)
        for h in range(NH):
            pad[:, :, h * HP:h * HP + HD] = wt[:, :, h * HD:(h + 1) * HD]
        d[nm] = _bf(pad.reshape(DEPTH, 6, 128, NH * HP))
    aot = np.ascontiguousarray(inp["ao_w"].transpose(0, 2, 1))                       # [12,768(dv),768]
    aop = np.zeros((DEPTH, NH * HP, DM), np.float32)
    for h in range(NH):
        aop[:, h * HP:h * HP + HD] = aot[:, h * HD:(h + 1) * HD]
    d["aoT"] = _bf(aop.reshape(DEPTH, 8, 128, DM))
    d["fw1T"] = _bf(np.ascontiguousarray(inp["fw1"].transpose(0, 2, 1)).reshape(DEPTH, 6, 128, 4 * DM))
    d["fw2T"] = _bf(np.ascontiguousarray(inp["fw2"].transpose(0, 2, 1)).reshape(DEPTH, 24, 128, DM))
    d["finadaT"] = _bf(inp["fin_ada_w"].T.reshape(6, 128, 2 * DM))
    d["finT"] = _bf(inp["fin_w"].T.reshape(6, 128, CIN))

    # per-partition scatters (fp32), layout [128, ...]
    d["xb_sc"] = _f(inp["xb"].reshape(6, 128).T)
    d["flb_sc"] = _f(inp["flb"].reshape(6, 128).T)
    d["tb1_r"] = _f(inp["tb1"].reshape(1, DM))
    d["tb2_r"] = _f(inp["tb2"].reshape(1, DM))
    d["fcb1_r"] = _f(inp["fcb1"].reshape(1, DM))
    d["fcb2_r"] = _f(inp["fcb2"].reshape(1, DM))
    d["flpos_sc"] = _f(np.ascontiguousarray(inp["flpos"][0].T).reshape(6, 128, LT).transpose(1, 0, 2))
    d["dtb_sc"] = _f(inp["dt_b"].reshape(DEPTH, 12, 128).transpose(2, 0, 1).reshape(128, -1))
    d["convw_sc"] = _f(inp["conv_w"].reshape(DEPTH, 12, 128, 4).transpose(2, 0, 1, 3).reshape(128, -1))
    d["convb_sc"] = _f(inp["conv_b"].reshape(DEPTH, 12, 128).transpose(2, 0, 1).reshape(128, -1))
    d["Dp_sc"] = _f(inp["Dp"].reshape(DEPTH, 12, 128).transpose(2, 0, 1).reshape(128, -1))
    d["alog_sc"] = _f(inp["A_log"].reshape(DEPTH, 12, 128, NST).transpose(2, 0, 1, 3).reshape(128, -1))
    d["adab_sc"] = _f(inp["ada_b"].reshape(DEPTH, 6, 6, 128).transpose(3, 0, 1, 2).reshape(128, -1))
    qb = inp["qkv_b"]
    for nm, bias in (("bq_sc", qb[:, :DM]), ("bk_sc", qb[:, DM:2 * DM])):
        arr = np.zeros((DEPTH, NH, HP), np.float32)
        arr[:, :, :HD] = np.asarray(bias, np.float32).reshape(DEPTH, NH, HD)
        d[nm] = _f(arr.transpose(2, 0, 1).reshape(128, -1))
    bv = np.zeros((DEPTH, NH, HP), np.float32)
    bv[:, :, :HD] = np.asarray(qb[:, 2 * DM:], np.float32).reshape(DEPTH, NH, HD)
    d["bv_pad"] = _bf(bv.reshape(DEPTH, NH * HP))
    d["aob_sc"] = _f(inp["ao_b"].reshape(DEPTH, 6, 128).transpose(2, 0, 1).reshape(128, -1))
    d["fb1_sc"] = _f(inp["fb1"].reshape(DEPTH, 24, 128).transpose(2, 0, 1).reshape(128, -1))
    d["fb2_sc"] = _f(inp["fb2"].reshape(DEPTH, 6, 128).transpose(2, 0, 1).reshape(128, -1))
    d["finadab_sc"] = _f(inp["fin_ada_b"].reshape(2, 6, 128).transpose(2, 0, 1).reshape(128, 12))
    d["finb_sc"] = _f(inp["fin_b"].reshape(2, 128).T)

    # constants (input independent)
    d["id128"] = _bf(np.eye(128))
    d["id128f"] = _f(np.eye(128))
    d["ones_col"] = _f(np.ones((128, 1)))
    d["ones_colb"] = _bf(np.ones((128, 1)))
    d["ones_row"] = _f(np.ones((1, 128)))
    half = FREQ // 2
    return d


def prep_core(inp, b):
    d = {}
    d["xT"] = _bf(np.asarray(inp["x"][b], np.float32).T.reshape(2, 128, L))
    half = FREQ // 2
    fr = np.exp(-math.log(10000.0) * np.arange(half) / half).reshape(128, 1)
    d["ftp"] = _f(np.concatenate([fr, np.full((128, 1), np.asarray(inp["t"][b], np.float32))], 1))
    d["fc_cols"] = _bf(np.asarray(inp["fc"][b], np.float32).reshape(16, 128).T)
    d["flT"] = _bf(np.asarray(inp["fl"][b], np.float32).T.reshape(16, 128, LT))
    return d


def build_nc(depth=DEPTH):
    nc = bacc.Bacc(None)
    for val in (math.pi / 2, 1e-6, -math.pi):
        t_ = nc.alloc_sbuf_tensor(f"const-f32-{val}", [128, 1], f32)
        nc.gpsimd.memset(t_.ap(), val)
        nc.const_aps.aps[(f32, val)] = t_.ap()
    nc.all_engine_barrier()
    P = nc.declare_dram_parameter

    W = {}
    for nm, shp, dt in [
        ("xwT", [2, 128, DM], bf16), ("tw1T", [2, 128, DM], bf16),
        ("tw2T", [6, 128, DM], bf16), ("fcw1T", [16, 128, DM], bf16),
        ("fcw2T", [6, 128, DM], bf16), ("flwT", [16, 128, DM], bf16),
        ("adaT", [DEPTH, 6, 128, 6 * DM], bf16), ("in_wT", [DEPTH, 6, 128, 2 * DI], bf16),
        ("xprojT", [DEPTH, 12, 128, 96], bf16), ("dt_wT", [DEPTH, 48, DI], bf16),
        ("out_wT", [DEPTH, 12, 128, DM], bf16),
        ("wqT", [DEPTH, 6, 128, NH * HP], bf16), ("wkT", [DEPTH, 6, 128, NH * HP], bf16),
        ("wvT", [DEPTH, 6, 128, NH * HP], bf16), ("aoT", [DEPTH, 8, 128, DM], bf16),
        ("fw1T", [DEPTH, 6, 128, 4 * DM], bf16), ("fw2T", [DEPTH, 24, 128, DM], bf16),
        ("finadaT", [6, 128, 2 * DM], bf16), ("finT", [6, 128, CIN], bf16),
        ("xb_sc", [128, 6], f32), ("flb_sc", [128, 6], f32),
        ("tb1_r", [1, DM], f32), ("tb2_r", [1, DM], f32),
        ("fcb1_r", [1, DM], f32), ("fcb2_r", [1, DM], f32),
        ("flpos_sc", [128, 6, LT], f32),
        ("dtb_sc", [128, DEPTH * 12], f32), ("convw_sc", [128, DEPTH * 48], f32),
        ("convb_sc", [128, DEPTH * 12], f32), ("Dp_sc", [128, DEPTH * 12], f32),
        ("alog_sc", [128, DEPTH * 192], f32), ("adab_sc", [128, DEPTH * 36], f32),
        ("bq_sc", [128, DEPTH * 8], f32), ("bk_sc", [128, DEPTH * 8], f32),
        ("bv_pad", [DEPTH, NH * HP], bf16),
        ("aob_sc", [128, DEPTH * 6], f32), ("fb1_sc", [128, DEPTH * 24], f32),
        ("fb2_sc", [128, DEPTH * 6], f32), ("finadab_sc", [128, 12], f32),
        ("finb_sc", [128, 2], f32),
        ("id128", [128, 128], bf16), ("id128f", [128, 128], f32),
        ("ones_col", [128, 1], f32), ("ones_colb", [128, 1], bf16), ("ones_row", [1, 128], f32),
        ("xT", [2, 128, L], bf16), ("ftp", [128, 2], f32),
        ("fc_cols", [128, 16], bf16), ("flT", [16, 128, LT], bf16),
    ]:
        W[nm] = P(nm, shp, dt, isOutput=False)
    out_d = P("out", [L, CIN], f32, isOutput=True)
    scr_b1 = P("scr_b1", [1, DM], f32, isOutput=True)
    scr_b2 = P("scr_b2", [1, DM], f32, isOutput=True)
    scr_b3 = P("scr_b3", [1, DM], f32, isOutput=True)
    scr_mod = P("scr_mod", [DEPTH, 6 * DM], f32, isOutput=True)
    scr_bc = P("scr_bc", [1, 2 * NST * L], bf16, isOutput=True)
    scr_fm = P("scr_fm", [1, 2 * DM], f32, isOutput=True)

    MM, ACT, DVE, GPS, DMA = nc.tensor, nc.scalar, nc.vector, nc.gpsimd, nc.sync

    def g3(ap, n=GI, w=None):
        return ap.rearrange("p (g t) -> p g t", g=n)

    def load_kpm(dst, srcap, kdim):
        DMA.dma_start(out=dst[:].rearrange("p (k m) -> p k m", k=kdim),
                      in_=srcap.rearrange("k p m -> p k m"))

    with tile.TileContext(nc) as tc:
      from contextlib import ExitStack
      with ExitStack() as top:
        cp = top.enter_context(tc.tile_pool(name="cp", bufs=1))
        stg = top.enter_context(tc.tile_pool(name="stg", bufs=2))
        wb = top.enter_context(tc.tile_pool(name="wb", bufs=3))
        ws = top.enter_context(tc.tile_pool(name="ws", bufs=8))

        # ---- persistent SBUF state ----
        x = cp.tile([128, GM * L], f32)
        u_pad = cp.tile([128, SCANW], bf16)
        uc = cp.tile([128, SCANW], bf16)
        z = cp.tile([128, GI * L], bf16)
        dt_t = cp.tile([128, GI * L], bf16)
        du = cp.tile([128, GI * L], bf16)
        brep = cp.tile([128, NST * L], bf16)
        crep = cp.tile([128, NST * L], bf16)
        dA = [cp.tile([128, SCANW], bf16, name="dA0")] * 2
        dB = [cp.tile([128, SCANW], bf16, name=f"dB{i}") for i in range(2)]
        hh = [cp.tile([128, SCANW], bf16, name=f"hh{i}") for i in range(2)]
        y = cp.tile([128, GI * L], bf16)
        y_odd = cp.tile([128, GI * L], bf16)
        tmp1 = cp.tile([128, GM * L], bf16)
        xn = cp.tile([128, GM * L], bf16)
        hffn = cp.tile([128, 12 * L], bf16)
        q_sb = cp.tile([128, NH, L], bf16)
        k_sb = cp.tile([128, NH, LT], bf16)
        vt_sb = cp.tile([LT, NH * HP], bf16)
        pt_sb = cp.tile([LT, NH * L], bf16)
        p_all = cp.tile([128, NH * 2 * LT], bf16)
        rs_all = cp.tile([128, NH * 2], f32)
        ri_all = cp.tile([128, NH * 2], f32)
        avt_sb = cp.tile([128, NH, L], bf16)
        mod_all = cp.tile([128, DEPTH * 36], f32)
        modx_all = cp.tile([128, DEPTH * 12], f32)
        xdbl_sb = cp.tile([48, L], bf16)
        bc_sb = cp.tile([32, L], bf16)
        bvb = cp.tile([LT, NH * HP], bf16)
        modx = cp.tile([128, 12], f32)
        aneg = cp.tile([128, DEPTH * 192], f32)
        fl_e = cp.tile([128, GM * LT], bf16)
        silu_c = cp.tile([128, 6], bf16)
        stat = cp.tile([1, 2 * L], f32)
        stat2 = cp.tile([1, L], f32)
        small = cp.tile([128, 16], f32)      # ang etc
        smalli = cp.tile([128, 2], mybir.dt.int32)
        temb_c = cp.tile([128, 2], bf16)
        cvec = cp.tile([1, DM], f32)
        fmod_sc = cp.tile([128, 12], f32)

        # consts / biases resident
        C = {}
        for nm in ["dtb_sc", "convw_sc", "convb_sc", "Dp_sc", "adab_sc",
                   "bq_sc", "bk_sc", "aob_sc", "fb1_sc", "fb2_sc", "finadab_sc", "finb_sc",
                   "id128", "id128f", "ones_col", "ones_colb", "ones_row"]:
            C[nm] = cp.tile(list(W[nm].shape), W[nm].dtype, name="c_" + nm)
            DMA.dma_start(out=C[nm][:], in_=W[nm][:])

        # zero the pad columns once; interiors are always written strided
        for tl in dA + dB + [u_pad]:
            GPS.memset(tl[:], 0.0)


        # ---------------- preamble ----------------
        with tc.tile_pool(name="pre", bufs=1) as pre:
            from contextlib import ExitStack as _ES
            _es = _ES()
            psv = _es.enter_context(tc.tile_pool(name="psv", bufs=2, space="PSUM"))
            for nm in ["xb_sc", "flb_sc", "flpos_sc", "ftp", "fc_cols"]:
                C[nm] = pre.tile(list(W[nm].shape), W[nm].dtype, tag="p_" + nm, name="c_" + nm)
                DMA.dma_start(out=C[nm][:], in_=W[nm][:])
            for nm in ["tb1_r", "fcb1_r", "tb2_r", "fcb2_r"]:
                C[nm] = pre.tile(list(W[nm].shape), W[nm].dtype, tag="pvb", name="c_" + nm)
                DMA.dma_start(out=C[nm][:], in_=W[nm][:])
            for i in range(8):
                alg = pre.tile([128, 288], f32, tag="pal", name=f"alg{i}")
                DMA.dma_start(out=alg[:], in_=W["alog_sc"][:, i * 288:(i + 1) * 288])
                ACT.activation(out=aneg[:, i * 288:(i + 1) * 288], in_=alg[:], func=Act.Exp)
            DVE.tensor_scalar_mul(out=aneg[:], in0=aneg[:], scalar1=-1.0)
            # time embedding: ang = t*freqs mod 2pi; temb = [cos ang, sin ang]
            DVE.tensor_tensor(out=small[:, 0:1], in0=C["ftp"][:, 0:1], in1=C["ftp"][:, 1:2], op=Alu.mult)
            TWO_PI = 2 * math.pi
            # cos(ang)=sin(ang+pi/2); reduce each argument into [-pi, pi]
            DVE.tensor_scalar_add(out=small[:, 1:2], in0=small[:, 0:1], scalar1=math.pi / 2)
            for j, col in ((0, 1), (1, 0)):  # j=0: cos arg; j=1: sin arg
                src_c = 1 - col  # small col holding the argument
                a_ = small[:, src_c + 0:src_c + 1] if False else None
            for j, srccol in ((0, 1), (1, 0)):
                arg = small[:, srccol:srccol + 1]
                DVE.tensor_scalar_mul(out=small[:, 4 + j:5 + j], in0=arg, scalar1=1.0 / TWO_PI)
                DVE.tensor_copy(out=smalli[:, j:j + 1], in_=small[:, 4 + j:5 + j])
                DVE.tensor_copy(out=small[:, 6 + j:7 + j], in_=smalli[:, j:j + 1])
                DVE.scalar_tensor_tensor(out=small[:, 8 + j:9 + j], in0=small[:, 6 + j:7 + j],
                                         scalar=-TWO_PI, in1=arg, op0=Alu.mult, op1=Alu.add)
                DVE.tensor_scalar(out=small[:, 10 + j:11 + j], in0=small[:, 8 + j:9 + j],
                                  scalar1=math.pi, scalar2=None, op0=Alu.is_gt)
                DVE.scalar_tensor_tensor(out=small[:, 12 + j:13 + j], in0=small[:, 10 + j:11 + j],
                                         scalar=-TWO_PI, in1=small[:, 8 + j:9 + j],
                                         op0=Alu.mult, op1=Alu.add)
                ACT.activation(out=temb_c[:, j:j + 1], in_=small[:, 12 + j:13 + j], func=Act.Sin)

            tw1 = wb.tile([128, 2 * DM], bf16, tag="wb")
            load_kpm(tw1, W["tw1T"][:], 2)
            h1p = psv.tile([1, DM], f32, tag="vec")
            for k in range(2):
                for lo, hi in ((0, 512), (512, 768)):
                    MM.matmul(out=h1p[:, lo:hi],
                              lhsT=temb_c[:, k:k + 1],
                              rhs=tw1[:, k * DM + lo:k * DM + hi],
                              start=(k == 0), stop=(k == 1))
            h1 = pre.tile([1, DM], f32, tag="pv")
            for lo, hi in ((0, 512), (512, 768)):
                DVE.tensor_tensor(out=h1[:, lo:hi], in0=h1p[:, lo:hi], in1=C["tb1_r"][:, lo:hi], op=Alu.add)
            ACT.activation(out=h1[:], in_=h1[:], func=Act.Silu)
            GPS.dma_start(out=scr_b1[:], in_=h1[:])
            h1f = pre.tile([128, 6], f32, tag="pcf")
            GPS.dma_start(out=h1f[:], in_=scr_b1[0, :].rearrange("(g p) -> p g", g=6))
            h1c = pre.tile([128, 6], bf16, tag="pc")
            ACT.activation(out=h1c[:], in_=h1f[:], func=Act.Copy)

            h2p = psv.tile([1, DM], f32, tag="vec")
            for k in range(16):
                if k % 4 == 0:
                    fcw1c = wb.tile([128, 4 * DM], bf16, tag="wb", name=f"fcw1_{k // 4}")
                    load_kpm(fcw1c, W["fcw1T"][k:k + 4], 4)
                for lo, hi in ((0, 512), (512, 768)):
                    MM.matmul(out=h2p[:, lo:hi],
                              lhsT=C["fc_cols"][:, k:k + 1],
                              rhs=fcw1c[:, (k % 4) * DM + lo:(k % 4) * DM + hi],
                              start=(k == 0), stop=(k == 15))
            h2 = pre.tile([1, DM], f32, tag="pv")
            for lo, hi in ((0, 512), (512, 768)):
                DVE.tensor_tensor(out=h2[:, lo:hi], in0=h2p[:, lo:hi], in1=C["fcb1_r"][:, lo:hi], op=Alu.add)
            ACT.activation(out=h2[:], in_=h2[:], func=Act.Silu)
            GPS.dma_start(out=scr_b2[:], in_=h2[:])
            h2f = pre.tile([128, 6], f32, tag="pcf2")
            GPS.dma_start(out=h2f[:], in_=scr_b2[0, :].rearrange("(g p) -> p g", g=6))
            h2c = pre.tile([128, 6], bf16, tag="pc3")
            ACT.activation(out=h2c[:], in_=h2f[:], func=Act.Copy)

            # c = tw2@h1 + fcw2@h2 + tb2 + fcb2 ; silu; scatter
            cp_ps = psv.tile([1, DM], f32, tag="vec")
            nmm = 0
            for hsrc, wnm in ((h1c, "tw2T"), (h2c, "fcw2T")):
                for k in range(6):
                    if k % 3 == 0:
                        wc = wb.tile([128, 3 * DM], bf16, tag="wb", name=f"cw_{wnm}_{k}")
                        load_kpm(wc, W[wnm][k:k + 3], 3)
                    for lo, hi in ((0, 512), (512, 768)):
                        MM.matmul(out=cp_ps[:, lo:hi],
                                  lhsT=hsrc[:, k:k + 1],
                                  rhs=wc[:, (k % 3) * DM + lo:(k % 3) * DM + hi],
                                  start=(nmm == 0), stop=(nmm == 11))
                    nmm += 1
            for lo, hi in ((0, 512), (512, 768)):
                DVE.tensor_tensor(out=cvec[:, lo:hi], in0=cp_ps[:, lo:hi], in1=C["tb2_r"][:, lo:hi], op=Alu.add)
            DVE.tensor_tensor(out=cvec[:], in0=cvec[:], in1=C["fcb2_r"][:], op=Alu.add)
            ACT.activation(out=cvec[:], in_=cvec[:], func=Act.Silu)
            GPS.dma_start(out=scr_b3[:], in_=cvec[:])
            scf32 = pre.tile([128, 6], f32, tag="pc4")
            GPS.dma_start(out=scf32[:], in_=scr_b3[0, :].rearrange("(g p) -> p g", g=6))
            ACT.activation(out=silu_c[:], in_=scf32[:], func=Act.Copy)

            # fl_e = flw@fl + flb + flpos
            flsb = pre.tile([128, 16, LT], bf16, tag="pfl")
            GPS.dma_start(out=flsb[:], in_=W["flT"][:].rearrange("k p m -> p k m"))
            _es.close()
            _es = _ES()
            psfl = _es.enter_context(tc.tile_pool(name="psfl", bufs=1, space="PSUM"))
            fps = [psfl.tile([128, LT], f32, tag=f"fl{m}", name=f"flp{m}") for m in range(6)]
            for k in range(16):
                if k % 4 == 0:
                    flwc = wb.tile([128, 4 * DM], bf16, tag="wb", name=f"flw_{k // 4}")
                    load_kpm(flwc, W["flwT"][k:k + 4], 4)
                for m in range(6):
                    MM.matmul(out=fps[m][:],
                              lhsT=flwc[:, (k % 4) * DM + m * 128:(k % 4) * DM + (m + 1) * 128],
                              rhs=flsb[:, k, :], start=(k == 0), stop=(k == 15))
            for m in range(6):
                t_ = pre.tile([128, LT], f32, tag="pt2", name=f"fle{m}")
                ACT.activation(out=t_[:], in_=fps[m][:], func=Act.Identity, bias=C["flb_sc"][:, m:m + 1])
                DVE.tensor_tensor(out=fl_e[:, m * LT:(m + 1) * LT], in0=t_[:],
                                  in1=C["flpos_sc"][:, m, :], op=Alu.add)

            # x embedding
            _es.close()
            _es = _ES()
            ps1 = _es.enter_context(tc.tile_pool(name="ps1", bufs=2, space="PSUM"))
            xw = wb.tile([128, 2 * DM], bf16, tag="wb")
            load_kpm(xw, W["xwT"][:], 2)
            xsb = pre.tile([128, 2, L], bf16, tag="pfl2")
            GPS.dma_start(out=xsb[:], in_=W["xT"][:].rearrange("k p m -> p k m"))
            for m in range(6):
                xp = ps1.tile([128, L], f32, tag="mm")
                for k in range(2):
                    MM.matmul(out=xp[:], lhsT=xw[:, k * DM + m * 128:k * DM + (m + 1) * 128],
                              rhs=xsb[:, k, :], start=(k == 0), stop=(k == 1))
                ACT.activation(out=x[:, m * L:(m + 1) * L], in_=xp[:],
                               func=Act.Identity, bias=C["xb_sc"][:, m:m + 1])
            _es.close()

        # ---------------- layers ----------------
        x3 = lambda g: x[:, g * L:(g + 1) * L]
        xn3 = lambda g: xn[:, g * L:(g + 1) * L]
        dt3 = dt_t[:].rearrange("p (g t) -> p g t", g=GI)
        du3 = du[:].rearrange("p (g t) -> p g t", g=GI)
        y3 = y[:].rearrange("p (g t) -> p g t", g=GI)
        uc3i = uc[:].rearrange("p (g s) -> p g s", g=GI)[:, :, 4:SEG]
        up3 = u_pad[:].rearrange("p (g s) -> p g s", g=GI)

        def ln_block(l, psA, psS, psB, scale_col, shift_col):
            """LayerNorm of x -> xn (bf16), optionally modulated."""
            ACT.activation(out=tmp1[:], in_=x[:], func=Act.Square)
            st = psS.tile([1, 512], f32, tag="st", name=f"st{l}")
            for g in range(GM):
                MM.matmul(out=st[:, 0:L], lhsT=C["ones_col"][:], rhs=x3(g),
                          start=(g == 0), stop=(g == GM - 1))
            for g in range(GM):
                MM.matmul(out=st[:, L:2 * L], lhsT=C["ones_colb"][:],
                          rhs=tmp1[:, g * L:(g + 1) * L],
                          start=(g == 0), stop=(g == GM - 1))
            ACT.activation(out=stat[:, 0:L], in_=st[:, 0:L], func=Act.Copy, scale=1.0 / DM)
            ACT.activation(out=stat2[:], in_=stat[:, 0:L], func=Act.Square)
            DVE.scalar_tensor_tensor(out=stat2[:], in0=st[:, L:2 * L], scalar=1.0 / DM,
                                     in1=stat2[:], op0=Alu.mult, op1=Alu.subtract)
            # rsqrt = exp(-0.5*ln(v+eps)): stays in the natural_log_exp ACT
            # table set (no Sqrt-set reload, no DVE reciprocal)
            ACT.activation(out=stat2[:], in_=stat2[:], func=Act.Ln, bias=1e-6)
            ACT.activation(out=stat[:, L:2 * L], in_=stat2[:], func=Act.Exp, scale=-0.5)
            bc = psB.tile([128, 512], f32, tag="bc", name=f"bc{l}")
            MM.matmul(out=bc[:], lhsT=C["ones_row"][:], rhs=stat[:, 0:512])
            for g in range(GM):
                DVE.tensor_tensor(out=tmp1[:, g * L:(g + 1) * L], in0=x3(g),
                                  in1=bc[:, 0:L], op=Alu.subtract)
                if scale_col is None:
                    DVE.tensor_tensor(out=xn3(g), in0=tmp1[:, g * L:(g + 1) * L],
                                      in1=bc[:, L:2 * L], op=Alu.mult)
                else:
                    DVE.tensor_tensor(out=tmp1[:, g * L:(g + 1) * L],
                                      in0=tmp1[:, g * L:(g + 1) * L],
                                      in1=bc[:, L:2 * L], op=Alu.mult)
                    DVE.scalar_tensor_tensor(
                        out=xn3(g), in0=tmp1[:, g * L:(g + 1) * L],
                        scalar=scale_col[:, g:g + 1],
                        in1=shift_col[:, g:g + 1].broadcast_to([128, L]),
                        op0=Alu.mult, op1=Alu.add)

        def ada_block(l, psM):
            """adaLN modulation matvecs for layer l -> mod_all/modx_all slices.

            Emitted two layers early so PE/DMA fill the scan window."""
            for r in range(2):
                for si, (lo, wdt) in enumerate(
                        ((0, 512), (512, 512), (1024, 512), (1536, 512), (2048, 256))):
                    ps = psM.tile([1, 512], f32, tag="m", name=f"mps{l}_{r}_{si}")
                    for k in range(6):
                        ah = ws.tile([128, 768], bf16, tag="ws", name=f"ada{l}_{r}_{si}_{k}")
                        DMA.dma_start(out=ah[:, 0:wdt],
                                      in_=W["adaT"][l, k][:, r * 2304 + lo:r * 2304 + lo + wdt])
                        MM.matmul(out=ps[:, 0:wdt], lhsT=silu_c[:, k:k + 1],
                                  rhs=ah[:, 0:wdt],
                                  start=(k == 0), stop=(k == 5))
                    sg = stg.tile([1, 512], f32, tag="stg", name=f"sg{l}_{r}_{si}")
                    ACT.activation(out=sg[:, 0:wdt], in_=ps[:, 0:wdt], func=Act.Copy)
                    GPS.dma_start(out=scr_mod[l:l + 1, r * 2304 + lo:r * 2304 + lo + wdt],
                                  in_=sg[:, 0:wdt])
            mf = mod_all[:, l * 36:(l + 1) * 36]
            GPS.dma_start(out=mf.rearrange("p (bl g) -> p bl g", bl=6),
                          in_=scr_mod[l, :].rearrange("(bl g p) -> p bl g", bl=6, g=6))
            DVE.tensor_tensor(out=mf, in0=mf,
                              in1=C["adab_sc"][:, l * 36:(l + 1) * 36], op=Alu.add)
            DVE.tensor_scalar_add(out=modx_all[:, l * 12:l * 12 + 6],
                                  in0=mod_all[:, l * 36 + 6:l * 36 + 12], scalar1=1.0)
            DVE.tensor_scalar_add(out=modx_all[:, l * 12 + 6:l * 12 + 12],
                                  in0=mod_all[:, l * 36 + 24:l * 36 + 30], scalar1=1.0)

        for l in range(2):
            with tc.tile_pool(name=f"psMp{l}", bufs=2, space="PSUM") as psM:
                ada_block(l, psM)

        for l in range(depth):
            mod_f = mod_all[:, l * 36:(l + 1) * 36]
            modx_l = modx_all[:, l * 12:(l + 1) * 12]

            # ---- mamba ----
            with tc.tile_pool(name=f"psA{l}", bufs=2, space="PSUM") as psA, \
                 tc.tile_pool(name=f"psS{l}", bufs=1, space="PSUM") as psS, \
                 tc.tile_pool(name=f"psB{l}", bufs=1, space="PSUM") as psB, \
                 tc.tile_pool(name=f"psV{l}", bufs=1, space="PSUM") as psV, \
                 tc.tile_pool(name=f"psM{l}", bufs=2, space="PSUM") as psM, \
                 tc.tile_pool(name=f"psX{l}", bufs=1, space="PSUM") as psX:
                ln_block(10 * l, psA, psS, psB, modx_l[:, 0:6], mod_f[:, 0:6])

                for mb in range(4):
                    wts = []
                    for k in range(6):
                        wi = ws.tile([128, 768], bf16, tag="ws", name=f"inw{l}_{mb}_{k}")
                        DMA.dma_start(out=wi[:], in_=W["in_wT"][l, k][:, mb * 768:(mb + 1) * 768])
                        wts.append(wi)
                    for mi in range(6):
                        m = mb * 6 + mi
                        ps = psA.tile([128, L], f32, tag="mm", name=f"ip{l}_{m}")
                        for k in range(6):
                            MM.matmul(out=ps[:], lhsT=wts[k][:, mi * 128:(mi + 1) * 128],
                                      rhs=xn3(k), start=(k == 0), stop=(k == 5))
                        if m < 12:
                            ACT.activation(out=up3[:, m, 4:SEG], in_=ps[:], func=Act.Copy)
                        else:
                            ACT.activation(out=z[:, (m - 12) * L:(m - 11) * L], in_=ps[:], func=Act.Silu)

                # ---- attention K/V: depend only on fl_e, so compute them here
                # so PE has work queued during the scan window ----
                wk_ = []
                for i in range(2):
                    t_ = wb.tile([128, 3 * NH * HP], bf16, tag="wb", name=f"wk{l}_{i}")
                    load_kpm(t_, W["wkT"][l, 3 * i:3 * i + 3], 3)
                    wk_.append(t_)
                for h in range(NH):
                    ps = psA.tile([128, L], f32, tag="mm", name=f"kp{l}_{h}")
                    for k in range(6):
                        MM.matmul(out=ps[:, 0:LT],
                                  lhsT=wk_[k // 3][:, (k % 3) * NH * HP + h * HP:(k % 3) * NH * HP + (h + 1) * HP],
                                  rhs=fl_e[:, k * LT:(k + 1) * LT], start=(k == 0), stop=(k == 5))
                    ACT.activation(out=k_sb[:, h, :], in_=ps[:, 0:LT], func=Act.Identity,
                                   bias=C["bk_sc"][:, l * 8 + h:l * 8 + h + 1])
                wv_ = []
                for i in range(2):
                    t_ = wb.tile([128, 3 * NH * HP], bf16, tag="wb", name=f"wv{l}_{i}")
                    load_kpm(t_, W["wvT"][l, 3 * i:3 * i + 3], 3)
                    wv_.append(t_)
                GPS.dma_start(out=bvb[:], in_=W["bv_pad"][l].partition_broadcast(LT))
                for fo in range(2):
                    vt_ps = psV.tile([LT, 512], f32, tag="vt", name=f"vtp{l}_{fo}")
                    for k in range(6):
                        MM.matmul(out=vt_ps[:],
                                  lhsT=fl_e[:, k * LT:(k + 1) * LT],
                                  rhs=wv_[k // 3][:, (k % 3) * NH * HP + fo * 512:(k % 3) * NH * HP + (fo + 1) * 512],
                                  start=(k == 0), stop=(k == 5))
                    DVE.tensor_tensor(out=vt_sb[:, fo * 512:(fo + 1) * 512],
                                      in0=vt_ps[:],
                                      in1=bvb[:, fo * 512:(fo + 1) * 512], op=Alu.add)

                # causal depthwise conv (acc in y, f32) + silu -> uc
                cw4 = C["convw_sc"][:].rearrange("p (a g k) -> p a g k", a=DEPTH, g=12)
                for k in range(4):
                    wkb = cw4[:, l, :, k].unsqueeze(2).broadcast_to([128, GI, L])
                    sh = up3[:, :, 1 + k:1 + k + L]
                    if k == 0:
                        GPS.tensor_tensor(out=y3, in0=sh, in1=wkb, op=Alu.mult)
                    else:
                        GPS.tensor_tensor(out=hh[0][:].rearrange("p (g s) -> p g s", g=GI)[:, :, 4:SEG],
                                          in0=sh, in1=wkb, op=Alu.mult)
                        GPS.tensor_tensor(out=y3, in0=y3,
                                          in1=hh[0][:].rearrange("p (g s) -> p g s", g=GI)[:, :, 4:SEG],
                                          op=Alu.add)
                for g in range(GI):
                    ACT.activation(out=uc3i[:, g, :], in_=y[:, g * L:(g + 1) * L],
                                   func=Act.Silu, bias=C["convb_sc"][:, l * 12 + g:l * 12 + g + 1])

                # xproj -> xdbl [80, L]
                xpw = wb.tile([128, 1152], bf16, tag="wb", name=f"xpw{l}")
                load_kpm(xpw, W["xprojT"][l], 12)
                xd = psX.tile([96, L], f32, tag="xd", name=f"xd{l}")
                for k in range(GI):
                    MM.matmul(out=xd[:], lhsT=xpw[:, k * 96:(k + 1) * 96],
                              rhs=uc3i[:, k, :], start=(k == 0), stop=(k == GI - 1))
                ACT.activation(out=xdbl_sb[:], in_=xd[0:48, :], func=Act.Copy)
                ACT.activation(out=bc_sb[:], in_=xd[64:96, :], func=Act.Copy)
                GPS.dma_start(out=scr_bc[0, :].rearrange("(a b) -> a b", a=32), in_=bc_sb[:])
                GPS.dma_start(out=brep[:], in_=scr_bc[0, 0:NST * L].partition_broadcast(128))
                GPS.dma_start(out=crep[:], in_=scr_bc[0, NST * L:2 * NST * L].partition_broadcast(128))

                # dt = softplus(dt_w @ xdbl[:48] + dt_b)
                dtw = wb.tile([48, DI], bf16, tag="wb", name=f"dtw{l}")
                DMA.dma_start(out=dtw[:], in_=W["dt_wT"][l])
                for m in range(GI):
                    ps = psA.tile([128, L], f32, tag="mm", name=f"dtp{l}_{m}")
                    MM.matmul(out=ps[:], lhsT=dtw[:, m * 128:(m + 1) * 128], rhs=xdbl_sb[:])
                    sl_ = dt_t[:, m * L:(m + 1) * L]
                    ACT.activation(out=sl_, in_=ps[:], func=Act.Exp,
                                   bias=C["dtb_sc"][:, l * 12 + m:l * 12 + m + 1])
                    DVE.tensor_scalar_add(out=sl_, in0=sl_, scalar1=1.0)
                    ACT.activation(out=sl_, in_=sl_, func=Act.Ln)
                DVE.tensor_tensor(out=du3, in0=dt3, in1=uc3i, op=Alu.mult)

                # modulation matvecs for layer l+2: PE+DMA work that overlaps
                # the DVE-bound scan below
                if l + 2 < depth:
                    ada_block(l + 2, psM)

                # selective scan over n (state dim), pad cols reset state.
                # DVE keeps the scan itself; the C-mults go mostly to GpSimd
                # and the y-accumulation alternates DVE/GPS so no single
                # engine serializes the whole n loop.
                y_odd3 = y_odd[:].rearrange("p (g t) -> p g t", g=GI)
                GPS.memset(y_odd[:], 0.0)
                for n in range(NST):
                    sl = n % 2
                    dAi = dA[sl][:].rearrange("p (g s) -> p g s", g=GI)[:, :, 4:SEG]
                    dBi = dB[sl][:].rearrange("p (g s) -> p g s", g=GI)[:, :, 4:SEG]
                    hi = hh[sl][:].rearrange("p (g s) -> p g s", g=GI)[:, :, 4:SEG]
                    ACT.activation(out=dAi, in_=dt3, func=Act.Exp,
                                   scale=aneg[:, l * 192 + n:l * 192 + n + 1])
                    DVE.tensor_tensor(out=dBi, in0=du3,
                                      in1=brep[:, n * L:(n + 1) * L].unsqueeze(1).broadcast_to([128, GI, L]),
                                      op=Alu.mult)
                    DVE.tensor_tensor_scan(out=hh[sl][:], data0=dA[sl][:], data1=dB[sl][:],
                                           initial=0.0, op0=Alu.mult, op1=Alu.add)
                    ceng = DVE if n % 4 == 3 else GPS
                    ceng.tensor_tensor(out=hi, in0=hi,
                                       in1=crep[:, n * L:(n + 1) * L].unsqueeze(1).broadcast_to([128, GI, L]),
                                       op=Alu.mult)
                    if n == 0:
                        DVE.tensor_copy(out=y3, in_=hi)
                    elif n % 2 == 0:
                        DVE.tensor_tensor(out=y3, in0=y3, in1=hi, op=Alu.add)
                    else:
                        GPS.tensor_tensor(out=y_odd3, in0=y_odd3, in1=hi, op=Alu.add)

                # y = (y + y_odd + uc*Dp) * silu(z);  out-proj; residual with gm
                DVE.tensor_tensor(out=y[:], in0=y[:], in1=y_odd[:], op=Alu.add)
                for g in range(GI):
                    DVE.scalar_tensor_tensor(out=y[:, g * L:(g + 1) * L], in0=uc3i[:, g, :],
                                             scalar=C["Dp_sc"][:, l * 12 + g:l * 12 + g + 1],
                                             in1=y[:, g * L:(g + 1) * L], op0=Alu.mult, op1=Alu.add)
                DVE.tensor_tensor(out=du[:], in0=y[:], in1=z[:], op=Alu.mult)

                ow = []
                for i in range(2):
                    wi = wb.tile([128, 6 * DM], bf16, tag="wb", name=f"ow{l}_{i}")
                    load_kpm(wi, W["out_wT"][l, 6 * i:6 * i + 6], 6)
                    ow.append(wi)
                for m in range(GM):
                    ps = psA.tile([128, L], f32, tag="mm", name=f"op{l}_{m}")
                    for k in range(GI):
                        MM.matmul(out=ps[:], lhsT=ow[k // 6][:, (k % 6) * DM + m * 128:(k % 6) * DM + (m + 1) * 128],
                                  rhs=du[:, k * L:(k + 1) * L], start=(k == 0), stop=(k == GI - 1))
                    DVE.scalar_tensor_tensor(out=x3(m), in0=ps[:], scalar=mod_f[:, 12 + m:13 + m],
                                             in1=x3(m), op0=Alu.mult, op1=Alu.add)

            # ---- cross attention ----
            with tc.tile_pool(name=f"qsA{l}", bufs=2, space="PSUM") as psA, \
                 tc.tile_pool(name=f"qsS{l}", bufs=1, space="PSUM") as psS, \
                 tc.tile_pool(name=f"qsB{l}", bufs=1, space="PSUM") as psB, \
                 tc.tile_pool(name=f"qsC{l}", bufs=1, space="PSUM") as psC, \
                 tc.tile_pool(name=f"qsP{l}", bufs=1, space="PSUM") as psP:
                ln_block(10 * l + 1, psA, psS, psB, None, None)

                wq = []
                for i in range(2):
                    t_ = wb.tile([128, 3 * NH * HP], bf16, tag="wb", name=f"wq{l}_{i}")
                    load_kpm(t_, W["wqT"][l, 3 * i:3 * i + 3], 3)
                    wq.append(t_)
                for h in range(NH):
                    ps = psA.tile([128, L], f32, tag="mm", name=f"qp{l}_{h}")
                    for k in range(6):
                        MM.matmul(out=ps[:], lhsT=wq[k // 3][:, (k % 3) * NH * HP + h * HP:(k % 3) * NH * HP + (h + 1) * HP],
                                  rhs=xn3(k), start=(k == 0), stop=(k == 5))
                    ACT.activation(out=q_sb[:, h, :], in_=ps[:], func=Act.Identity,
                                   bias=C["bq_sc"][:, l * 8 + h:l * 8 + h + 1])

                for h in range(NH):
                    for tc2 in range(2):
                        idx = h * 2 + tc2
                        sc_ps = psC.tile([128, LT], f32, tag="sc", name=f"scp{l}_{idx}")
                        MM.matmul(out=sc_ps[:], lhsT=q_sb[:, h, tc2 * 128:(tc2 + 1) * 128],
                                  rhs=k_sb[:, h, :])
                        mx = small[:, 2:3]
                        DVE.tensor_reduce(out=mx, in_=sc_ps[:], axis=mybir.AxisListType.X, op=Alu.max)
                        DVE.tensor_scalar_mul(out=small[:, 3:4], in0=mx, scalar1=-SQ)
                        ACT.activation(out=p_all[:, idx * LT:(idx + 1) * LT], in_=sc_ps[:],
                                       func=Act.Exp, scale=SQ, bias=small[:, 3:4],
                                       accum_out=rs_all[:, idx:idx + 1])
                DVE.reciprocal(out=ri_all[:], in_=rs_all[:])
                for h in range(NH):
                    for tc2 in range(2):
                        idx = h * 2 + tc2
                        DVE.tensor_scalar_mul(out=p_all[:, idx * LT:(idx + 1) * LT],
                                              in0=p_all[:, idx * LT:(idx + 1) * LT],
                                              scalar1=ri_all[:, idx:idx + 1])
                        pt_ps = psP.tile([LT, 128], bf16, tag="pt", name=f"ptp{l}_{idx}")
                        MM.transpose(out=pt_ps[:], in_=p_all[:, idx * LT:(idx + 1) * LT], identity=C["id128"][:])
                        ACT.activation(out=pt_sb[:, h * L + tc2 * 128:h * L + (tc2 + 1) * 128],
                                       in_=pt_ps[:], func=Act.Copy)
                for h in range(NH):
                    ps = psA.tile([128, L], f32, tag="mm", name=f"avp{l}_{h}")
                    for tc2 in range(2):
                        MM.matmul(out=ps[:, tc2 * 128:(tc2 + 1) * 128],
                                  lhsT=vt_sb[:, h * HP:(h + 1) * HP],
                                  rhs=pt_sb[:, h * L + tc2 * 128:h * L + (tc2 + 1) * 128])
                    ACT.activation(out=avt_sb[:, h, :], in_=ps[:], func=Act.Copy)

                ao = []
                for i in range(2):
                    t_ = wb.tile([128, 4 * DM], bf16, tag="wb", name=f"ao{l}_{i}")
                    load_kpm(t_, W["aoT"][l, 4 * i:4 * i + 4], 4)
                    ao.append(t_)
                for m in range(GM):
                    ps = psA.tile([128, L], f32, tag="mm", name=f"aop{l}_{m}")
                    for k in range(NH):
                        MM.matmul(out=ps[:], lhsT=ao[k // 4][:, (k % 4) * DM + m * 128:(k % 4) * DM + (m + 1) * 128],
                                  rhs=avt_sb[:, k, :], start=(k == 0), stop=(k == NH - 1))
                    DVE.scalar_tensor_tensor(out=x3(m), in0=ps[:], scalar=C["aob_sc"][:, l * 6 + m:l * 6 + m + 1],
                                             in1=x3(m), op0=Alu.add, op1=Alu.add)

            # ---- FFN ----
            with tc.tile_pool(name=f"fsA{l}", bufs=2, space="PSUM") as psA, \
                 tc.tile_pool(name=f"fsS{l}", bufs=1, space="PSUM") as psS, \
                 tc.tile_pool(name=f"fsB{l}", bufs=1, space="PSUM") as psB:
                ln_block(10 * l + 2, psA, psS, psB, modx_l[:, 6:12], mod_f[:, 18:24])
                for half in range(2):
                    for mb in range(2):
                        wts = []
                        for k in range(6):
                            wi = ws.tile([128, 768], bf16, tag="ws", name=f"f1w{l}_{half}_{mb}_{k}")
                            DMA.dma_start(out=wi[:],
                                          in_=W["fw1T"][l, k][:, (half * 2 + mb) * 768:(half * 2 + mb + 1) * 768])
                            wts.append(wi)
                        for mi in range(6):
                            m = half * 12 + mb * 6 + mi
                            ml = mb * 6 + mi
                            ps = psA.tile([128, L], f32, tag="mm", name=f"f1p{l}_{m}")
                            for k in range(6):
                                MM.matmul(out=ps[:], lhsT=wts[k][:, mi * 128:(mi + 1) * 128],
                                          rhs=xn3(k), start=(k == 0), stop=(k == 5))
                            ACT.activation(out=hffn[:, ml * L:(ml + 1) * L], in_=ps[:], func=Act.Gelu,
                                           bias=C["fb1_sc"][:, l * 24 + m:l * 24 + m + 1])
                    f2 = []
                    for i in range(2):
                        t_ = wb.tile([128, 6 * DM], bf16, tag="wb", name=f"f2{l}_{half}_{i}")
                        load_kpm(t_, W["fw2T"][l, half * 12 + 6 * i:half * 12 + 6 * i + 6], 6)
                        f2.append(t_)
                    for m in range(GM):
                        ps = psA.tile([128, L], f32, tag="mm", name=f"f2p{l}_{half}_{m}")
                        for k in range(12):
                            MM.matmul(out=ps[:], lhsT=f2[k // 6][:, (k % 6) * DM + m * 128:(k % 6) * DM + (m + 1) * 128],
                                      rhs=hffn[:, k * L:(k + 1) * L], start=(k == 0), stop=(k == 11))
                        if half == 0:
                            ACT.activation(out=tmp1[:, m * 256:(m + 1) * 256], in_=ps[:], func=Act.Copy)
                        else:
                            DVE.tensor_tensor(out=tmp1[:, m * 256:(m + 1) * 256],
                                              in0=tmp1[:, m * 256:(m + 1) * 256], in1=ps[:], op=Alu.add)
                            DVE.tensor_scalar(out=tmp1[:, m * 256:(m + 1) * 256],
                                              in0=tmp1[:, m * 256:(m + 1) * 256],
                                              scalar1=C["fb2_sc"][:, l * 6 + m:l * 6 + m + 1],
                                              scalar2=mod_f[:, 30 + m:31 + m], op0=Alu.add, op1=Alu.mult)
                            DVE.tensor_tensor(out=x3(m), in0=x3(m), in1=tmp1[:, m * 256:(m + 1) * 256], op=Alu.add)

        # ---------------- final ----------------
        with tc.tile_pool(name="fin", bufs=1) as fin, \
             tc.tile_pool(name="zsA", bufs=2, space="PSUM") as psA, \
             tc.tile_pool(name="zsS", bufs=1, space="PSUM") as psS, \
             tc.tile_pool(name="zsB", bufs=1, space="PSUM") as psB, \
             tc.tile_pool(name="zsV", bufs=3, space="PSUM") as psV:
            fm_ps = [psV.tile([1, 512], f32, tag="fm5", name=f"fmps{s}") for s in range(3)]
            for k in range(6):
                fad = fin.tile([128, 2 * DM], bf16, tag="fw", name=f"fad{k}")
                load_kpm(fad, W["finadaT"][k:k + 1], 1)
                for s in range(3):
                    MM.matmul(out=fm_ps[s][:],
                              lhsT=silu_c[:, k:k + 1],
                              rhs=fad[:, s * 512:(s + 1) * 512],
                              start=(k == 0), stop=(k == 5))
            for s in range(3):
                sg = stg.tile([1, 512], f32, tag="stg", name=f"fsg{s}")
                ACT.activation(out=sg[:], in_=fm_ps[s][:], func=Act.Copy)
                GPS.dma_start(out=scr_fm[:, s * 512:(s + 1) * 512], in_=sg[:])
            GPS.dma_start(out=fmod_sc[:].rearrange("p (bl g) -> p bl g", bl=2),
                          in_=scr_fm[0, :].rearrange("(bl g p) -> p bl g", bl=2, g=6))
            DVE.tensor_tensor(out=fmod_sc[:], in0=fmod_sc[:], in1=C["finadab_sc"][:], op=Alu.add)
            DVE.tensor_scalar_add(out=modx[:, 0:6], in0=fmod_sc[:, 6:12], scalar1=1.0)
            ln_block(999, psA, psS, psB, modx[:, 0:6], fmod_sc[:, 0:6])

            xo_sb = fin.tile([128, 2, CIN], f32, tag="fxo")
            outT = fin.tile([128, 2 * CIN], f32, tag="fot")
            fw = fin.tile([128, 6 * CIN], bf16, tag="fw2")
            load_kpm(fw, W["finT"][:], 6)
            for m in range(2):
                ps = psA.tile([128, L], f32, tag="mm", name=f"fop{m}")
                for k in range(6):
                    MM.matmul(out=ps[:], lhsT=fw[:, k * CIN + m * 128:k * CIN + (m + 1) * 128],
                              rhs=xn3(k), start=(k == 0), stop=(k == 5))
                ACT.activation(out=xo_sb[:, m, :], in_=ps[:], func=Act.Identity,
                               bias=C["finb_sc"][:, m:m + 1])
            # transpose [ch, t] -> [t, ch] and store
            for tc2 in range(2):
                for m in range(2):
                    tp = psA.tile([128, 128], f32, tag="mm", name=f"tp{tc2}_{m}")
                    MM.transpose(out=tp[:], in_=xo_sb[:, m, tc2 * 128:(tc2 + 1) * 128],
                                 identity=C["id128f"][:])
                    ACT.activation(out=outT[:, tc2 * CIN + m * 128:tc2 * CIN + (m + 1) * 128],
                                   in_=tp[:], func=Act.Copy)
            GPS.dma_start(out=out_d[:].rearrange("(a p) c -> p a c", a=2),
                          in_=outT[:].rearrange("p (a c) -> p a c", a=2))
    nc.finalize()
    # walrus' verifier rejects leftover unused framework registers with
    # reg_id=-1; give each a harmless unique id per engine.
    from collections import defaultdict
    nxt = defaultdict(int)
    for fn in nc.m.functions:
        for a in fn.allocations:
            if getattr(a, "reg_id", None) == -1:
                eng = str(getattr(a, "engine", "?"))
                n = getattr(a, "num_physical_regs", None) or 1
                if n > 1 and nxt[eng] % 2:
                    nxt[eng] += 1
                a.reg_id = nxt[eng]
                nxt[eng] += n
    return nc


_CACHE = {}


def kernel(**inputs):
    depth = DEPTH
    if "nc" not in _CACHE:
        _CACHE["nc"] = build_nc(depth)
    nc = _CACHE["nc"]
    shared = prep_shared(inputs)
    in_maps = []
    for b in range(N_CORES):
        m = dict(shared)
        m.update(prep_core(inputs, b))
        in_maps.append(m)
    res = run_bass_kernel_spmd(nc, in_maps, list(range(N_CORES)))
    out = np.stack([np.asarray(res.results[b]["out"], np.float32) for b in range(N_CORES)])
    return out


def kernel_profiled(**inputs):
    if "nc" not in _CACHE:
        _CACHE["nc"] = build_nc(DEPTH)
    nc = _CACHE["nc"]
    shared = prep_shared(inputs)
    in_maps = []
    for b in range(N_CORES):
        m = dict(shared)
        m.update(prep_core(inputs, b))
        in_maps.append(m)
    res = run_bass_kernel_spmd(nc, in_maps, list(range(N_CORES)), trace=True)
    out = np.stack([np.asarray(res.results[b]["out"], np.float32) for b in range(N_CORES)])
    return out, res.exec_time_ns



# revision 18
# speedup vs baseline: 1.1676x; 1.1676x over previous
import sys

sys.path.insert(0, "/opt/trn_rl_repo")
import math

import numpy as np
import ml_dtypes

from concourse import bass, bacc, mybir
from concourse import tile
from concourse.bass_utils import run_bass_kernel_spmd

BF = ml_dtypes.bfloat16
bf16 = mybir.dt.bfloat16
f32 = mybir.dt.float32
Alu = mybir.AluOpType
Act = mybir.ActivationFunctionType

B, L, CIN, COND, DM, DEPTH = 8, 256, 256, 2048, 768, 12
NST, DCONV, DI, DTR = 16, 4, 1536, 48
NH, HD, LT, FREQ = 8, 96, 35, 256
GM, GI = DM // 128, DI // 128          # 6, 12
SEG = L + 4                            # 260, 4 zero pad cols reset scan state
SCANW = GI * SEG                       # 3120
HP = 128                               # padded head dim
SQ = 1.0 / math.sqrt(HD)
N_CORES = 8


def _bf(a):
    return np.ascontiguousarray(a, dtype=np.float32).astype(BF)


def _f(a):
    return np.ascontiguousarray(a, dtype=np.float32)


def prep_shared(inp):
    """Host-side layout/dtype staging of the weights (shared by all cores)."""
    d = {}
    d["xwT"] = _bf(inp["xw"].T.reshape(2, 128, DM))
    d["tw1T"] = _bf(inp["tw1"].T.reshape(2, 128, DM))
    d["tw2T"] = _bf(inp["tw2"].T.reshape(6, 128, DM))
    d["fcw1T"] = _bf(inp["fcw1"].T.reshape(16, 128, DM))
    d["fcw2T"] = _bf(inp["fcw2"].T.reshape(6, 128, DM))
    d["flwT"] = _bf(inp["flw"].T.reshape(16, 128, DM))
    d["adaT"] = _bf(np.ascontiguousarray(inp["ada_w"].transpose(0, 2, 1)).reshape(DEPTH, 6, 128, 6 * DM))
    d["in_wT"] = _bf(np.ascontiguousarray(inp["in_w"].transpose(0, 2, 1)).reshape(DEPTH, 6, 128, 2 * DI))
    xpt = np.ascontiguousarray(inp["xproj_w"].transpose(0, 2, 1)).astype(np.float32)  # [12,1536,80]
    xpp = np.zeros((DEPTH, DI, 96), np.float32)
    xpp[:, :, 0:48] = xpt[:, :, 0:48]
    xpp[:, :, 64:96] = xpt[:, :, 48:80]
    d["xprojT"] = _bf(xpp.reshape(DEPTH, 12, 128, 96))
    d["dt_wT"] = _bf(np.ascontiguousarray(inp["dt_w"].transpose(0, 2, 1)))          # [12,48,1536]
    d["out_wT"] = _bf(np.ascontiguousarray(inp["out_w"].transpose(0, 2, 1)).reshape(DEPTH, 12, 128, DM))
    qkv = inp["qkv_w"]
    wq, wk, wv = qkv[:, :DM], qkv[:, DM:2 * DM], qkv[:, 2 * DM:]
    for nm, w in (("wqT", wq), ("wkT", wk), ("wvT", wv)):
        wt = np.ascontiguousarray(w.transpose(0, 2, 1))                              # [12,768,768]
        pad = np.zeros((DEPTH, DM, NH * HP), np.float32)
        for h in range(NH):
            pad[:, :, h * HP:h * HP + HD] = wt[:, :, h * HD:(h + 1) * HD]
        d[nm] = _bf(pad.reshape(DEPTH, 6, 128, NH * HP))
    aot = np.ascontiguousarray(inp["ao_w"].transpose(0, 2, 1))                       # [12,768(dv),768]
    aop = np.zeros((DEPTH, NH * HP, DM), np.float32)
    for h in range(NH):
        aop[:, h * HP:h * HP + HD] = aot[:, h * HD:(h + 1) * HD]
    d["aoT"] = _bf(aop.reshape(DEPTH, 8, 128, DM))
    d["fw1T"] = _bf(np.ascontiguousarray(inp["fw1"].transpose(0, 2, 1)).reshape(DEPTH, 6, 128, 4 * DM))
    d["fw2T"] = _bf(np.ascontiguousarray(inp["fw2"].transpose(0, 2, 1)).reshape(DEPTH, 24, 128, DM))
    d["finadaT"] = _bf(inp["fin_ada_w"].T.reshape(6, 128, 2 * DM))
    d["finT"] = _bf(inp["fin_w"].T.reshape(6, 128, CIN))

    # per-partition scatters (fp32), layout [128, ...]
    d["xb_sc"] = _f(inp["xb"].reshape(6, 128).T)
    d["flb_sc"] = _f(inp["flb"].reshape(6, 128).T)
    d["tb1_r"] = _f(inp["tb1"].reshape(1, DM))
    d["tb2_r"] = _f(inp["tb2"].reshape(1, DM))
    d["fcb1_r"] = _f(inp["fcb1"].reshape(1, DM))
    d["fcb2_r"] = _f(inp["fcb2"].reshape(1, DM))
    d["flpos_sc"] = _f(np.ascontiguousarray(inp["flpos"][0].T).reshape(6, 128, LT).transpose(1, 0, 2))
    d["dtb_sc"] = _f(inp["dt_b"].reshape(DEPTH, 12, 128).transpose(2, 0, 1).reshape(128, -1))
    d["convw_sc"] = _f(inp["conv_w"].reshape(DEPTH, 12, 128, 4).transpose(2, 0, 1, 3).reshape(128, -1))
    d["convb_sc"] = _f(inp["conv_b"].reshape(DEPTH, 12, 128).transpose(2, 0, 1).reshape(128, -1))
    d["Dp_sc"] = _f(inp["Dp"].reshape(DEPTH, 12, 128).transpose(2, 0, 1).reshape(128, -1))
    d["alog_sc"] = _f(inp["A_log"].reshape(DEPTH, 12, 128, NST).transpose(2, 0, 1, 3).reshape(128, -1))
    d["adab_sc"] = _f(inp["ada_b"].reshape(DEPTH, 6, 6, 128).transpose(3, 0, 1, 2).reshape(128, -1))
    qb = inp["qkv_b"]
    for nm, bias in (("bq_sc", qb[:, :DM]), ("bk_sc", qb[:, DM:2 * DM])):
        arr = np.zeros((DEPTH, NH, HP), np.float32)
        arr[:, :, :HD] = np.asarray(bias, np.float32).reshape(DEPTH, NH, HD)
        d[nm] = _f(arr.transpose(2, 0, 1).reshape(128, -1))
    bv = np.zeros((DEPTH, NH, HP), np.float32)
    bv[:, :, :HD] = np.asarray(qb[:, 2 * DM:], np.float32).reshape(DEPTH, NH, HD)
    d["bv_pad"] = _bf(bv.reshape(DEPTH, NH * HP))
    d["aob_sc"] = _f(inp["ao_b"].reshape(DEPTH, 6, 128).transpose(2, 0, 1).reshape(128, -1))
    d["fb1_sc"] = _f(inp["fb1"].reshape(DEPTH, 24, 128).transpose(2, 0, 1).reshape(128, -1))
    d["fb2_sc"] = _f(inp["fb2"].reshape(DEPTH, 6, 128).transpose(2, 0, 1).reshape(128, -1))
    d["finadab_sc"] = _f(inp["fin_ada_b"].reshape(2, 6, 128).transpose(2, 0, 1).reshape(128, 12))
    d["finb_sc"] = _f(inp["fin_b"].reshape(2, 128).T)

    # constants (input independent)
    d["id128"] = _bf(np.eye(128))
    d["id128f"] = _f(np.eye(128))
    d["ones_col"] = _f(np.ones((128, 1)))
    d["ones_colb"] = _bf(np.ones((128, 1)))
    d["ones_row"] = _f(np.ones((1, 128)))
    half = FREQ // 2
    return d


def prep_core(inp, b):
    d = {}
    d["xT"] = _bf(np.asarray(inp["x"][b], np.float32).T.reshape(2, 128, L))
    half = FREQ // 2
    fr = np.exp(-math.log(10000.0) * np.arange(half) / half).reshape(128, 1)
    d["ftp"] = _f(np.concatenate([fr, np.full((128, 1), np.asarray(inp["t"][b], np.float32))], 1))
    d["fc_cols"] = _bf(np.asarray(inp["fc"][b], np.float32).reshape(16, 128).T)
    d["flT"] = _bf(np.asarray(inp["fl"][b], np.float32).T.reshape(16, 128, LT))
    return d


def build_nc(depth=DEPTH):
    nc = bacc.Bacc(None)
    for val in (math.pi / 2, 1e-6, -math.pi):
        t_ = nc.alloc_sbuf_tensor(f"const-f32-{val}", [128, 1], f32)
        nc.gpsimd.memset(t_.ap(), val)
        nc.const_aps.aps[(f32, val)] = t_.ap()
    nc.all_engine_barrier()
    P = nc.declare_dram_parameter

    W = {}
    for nm, shp, dt in [
        ("xwT", [2, 128, DM], bf16), ("tw1T", [2, 128, DM], bf16),
        ("tw2T", [6, 128, DM], bf16), ("fcw1T", [16, 128, DM], bf16),
        ("fcw2T", [6, 128, DM], bf16), ("flwT", [16, 128, DM], bf16),
        ("adaT", [DEPTH, 6, 128, 6 * DM], bf16), ("in_wT", [DEPTH, 6, 128, 2 * DI], bf16),
        ("xprojT", [DEPTH, 12, 128, 96], bf16), ("dt_wT", [DEPTH, 48, DI], bf16),
        ("out_wT", [DEPTH, 12, 128, DM], bf16),
        ("wqT", [DEPTH, 6, 128, NH * HP], bf16), ("wkT", [DEPTH, 6, 128, NH * HP], bf16),
        ("wvT", [DEPTH, 6, 128, NH * HP], bf16), ("aoT", [DEPTH, 8, 128, DM], bf16),
        ("fw1T", [DEPTH, 6, 128, 4 * DM], bf16), ("fw2T", [DEPTH, 24, 128, DM], bf16),
        ("finadaT", [6, 128, 2 * DM], bf16), ("finT", [6, 128, CIN], bf16),
        ("xb_sc", [128, 6], f32), ("flb_sc", [128, 6], f32),
        ("tb1_r", [1, DM], f32), ("tb2_r", [1, DM], f32),
        ("fcb1_r", [1, DM], f32), ("fcb2_r", [1, DM], f32),
        ("flpos_sc", [128, 6, LT], f32),
        ("dtb_sc", [128, DEPTH * 12], f32), ("convw_sc", [128, DEPTH * 48], f32),
        ("convb_sc", [128, DEPTH * 12], f32), ("Dp_sc", [128, DEPTH * 12], f32),
        ("alog_sc", [128, DEPTH * 192], f32), ("adab_sc", [128, DEPTH * 36], f32),
        ("bq_sc", [128, DEPTH * 8], f32), ("bk_sc", [128, DEPTH * 8], f32),
        ("bv_pad", [DEPTH, NH * HP], bf16),
        ("aob_sc", [128, DEPTH * 6], f32), ("fb1_sc", [128, DEPTH * 24], f32),
        ("fb2_sc", [128, DEPTH * 6], f32), ("finadab_sc", [128, 12], f32),
        ("finb_sc", [128, 2], f32),
        ("id128", [128, 128], bf16), ("id128f", [128, 128], f32),
        ("ones_col", [128, 1], f32), ("ones_colb", [128, 1], bf16), ("ones_row", [1, 128], f32),
        ("xT", [2, 128, L], bf16), ("ftp", [128, 2], f32),
        ("fc_cols", [128, 16], bf16), ("flT", [16, 128, LT], bf16),
    ]:
        W[nm] = P(nm, shp, dt, isOutput=False)
    out_d = P("out", [L, CIN], f32, isOutput=True)
    scr_b1 = P("scr_b1", [1, DM], f32, isOutput=True)
    scr_b2 = P("scr_b2", [1, DM], f32, isOutput=True)
    scr_b3 = P("scr_b3", [1, DM], f32, isOutput=True)
    scr_mod = P("scr_mod", [DEPTH, 6 * DM], f32, isOutput=True)
    scr_bc = P("scr_bc", [1, 2 * NST * L], bf16, isOutput=True)
    scr_fm = P("scr_fm", [1, 2 * DM], f32, isOutput=True)

    MM, ACT, DVE, GPS, DMA = nc.tensor, nc.scalar, nc.vector, nc.gpsimd, nc.sync

    def g3(ap, n=GI, w=None):
        return ap.rearrange("p (g t) -> p g t", g=n)

    def load_kpm(dst, srcap, kdim):
        DMA.dma_start(out=dst[:].rearrange("p (k m) -> p k m", k=kdim),
                      in_=srcap.rearrange("k p m -> p k m"))

    with tile.TileContext(nc) as tc:
      from contextlib import ExitStack
      with ExitStack() as top:
        cp = top.enter_context(tc.tile_pool(name="cp", bufs=1))
        stg = top.enter_context(tc.tile_pool(name="stg", bufs=2))
        wb = top.enter_context(tc.tile_pool(name="wb", bufs=3))
        ws = top.enter_context(tc.tile_pool(name="ws", bufs=8))

        # ---- persistent SBUF state ----
        x = cp.tile([128, GM * L], f32)
        u_pad = cp.tile([128, SCANW], bf16)
        uc = cp.tile([128, SCANW], bf16)
        z = cp.tile([128, GI * L], bf16)
        dt_t = cp.tile([128, GI * L], bf16)
        du = cp.tile([128, GI * L], bf16)
        brep = cp.tile([128, NST * L], bf16)
        crep = cp.tile([128, NST * L], bf16)
        dA = [cp.tile([128, SCANW], bf16, name="dA0")] * 2
        dB = [cp.tile([128, SCANW], bf16, name=f"dB{i}") for i in range(2)]
        hh = [cp.tile([128, SCANW], bf16, name=f"hh{i}") for i in range(2)]
        y = cp.tile([128, GI * L], bf16)
        tmp1 = cp.tile([128, GM * L], bf16)
        xn = cp.tile([128, GM * L], bf16)
        hffn = cp.tile([128, 12 * L], bf16)
        q_sb = cp.tile([128, NH, L], bf16)
        k_sb = cp.tile([128, NH, LT], bf16)
        vt_sb = cp.tile([LT, NH * HP], bf16)
        pt_sb = cp.tile([LT, NH * L], bf16)
        p_all = cp.tile([128, NH * 2 * LT], bf16)
        rs_all = cp.tile([128, NH * 2], f32)
        ri_all = cp.tile([128, NH * 2], f32)
        avt_sb = cp.tile([128, NH, L], bf16)
        mod_all = cp.tile([128, DEPTH * 36], f32)
        modx_all = cp.tile([128, DEPTH * 12], f32)
        xdbl_sb = cp.tile([48, L], bf16)
        bc_sb = cp.tile([32, L], bf16)
        bvb = cp.tile([LT, NH * HP], bf16)
        modx = cp.tile([128, 12], f32)
        aneg = cp.tile([128, DEPTH * 192], f32)
        fl_e = cp.tile([128, GM * LT], bf16)
        silu_c = cp.tile([128, 6], bf16)
        stat = cp.tile([1, 2 * L], f32)
        stat2 = cp.tile([1, L], f32)
        small = cp.tile([128, 16], f32)      # ang etc
        smalli = cp.tile([128, 2], mybir.dt.int32)
        temb_c = cp.tile([128, 2], bf16)
        cvec = cp.tile([1, DM], f32)
        fmod_sc = cp.tile([128, 12], f32)

        # consts / biases resident
        C = {}
        for nm in ["dtb_sc", "convw_sc", "convb_sc", "Dp_sc", "adab_sc",
                   "bq_sc", "bk_sc", "aob_sc", "fb1_sc", "fb2_sc", "finadab_sc", "finb_sc",
                   "id128", "id128f", "ones_col", "ones_colb", "ones_row"]:
            C[nm] = cp.tile(list(W[nm].shape), W[nm].dtype, name="c_" + nm)
            DMA.dma_start(out=C[nm][:], in_=W[nm][:])

        # zero the pad columns once; interiors are always written strided
        for tl in dA + dB + [u_pad]:
            GPS.memset(tl[:], 0.0)


        # ---------------- preamble ----------------
        with tc.tile_pool(name="pre", bufs=1) as pre:
            from contextlib import ExitStack as _ES
            _es = _ES()
            psv = _es.enter_context(tc.tile_pool(name="psv", bufs=2, space="PSUM"))
            for nm in ["xb_sc", "flb_sc", "flpos_sc", "ftp", "fc_cols"]:
                C[nm] = pre.tile(list(W[nm].shape), W[nm].dtype, tag="p_" + nm, name="c_" + nm)
                DMA.dma_start(out=C[nm][:], in_=W[nm][:])
            for nm in ["tb1_r", "fcb1_r", "tb2_r", "fcb2_r"]:
                C[nm] = pre.tile(list(W[nm].shape), W[nm].dtype, tag="pvb", name="c_" + nm)
                DMA.dma_start(out=C[nm][:], in_=W[nm][:])
            for i in range(8):
                alg = pre.tile([128, 288], f32, tag="pal", name=f"alg{i}")
                DMA.dma_start(out=alg[:], in_=W["alog_sc"][:, i * 288:(i + 1) * 288])
                ACT.activation(out=aneg[:, i * 288:(i + 1) * 288], in_=alg[:], func=Act.Exp)
            DVE.tensor_scalar_mul(out=aneg[:], in0=aneg[:], scalar1=-1.0)
            # time embedding: ang = t*freqs mod 2pi; temb = [cos ang, sin ang]
            DVE.tensor_tensor(out=small[:, 0:1], in0=C["ftp"][:, 0:1], in1=C["ftp"][:, 1:2], op=Alu.mult)
            TWO_PI = 2 * math.pi
            # cos(ang)=sin(ang+pi/2); reduce each argument into [-pi, pi]
            DVE.tensor_scalar_add(out=small[:, 1:2], in0=small[:, 0:1], scalar1=math.pi / 2)
            for j, col in ((0, 1), (1, 0)):  # j=0: cos arg; j=1: sin arg
                src_c = 1 - col  # small col holding the argument
                a_ = small[:, src_c + 0:src_c + 1] if False else None
            for j, srccol in ((0, 1), (1, 0)):
                arg = small[:, srccol:srccol + 1]
                DVE.tensor_scalar_mul(out=small[:, 4 + j:5 + j], in0=arg, scalar1=1.0 / TWO_PI)
                DVE.tensor_copy(out=smalli[:, j:j + 1], in_=small[:, 4 + j:5 + j])
                DVE.tensor_copy(out=small[:, 6 + j:7 + j], in_=smalli[:, j:j + 1])
                DVE.scalar_tensor_tensor(out=small[:, 8 + j:9 + j], in0=small[:, 6 + j:7 + j],
                                         scalar=-TWO_PI, in1=arg, op0=Alu.mult, op1=Alu.add)
                DVE.tensor_scalar(out=small[:, 10 + j:11 + j], in0=small[:, 8 + j:9 + j],
                                  scalar1=math.pi, scalar2=None, op0=Alu.is_gt)
                DVE.scalar_tensor_tensor(out=small[:, 12 + j:13 + j], in0=small[:, 10 + j:11 + j],
                                         scalar=-TWO_PI, in1=small[:, 8 + j:9 + j],
                                         op0=Alu.mult, op1=Alu.add)
                ACT.activation(out=temb_c[:, j:j + 1], in_=small[:, 12 + j:13 + j], func=Act.Sin)

            tw1 = wb.tile([128, 2 * DM], bf16, tag="wb")
            load_kpm(tw1, W["tw1T"][:], 2)
            h1p = psv.tile([1, DM], f32, tag="vec")
            for k in range(2):
                for lo, hi in ((0, 512), (512, 768)):
                    MM.matmul(out=h1p[:, lo:hi],
                              lhsT=temb_c[:, k:k + 1],
                              rhs=tw1[:, k * DM + lo:k * DM + hi],
                              start=(k == 0), stop=(k == 1))
            h1 = pre.tile([1, DM], f32, tag="pv")
            for lo, hi in ((0, 512), (512, 768)):
                DVE.tensor_tensor(out=h1[:, lo:hi], in0=h1p[:, lo:hi], in1=C["tb1_r"][:, lo:hi], op=Alu.add)
            ACT.activation(out=h1[:], in_=h1[:], func=Act.Silu)
            GPS.dma_start(out=scr_b1[:], in_=h1[:])
            h1f = pre.tile([128, 6], f32, tag="pcf")
            GPS.dma_start(out=h1f[:], in_=scr_b1[0, :].rearrange("(g p) -> p g", g=6))
            h1c = pre.tile([128, 6], bf16, tag="pc")
            ACT.activation(out=h1c[:], in_=h1f[:], func=Act.Copy)

            h2p = psv.tile([1, DM], f32, tag="vec")
            for k in range(16):
                if k % 4 == 0:
                    fcw1c = wb.tile([128, 4 * DM], bf16, tag="wb", name=f"fcw1_{k // 4}")
                    load_kpm(fcw1c, W["fcw1T"][k:k + 4], 4)
                for lo, hi in ((0, 512), (512, 768)):
                    MM.matmul(out=h2p[:, lo:hi],
                              lhsT=C["fc_cols"][:, k:k + 1],
                              rhs=fcw1c[:, (k % 4) * DM + lo:(k % 4) * DM + hi],
                              start=(k == 0), stop=(k == 15))
            h2 = pre.tile([1, DM], f32, tag="pv")
            for lo, hi in ((0, 512), (512, 768)):
                DVE.tensor_tensor(out=h2[:, lo:hi], in0=h2p[:, lo:hi], in1=C["fcb1_r"][:, lo:hi], op=Alu.add)
            ACT.activation(out=h2[:], in_=h2[:], func=Act.Silu)
            GPS.dma_start(out=scr_b2[:], in_=h2[:])
            h2f = pre.tile([128, 6], f32, tag="pcf2")
            GPS.dma_start(out=h2f[:], in_=scr_b2[0, :].rearrange("(g p) -> p g", g=6))
            h2c = pre.tile([128, 6], bf16, tag="pc3")
            ACT.activation(out=h2c[:], in_=h2f[:], func=Act.Copy)

            # c = tw2@h1 + fcw2@h2 + tb2 + fcb2 ; silu; scatter
            cp_ps = psv.tile([1, DM], f32, tag="vec")
            nmm = 0
            for hsrc, wnm in ((h1c, "tw2T"), (h2c, "fcw2T")):
                for k in range(6):
                    if k % 3 == 0:
                        wc = wb.tile([128, 3 * DM], bf16, tag="wb", name=f"cw_{wnm}_{k}")
                        load_kpm(wc, W[wnm][k:k + 3], 3)
                    for lo, hi in ((0, 512), (512, 768)):
                        MM.matmul(out=cp_ps[:, lo:hi],
                                  lhsT=hsrc[:, k:k + 1],
                                  rhs=wc[:, (k % 3) * DM + lo:(k % 3) * DM + hi],
                                  start=(nmm == 0), stop=(nmm == 11))
                    nmm += 1
            for lo, hi in ((0, 512), (512, 768)):
                DVE.tensor_tensor(out=cvec[:, lo:hi], in0=cp_ps[:, lo:hi], in1=C["tb2_r"][:, lo:hi], op=Alu.add)
            DVE.tensor_tensor(out=cvec[:], in0=cvec[:], in1=C["fcb2_r"][:], op=Alu.add)
            ACT.activation(out=cvec[:], in_=cvec[:], func=Act.Silu)
            GPS.dma_start(out=scr_b3[:], in_=cvec[:])
            scf32 = pre.tile([128, 6], f32, tag="pc4")
            GPS.dma_start(out=scf32[:], in_=scr_b3[0, :].rearrange("(g p) -> p g", g=6))
            ACT.activation(out=silu_c[:], in_=scf32[:], func=Act.Copy)

            # fl_e = flw@fl + flb + flpos
            flsb = pre.tile([128, 16, LT], bf16, tag="pfl")
            GPS.dma_start(out=flsb[:], in_=W["flT"][:].rearrange("k p m -> p k m"))
            _es.close()
            _es = _ES()
            psfl = _es.enter_context(tc.tile_pool(name="psfl", bufs=1, space="PSUM"))
            fps = [psfl.tile([128, LT], f32, tag=f"fl{m}", name=f"flp{m}") for m in range(6)]
            for k in range(16):
                if k % 4 == 0:
                    flwc = wb.tile([128, 4 * DM], bf16, tag="wb", name=f"flw_{k // 4}")
                    load_kpm(flwc, W["flwT"][k:k + 4], 4)
                for m in range(6):
                    MM.matmul(out=fps[m][:],
                              lhsT=flwc[:, (k % 4) * DM + m * 128:(k % 4) * DM + (m + 1) * 128],
                              rhs=flsb[:, k, :], start=(k == 0), stop=(k == 15))
            for m in range(6):
                t_ = pre.tile([128, LT], f32, tag="pt2", name=f"fle{m}")
                ACT.activation(out=t_[:], in_=fps[m][:], func=Act.Identity, bias=C["flb_sc"][:, m:m + 1])
                DVE.tensor_tensor(out=fl_e[:, m * LT:(m + 1) * LT], in0=t_[:],
                                  in1=C["flpos_sc"][:, m, :], op=Alu.add)

            # x embedding
            _es.close()
            _es = _ES()
            ps1 = _es.enter_context(tc.tile_pool(name="ps1", bufs=2, space="PSUM"))
            xw = wb.tile([128, 2 * DM], bf16, tag="wb")
            load_kpm(xw, W["xwT"][:], 2)
            xsb = pre.tile([128, 2, L], bf16, tag="pfl2")
            GPS.dma_start(out=xsb[:], in_=W["xT"][:].rearrange("k p m -> p k m"))
            for m in range(6):
                xp = ps1.tile([128, L], f32, tag="mm")
                for k in range(2):
                    MM.matmul(out=xp[:], lhsT=xw[:, k * DM + m * 128:k * DM + (m + 1) * 128],
                              rhs=xsb[:, k, :], start=(k == 0), stop=(k == 1))
                ACT.activation(out=x[:, m * L:(m + 1) * L], in_=xp[:],
                               func=Act.Identity, bias=C["xb_sc"][:, m:m + 1])
            _es.close()

        # ---------------- layers ----------------
        x3 = lambda g: x[:, g * L:(g + 1) * L]
        xn3 = lambda g: xn[:, g * L:(g + 1) * L]
        dt3 = dt_t[:].rearrange("p (g t) -> p g t", g=GI)
        du3 = du[:].rearrange("p (g t) -> p g t", g=GI)
        y3 = y[:].rearrange("p (g t) -> p g t", g=GI)
        uc3i = uc[:].rearrange("p (g s) -> p g s", g=GI)[:, :, 4:SEG]
        up3 = u_pad[:].rearrange("p (g s) -> p g s", g=GI)

        def ln_block(l, psA, psS, psB, scale_col, shift_col):
            """LayerNorm of x -> xn (bf16), optionally modulated."""
            ACT.activation(out=tmp1[:], in_=x[:], func=Act.Square)
            st = psS.tile([1, 512], f32, tag="st", name=f"st{l}")
            for g in range(GM):
                MM.matmul(out=st[:, 0:L], lhsT=C["ones_col"][:], rhs=x3(g),
                          start=(g == 0), stop=(g == GM - 1))
            for g in range(GM):
                MM.matmul(out=st[:, L:2 * L], lhsT=C["ones_colb"][:],
                          rhs=tmp1[:, g * L:(g + 1) * L],
                          start=(g == 0), stop=(g == GM - 1))
            ACT.activation(out=stat[:, 0:L], in_=st[:, 0:L], func=Act.Copy, scale=1.0 / DM)
            ACT.activation(out=stat2[:], in_=stat[:, 0:L], func=Act.Square)
            DVE.scalar_tensor_tensor(out=stat2[:], in0=st[:, L:2 * L], scalar=1.0 / DM,
                                     in1=stat2[:], op0=Alu.mult, op1=Alu.subtract)
            ACT.activation(out=stat2[:], in_=stat2[:], func=Act.Sqrt, bias=1e-6)
            DVE.reciprocal(out=stat[:, L:2 * L], in_=stat2[:])
            bc = psB.tile([128, 512], f32, tag="bc", name=f"bc{l}")
            MM.matmul(out=bc[:], lhsT=C["ones_row"][:], rhs=stat[:, 0:512])
            for g in range(GM):
                DVE.tensor_tensor(out=tmp1[:, g * L:(g + 1) * L], in0=x3(g),
                                  in1=bc[:, 0:L], op=Alu.subtract)
                if scale_col is None:
                    DVE.tensor_tensor(out=xn3(g), in0=tmp1[:, g * L:(g + 1) * L],
                                      in1=bc[:, L:2 * L], op=Alu.mult)
                else:
                    DVE.tensor_tensor(out=tmp1[:, g * L:(g + 1) * L],
                                      in0=tmp1[:, g * L:(g + 1) * L],
                                      in1=bc[:, L:2 * L], op=Alu.mult)
                    DVE.scalar_tensor_tensor(
                        out=xn3(g), in0=tmp1[:, g * L:(g + 1) * L],
                        scalar=scale_col[:, g:g + 1],
                        in1=shift_col[:, g:g + 1].broadcast_to([128, L]),
                        op0=Alu.mult, op1=Alu.add)

        def ada_block(l, psM):
            """adaLN modulation matvecs for layer l -> mod_all/modx_all slices.

            Emitted two layers early so PE/DMA fill the scan window."""
            for r in range(2):
                for si, (lo, wdt) in enumerate(
                        ((0, 512), (512, 512), (1024, 512), (1536, 512), (2048, 256))):
                    ps = psM.tile([1, 512], f32, tag="m", name=f"mps{l}_{r}_{si}")
                    for k in range(6):
                        ah = ws.tile([128, 768], bf16, tag="ws", name=f"ada{l}_{r}_{si}_{k}")
                        DMA.dma_start(out=ah[:, 0:wdt],
                                      in_=W["adaT"][l, k][:, r * 2304 + lo:r * 2304 + lo + wdt])
                        MM.matmul(out=ps[:, 0:wdt], lhsT=silu_c[:, k:k + 1],
                                  rhs=ah[:, 0:wdt],
                                  start=(k == 0), stop=(k == 5))
                    sg = stg.tile([1, 512], f32, tag="stg", name=f"sg{l}_{r}_{si}")
                    ACT.activation(out=sg[:, 0:wdt], in_=ps[:, 0:wdt], func=Act.Copy)
                    GPS.dma_start(out=scr_mod[l:l + 1, r * 2304 + lo:r * 2304 + lo + wdt],
                                  in_=sg[:, 0:wdt])
            mf = mod_all[:, l * 36:(l + 1) * 36]
            GPS.dma_start(out=mf.rearrange("p (bl g) -> p bl g", bl=6),
                          in_=scr_mod[l, :].rearrange("(bl g p) -> p bl g", bl=6, g=6))
            DVE.tensor_tensor(out=mf, in0=mf,
                              in1=C["adab_sc"][:, l * 36:(l + 1) * 36], op=Alu.add)
            DVE.tensor_scalar_add(out=modx_all[:, l * 12:l * 12 + 6],
                                  in0=mod_all[:, l * 36 + 6:l * 36 + 12], scalar1=1.0)
            DVE.tensor_scalar_add(out=modx_all[:, l * 12 + 6:l * 12 + 12],
                                  in0=mod_all[:, l * 36 + 24:l * 36 + 30], scalar1=1.0)

        for l in range(2):
            with tc.tile_pool(name=f"psMp{l}", bufs=2, space="PSUM") as psM:
                ada_block(l, psM)

        for l in range(depth):
            mod_f = mod_all[:, l * 36:(l + 1) * 36]
            modx_l = modx_all[:, l * 12:(l + 1) * 12]

            # ---- mamba ----
            with tc.tile_pool(name=f"psA{l}", bufs=2, space="PSUM") as psA, \
                 tc.tile_pool(name=f"psS{l}", bufs=1, space="PSUM") as psS, \
                 tc.tile_pool(name=f"psB{l}", bufs=1, space="PSUM") as psB, \
                 tc.tile_pool(name=f"psV{l}", bufs=1, space="PSUM") as psV, \
                 tc.tile_pool(name=f"psM{l}", bufs=2, space="PSUM") as psM, \
                 tc.tile_pool(name=f"psX{l}", bufs=1, space="PSUM") as psX:
                ln_block(10 * l, psA, psS, psB, modx_l[:, 0:6], mod_f[:, 0:6])

                for mb in range(4):
                    wts = []
                    for k in range(6):
                        wi = ws.tile([128, 768], bf16, tag="ws", name=f"inw{l}_{mb}_{k}")
                        DMA.dma_start(out=wi[:], in_=W["in_wT"][l, k][:, mb * 768:(mb + 1) * 768])
                        wts.append(wi)
                    for mi in range(6):
                        m = mb * 6 + mi
                        ps = psA.tile([128, L], f32, tag="mm", name=f"ip{l}_{m}")
                        for k in range(6):
                            MM.matmul(out=ps[:], lhsT=wts[k][:, mi * 128:(mi + 1) * 128],
                                      rhs=xn3(k), start=(k == 0), stop=(k == 5))
                        if m < 12:
                            ACT.activation(out=up3[:, m, 4:SEG], in_=ps[:], func=Act.Copy)
                        else:
                            ACT.activation(out=z[:, (m - 12) * L:(m - 11) * L], in_=ps[:], func=Act.Silu)

                # ---- attention K/V: depend only on fl_e, so compute them here
                # so PE has work queued during the scan window ----
                wk_ = []
                for i in range(2):
                    t_ = wb.tile([128, 3 * NH * HP], bf16, tag="wb", name=f"wk{l}_{i}")
                    load_kpm(t_, W["wkT"][l, 3 * i:3 * i + 3], 3)
                    wk_.append(t_)
                for h in range(NH):
                    ps = psA.tile([128, L], f32, tag="mm", name=f"kp{l}_{h}")
                    for k in range(6):
                        MM.matmul(out=ps[:, 0:LT],
                                  lhsT=wk_[k // 3][:, (k % 3) * NH * HP + h * HP:(k % 3) * NH * HP + (h + 1) * HP],
                                  rhs=fl_e[:, k * LT:(k + 1) * LT], start=(k == 0), stop=(k == 5))
                    ACT.activation(out=k_sb[:, h, :], in_=ps[:, 0:LT], func=Act.Identity,
                                   bias=C["bk_sc"][:, l * 8 + h:l * 8 + h + 1])
                wv_ = []
                for i in range(2):
                    t_ = wb.tile([128, 3 * NH * HP], bf16, tag="wb", name=f"wv{l}_{i}")
                    load_kpm(t_, W["wvT"][l, 3 * i:3 * i + 3], 3)
                    wv_.append(t_)
                GPS.dma_start(out=bvb[:], in_=W["bv_pad"][l].partition_broadcast(LT))
                for fo in range(2):
                    vt_ps = psV.tile([LT, 512], f32, tag="vt", name=f"vtp{l}_{fo}")
                    for k in range(6):
                        MM.matmul(out=vt_ps[:],
                                  lhsT=fl_e[:, k * LT:(k + 1) * LT],
                                  rhs=wv_[k // 3][:, (k % 3) * NH * HP + fo * 512:(k % 3) * NH * HP + (fo + 1) * 512],
                                  start=(k == 0), stop=(k == 5))
                    DVE.tensor_tensor(out=vt_sb[:, fo * 512:(fo + 1) * 512],
                                      in0=vt_ps[:],
                                      in1=bvb[:, fo * 512:(fo + 1) * 512], op=Alu.add)

                # causal depthwise conv (acc in y, f32) + silu -> uc
                cw4 = C["convw_sc"][:].rearrange("p (a g k) -> p a g k", a=DEPTH, g=12)
                for k in range(4):
                    wkb = cw4[:, l, :, k].unsqueeze(2).broadcast_to([128, GI, L])
                    sh = up3[:, :, 1 + k:1 + k + L]
                    if k == 0:
                        GPS.tensor_tensor(out=y3, in0=sh, in1=wkb, op=Alu.mult)
                    else:
                        GPS.tensor_tensor(out=hh[0][:].rearrange("p (g s) -> p g s", g=GI)[:, :, 4:SEG],
                                          in0=sh, in1=wkb, op=Alu.mult)
                        GPS.tensor_tensor(out=y3, in0=y3,
                                          in1=hh[0][:].rearrange("p (g s) -> p g s", g=GI)[:, :, 4:SEG],
                                          op=Alu.add)
                for g in range(GI):
                    ACT.activation(out=uc3i[:, g, :], in_=y[:, g * L:(g + 1) * L],
                                   func=Act.Silu, bias=C["convb_sc"][:, l * 12 + g:l * 12 + g + 1])

                # xproj -> xdbl [80, L]
                xpw = wb.tile([128, 1152], bf16, tag="wb", name=f"xpw{l}")
                load_kpm(xpw, W["xprojT"][l], 12)
                xd = psX.tile([96, L], f32, tag="xd", name=f"xd{l}")
                for k in range(GI):
                    MM.matmul(out=xd[:], lhsT=xpw[:, k * 96:(k + 1) * 96],
                              rhs=uc3i[:, k, :], start=(k == 0), stop=(k == GI - 1))
                ACT.activation(out=xdbl_sb[:], in_=xd[0:48, :], func=Act.Copy)
                ACT.activation(out=bc_sb[:], in_=xd[64:96, :], func=Act.Copy)
                GPS.dma_start(out=scr_bc[0, :].rearrange("(a b) -> a b", a=32), in_=bc_sb[:])
                GPS.dma_start(out=brep[:], in_=scr_bc[0, 0:NST * L].partition_broadcast(128))
                GPS.dma_start(out=crep[:], in_=scr_bc[0, NST * L:2 * NST * L].partition_broadcast(128))

                # dt = softplus(dt_w @ xdbl[:48] + dt_b)
                dtw = wb.tile([48, DI], bf16, tag="wb", name=f"dtw{l}")
                DMA.dma_start(out=dtw[:], in_=W["dt_wT"][l])
                for m in range(GI):
                    ps = psA.tile([128, L], f32, tag="mm", name=f"dtp{l}_{m}")
                    MM.matmul(out=ps[:], lhsT=dtw[:, m * 128:(m + 1) * 128], rhs=xdbl_sb[:])
                    sl_ = dt_t[:, m * L:(m + 1) * L]
                    ACT.activation(out=sl_, in_=ps[:], func=Act.Exp,
                                   bias=C["dtb_sc"][:, l * 12 + m:l * 12 + m + 1])
                    DVE.tensor_scalar_add(out=sl_, in0=sl_, scalar1=1.0)
                    ACT.activation(out=sl_, in_=sl_, func=Act.Ln)
                DVE.tensor_tensor(out=du3, in0=dt3, in1=uc3i, op=Alu.mult)

                # modulation matvecs for layer l+2: PE+DMA work that overlaps
                # the DVE-bound scan below
                if l + 2 < depth:
                    ada_block(l + 2, psM)

                # selective scan over n (state dim), pad cols reset state
                for n in range(NST):
                    sl = n % 2
                    dAi = dA[sl][:].rearrange("p (g s) -> p g s", g=GI)[:, :, 4:SEG]
                    dBi = dB[sl][:].rearrange("p (g s) -> p g s", g=GI)[:, :, 4:SEG]
                    hi = hh[sl][:].rearrange("p (g s) -> p g s", g=GI)[:, :, 4:SEG]
                    ACT.activation(out=dAi, in_=dt3, func=Act.Exp,
                                   scale=aneg[:, l * 192 + n:l * 192 + n + 1])
                    DVE.tensor_tensor(out=dBi, in0=du3,
                                      in1=brep[:, n * L:(n + 1) * L].unsqueeze(1).broadcast_to([128, GI, L]),
                                      op=Alu.mult)
                    DVE.tensor_tensor_scan(out=hh[sl][:], data0=dA[sl][:], data1=dB[sl][:],
                                           initial=0.0, op0=Alu.mult, op1=Alu.add)
                    DVE.tensor_tensor(out=hi, in0=hi,
                                      in1=crep[:, n * L:(n + 1) * L].unsqueeze(1).broadcast_to([128, GI, L]),
                                      op=Alu.mult)
                    if n == 0:
                        DVE.tensor_copy(out=y3, in_=hi)
                    else:
                        DVE.tensor_tensor(out=y3, in0=y3, in1=hi, op=Alu.add)

                # y = (y + uc*Dp) * silu(z);  out-proj; residual with gm
                for g in range(GI):
                    DVE.scalar_tensor_tensor(out=y[:, g * L:(g + 1) * L], in0=uc3i[:, g, :],
                                             scalar=C["Dp_sc"][:, l * 12 + g:l * 12 + g + 1],
                                             in1=y[:, g * L:(g + 1) * L], op0=Alu.mult, op1=Alu.add)
                DVE.tensor_tensor(out=du[:], in0=y[:], in1=z[:], op=Alu.mult)

                ow = []
                for i in range(2):
                    wi = wb.tile([128, 6 * DM], bf16, tag="wb", name=f"ow{l}_{i}")
                    load_kpm(wi, W["out_wT"][l, 6 * i:6 * i + 6], 6)
                    ow.append(wi)
                for m in range(GM):
                    ps = psA.tile([128, L], f32, tag="mm", name=f"op{l}_{m}")
                    for k in range(GI):
                        MM.matmul(out=ps[:], lhsT=ow[k // 6][:, (k % 6) * DM + m * 128:(k % 6) * DM + (m + 1) * 128],
                                  rhs=du[:, k * L:(k + 1) * L], start=(k == 0), stop=(k == GI - 1))
                    DVE.scalar_tensor_tensor(out=x3(m), in0=ps[:], scalar=mod_f[:, 12 + m:13 + m],
                                             in1=x3(m), op0=Alu.mult, op1=Alu.add)

            # ---- cross attention ----
            with tc.tile_pool(name=f"qsA{l}", bufs=2, space="PSUM") as psA, \
                 tc.tile_pool(name=f"qsS{l}", bufs=1, space="PSUM") as psS, \
                 tc.tile_pool(name=f"qsB{l}", bufs=1, space="PSUM") as psB, \
                 tc.tile_pool(name=f"qsC{l}", bufs=1, space="PSUM") as psC, \
                 tc.tile_pool(name=f"qsP{l}", bufs=1, space="PSUM") as psP:
                ln_block(10 * l + 1, psA, psS, psB, None, None)

                wq = []
                for i in range(2):
                    t_ = wb.tile([128, 3 * NH * HP], bf16, tag="wb", name=f"wq{l}_{i}")
                    load_kpm(t_, W["wqT"][l, 3 * i:3 * i + 3], 3)
                    wq.append(t_)
                for h in range(NH):
                    ps = psA.tile([128, L], f32, tag="mm", name=f"qp{l}_{h}")
                    for k in range(6):
                        MM.matmul(out=ps[:], lhsT=wq[k // 3][:, (k % 3) * NH * HP + h * HP:(k % 3) * NH * HP + (h + 1) * HP],
                                  rhs=xn3(k), start=(k == 0), stop=(k == 5))
                    ACT.activation(out=q_sb[:, h, :], in_=ps[:], func=Act.Identity,
                                   bias=C["bq_sc"][:, l * 8 + h:l * 8 + h + 1])

                for h in range(NH):
                    for tc2 in range(2):
                        idx = h * 2 + tc2
                        sc_ps = psC.tile([128, LT], f32, tag="sc", name=f"scp{l}_{idx}")
                        MM.matmul(out=sc_ps[:], lhsT=q_sb[:, h, tc2 * 128:(tc2 + 1) * 128],
                                  rhs=k_sb[:, h, :])
                        mx = small[:, 2:3]
                        DVE.tensor_reduce(out=mx, in_=sc_ps[:], axis=mybir.AxisListType.X, op=Alu.max)
                        DVE.tensor_scalar_mul(out=small[:, 3:4], in0=mx, scalar1=-SQ)
                        ACT.activation(out=p_all[:, idx * LT:(idx + 1) * LT], in_=sc_ps[:],
                                       func=Act.Exp, scale=SQ, bias=small[:, 3:4],
                                       accum_out=rs_all[:, idx:idx + 1])
                DVE.reciprocal(out=ri_all[:], in_=rs_all[:])
                for h in range(NH):
                    for tc2 in range(2):
                        idx = h * 2 + tc2
                        DVE.tensor_scalar_mul(out=p_all[:, idx * LT:(idx + 1) * LT],
                                              in0=p_all[:, idx * LT:(idx + 1) * LT],
                                              scalar1=ri_all[:, idx:idx + 1])
                        pt_ps = psP.tile([LT, 128], bf16, tag="pt", name=f"ptp{l}_{idx}")
                        MM.transpose(out=pt_ps[:], in_=p_all[:, idx * LT:(idx + 1) * LT], identity=C["id128"][:])
                        ACT.activation(out=pt_sb[:, h * L + tc2 * 128:h * L + (tc2 + 1) * 128],
                                       in_=pt_ps[:], func=Act.Copy)
                for h in range(NH):
                    ps = psA.tile([128, L], f32, tag="mm", name=f"avp{l}_{h}")
                    for tc2 in range(2):
                        MM.matmul(out=ps[:, tc2 * 128:(tc2 + 1) * 128],
                                  lhsT=vt_sb[:, h * HP:(h + 1) * HP],
                                  rhs=pt_sb[:, h * L + tc2 * 128:h * L + (tc2 + 1) * 128])
                    ACT.activation(out=avt_sb[:, h, :], in_=ps[:], func=Act.Copy)

                ao = []
                for i in range(2):
                    t_ = wb.tile([128, 4 * DM], bf16, tag="wb", name=f"ao{l}_{i}")
                    load_kpm(t_, W["aoT"][l, 4 * i:4 * i + 4], 4)
                    ao.append(t_)
                for m in range(GM):
                    ps = psA.tile([128, L], f32, tag="mm", name=f"aop{l}_{m}")
                    for k in range(NH):
                        MM.matmul(out=ps[:], lhsT=ao[k // 4][:, (k % 4) * DM + m * 128:(k % 4) * DM + (m + 1) * 128],
                                  rhs=avt_sb[:, k, :], start=(k == 0), stop=(k == NH - 1))
                    DVE.scalar_tensor_tensor(out=x3(m), in0=ps[:], scalar=C["aob_sc"][:, l * 6 + m:l * 6 + m + 1],
                                             in1=x3(m), op0=Alu.add, op1=Alu.add)

            # ---- FFN ----
            with tc.tile_pool(name=f"fsA{l}", bufs=2, space="PSUM") as psA, \
                 tc.tile_pool(name=f"fsS{l}", bufs=1, space="PSUM") as psS, \
                 tc.tile_pool(name=f"fsB{l}", bufs=1, space="PSUM") as psB:
                ln_block(10 * l + 2, psA, psS, psB, modx_l[:, 6:12], mod_f[:, 18:24])
                for half in range(2):
                    for mb in range(2):
                        wts = []
                        for k in range(6):
                            wi = ws.tile([128, 768], bf16, tag="ws", name=f"f1w{l}_{half}_{mb}_{k}")
                            DMA.dma_start(out=wi[:],
                                          in_=W["fw1T"][l, k][:, (half * 2 + mb) * 768:(half * 2 + mb + 1) * 768])
                            wts.append(wi)
                        for mi in range(6):
                            m = half * 12 + mb * 6 + mi
                            ml = mb * 6 + mi
                            ps = psA.tile([128, L], f32, tag="mm", name=f"f1p{l}_{m}")
                            for k in range(6):
                                MM.matmul(out=ps[:], lhsT=wts[k][:, mi * 128:(mi + 1) * 128],
                                          rhs=xn3(k), start=(k == 0), stop=(k == 5))
                            ACT.activation(out=hffn[:, ml * L:(ml + 1) * L], in_=ps[:], func=Act.Gelu,
                                           bias=C["fb1_sc"][:, l * 24 + m:l * 24 + m + 1])
                    f2 = []
                    for i in range(2):
                        t_ = wb.tile([128, 6 * DM], bf16, tag="wb", name=f"f2{l}_{half}_{i}")
                        load_kpm(t_, W["fw2T"][l, half * 12 + 6 * i:half * 12 + 6 * i + 6], 6)
                        f2.append(t_)
                    for m in range(GM):
                        ps = psA.tile([128, L], f32, tag="mm", name=f"f2p{l}_{half}_{m}")
                        for k in range(12):
                            MM.matmul(out=ps[:], lhsT=f2[k // 6][:, (k % 6) * DM + m * 128:(k % 6) * DM + (m + 1) * 128],
                                      rhs=hffn[:, k * L:(k + 1) * L], start=(k == 0), stop=(k == 11))
                        if half == 0:
                            ACT.activation(out=tmp1[:, m * 256:(m + 1) * 256], in_=ps[:], func=Act.Copy)
                        else:
                            DVE.tensor_tensor(out=tmp1[:, m * 256:(m + 1) * 256],
                                              in0=tmp1[:, m * 256:(m + 1) * 256], in1=ps[:], op=Alu.add)
                            DVE.tensor_scalar(out=tmp1[:, m * 256:(m + 1) * 256],
                                              in0=tmp1[:, m * 256:(m + 1) * 256],
                                              scalar1=C["fb2_sc"][:, l * 6 + m:l * 6 + m + 1],
                                              scalar2=mod_f[:, 30 + m:31 + m], op0=Alu.add, op1=Alu.mult)
                            DVE.tensor_tensor(out=x3(m), in0=x3(m), in1=tmp1[:, m * 256:(m + 1) * 256], op=Alu.add)

        # ---------------- final ----------------
        with tc.tile_pool(name="fin", bufs=1) as fin, \
             tc.tile_pool(name="zsA", bufs=2, space="PSUM") as psA, \
             tc.tile_pool(name="zsS", bufs=1, space="PSUM") as psS, \
             tc.tile_pool(name="zsB", bufs=1, space="PSUM") as psB, \
             tc.tile_pool(name="zsV", bufs=3, space="PSUM") as psV:
            fm_ps = [psV.tile([1, 512], f32, tag="fm5", name=f"fmps{s}") for s in range(3)]
            for k in range(6):
                fad = fin.tile([128, 2 * DM], bf16, tag="fw", name=f"fad{k}")
                load_kpm(fad, W["finadaT"][k:k + 1], 1)
                for s in range(3):
                    MM.matmul(out=fm_ps[s][:],
                              lhsT=silu_c[:, k:k + 1],
                              rhs=fad[:, s * 512:(s + 1) * 512],
                              start=(k == 0), stop=(k == 5))
            for s in range(3):
                sg = stg.tile([1, 512], f32, tag="stg", name=f"fsg{s}")
                ACT.activation(out=sg[:], in_=fm_ps[s][:], func=Act.Copy)
                GPS.dma_start(out=scr_fm[:, s * 512:(s + 1) * 512], in_=sg[:])
            GPS.dma_start(out=fmod_sc[:].rearrange("p (bl g) -> p bl g", bl=2),
                          in_=scr_fm[0, :].rearrange("(bl g p) -> p bl g", bl=2, g=6))
            DVE.tensor_tensor(out=fmod_sc[:], in0=fmod_sc[:], in1=C["finadab_sc"][:], op=Alu.add)
            DVE.tensor_scalar_add(out=modx[:, 0:6], in0=fmod_sc[:, 6:12], scalar1=1.0)
            ln_block(999, psA, psS, psB, modx[:, 0:6], fmod_sc[:, 0:6])

            xo_sb = fin.tile([128, 2, CIN], f32, tag="fxo")
            outT = fin.tile([128, 2 * CIN], f32, tag="fot")
            fw = fin.tile([128, 6 * CIN], bf16, tag="fw2")
            load_kpm(fw, W["finT"][:], 6)
            for m in range(2):
                ps = psA.tile([128, L], f32, tag="mm", name=f"fop{m}")
                for k in range(6):
                    MM.matmul(out=ps[:], lhsT=fw[:, k * CIN + m * 128:k * CIN + (m + 1) * 128],
                              rhs=xn3(k), start=(k == 0), stop=(k == 5))
                ACT.activation(out=xo_sb[:, m, :], in_=ps[:], func=Act.Identity,
                               bias=C["finb_sc"][:, m:m + 1])
            # transpose [ch, t] -> [t, ch] and store
            for tc2 in range(2):
                for m in range(2):
                    tp = psA.tile([128, 128], f32, tag="mm", name=f"tp{tc2}_{m}")
                    MM.transpose(out=tp[:], in_=xo_sb[:, m, tc2 * 128:(tc2 + 1) * 128],
                                 identity=C["id128f"][:])
                    ACT.activation(out=outT[:, tc2 * CIN + m * 128:tc2 * CIN + (m + 1) * 128],
                                   in_=tp[:], func=Act.Copy)
            GPS.dma_start(out=out_d[:].rearrange("(a p) c -> p a c", a=2),
                          in_=outT[:].rearrange("p (a c) -> p a c", a=2))
    nc.finalize()
    # walrus' verifier rejects leftover unused framework registers with
    # reg_id=-1; give each a harmless unique id per engine.
    from collections import defaultdict
    nxt = defaultdict(int)
    for fn in nc.m.functions:
        for a in fn.allocations:
            if getattr(a, "reg_id", None) == -1:
                eng = str(getattr(a, "engine", "?"))
                n = getattr(a, "num_physical_regs", None) or 1
                if n > 1 and nxt[eng] % 2:
                    nxt[eng] += 1
                a.reg_id = nxt[eng]
                nxt[eng] += n
    return nc


_CACHE = {}


def kernel(**inputs):
    depth = DEPTH
    if "nc" not in _CACHE:
        _CACHE["nc"] = build_nc(depth)
    nc = _CACHE["nc"]
    shared = prep_shared(inputs)
    in_maps = []
    for b in range(N_CORES):
        m = dict(shared)
        m.update(prep_core(inputs, b))
        in_maps.append(m)
    res = run_bass_kernel_spmd(nc, in_maps, list(range(N_CORES)))
    out = np.stack([np.asarray(res.results[b]["out"], np.float32) for b in range(N_CORES)])
    return out


def kernel_profiled(**inputs):
    if "nc" not in _CACHE:
        _CACHE["nc"] = build_nc(DEPTH)
    nc = _CACHE["nc"]
    shared = prep_shared(inputs)
    in_maps = []
    for b in range(N_CORES):
        m = dict(shared)
        m.update(prep_core(inputs, b))
        in_maps.append(m)
    res = run_bass_kernel_spmd(nc, in_maps, list(range(N_CORES)), trace=True)
    out = np.stack([np.asarray(res.results[b]["out"], np.float32) for b in range(N_CORES)])
    return out, res.exec_time_ns



# revision 19
# speedup vs baseline: 1.1767x; 1.0078x over previous
import sys

sys.path.insert(0, "/opt/trn_rl_repo")
import math

import numpy as np
import ml_dtypes

from concourse import bass, bacc, mybir
from concourse import tile
from concourse.bass_utils import run_bass_kernel_spmd

BF = ml_dtypes.bfloat16
bf16 = mybir.dt.bfloat16
f32 = mybir.dt.float32
Alu = mybir.AluOpType
Act = mybir.ActivationFunctionType

B, L, CIN, COND, DM, DEPTH = 8, 256, 256, 2048, 768, 12
NST, DCONV, DI, DTR = 16, 4, 1536, 48
NH, HD, LT, FREQ = 8, 96, 35, 256
GM, GI = DM // 128, DI // 128          # 6, 12
SEG = L + 4                            # 260, 4 zero pad cols reset scan state
SCANW = GI * SEG                       # 3120
HP = 128                               # padded head dim
SQ = 1.0 / math.sqrt(HD)
N_CORES = 8


def _bf(a):
    return np.ascontiguousarray(a, dtype=np.float32).astype(BF)


def _f(a):
    return np.ascontiguousarray(a, dtype=np.float32)


def prep_shared(inp):
    """Host-side layout/dtype staging of the weights (shared by all cores)."""
    d = {}
    d["xwT"] = _bf(inp["xw"].T.reshape(2, 128, DM))
    d["tw1T"] = _bf(inp["tw1"].T.reshape(2, 128, DM))
    d["tw2T"] = _bf(inp["tw2"].T.reshape(6, 128, DM))
    d["fcw1T"] = _bf(inp["fcw1"].T.reshape(16, 128, DM))
    d["fcw2T"] = _bf(inp["fcw2"].T.reshape(6, 128, DM))
    d["flwT"] = _bf(inp["flw"].T.reshape(16, 128, DM))
    d["adaT"] = _bf(np.ascontiguousarray(inp["ada_w"].transpose(0, 2, 1)).reshape(DEPTH, 6, 128, 6 * DM))
    d["in_wT"] = _bf(np.ascontiguousarray(inp["in_w"].transpose(0, 2, 1)).reshape(DEPTH, 6, 128, 2 * DI))
    xpt = np.ascontiguousarray(inp["xproj_w"].transpose(0, 2, 1)).astype(np.float32)  # [12,1536,80]
    xpp = np.zeros((DEPTH, DI, 96), np.float32)
    xpp[:, :, 0:48] = xpt[:, :, 0:48]
    xpp[:, :, 64:96] = xpt[:, :, 48:80]
    d["xprojT"] = _bf(xpp.reshape(DEPTH, 12, 128, 96))
    d["dt_wT"] = _bf(np.ascontiguousarray(inp["dt_w"].transpose(0, 2, 1)))          # [12,48,1536]
    d["out_wT"] = _bf(np.ascontiguousarray(inp["out_w"].transpose(0, 2, 1)).reshape(DEPTH, 12, 128, DM))
    qkv = inp["qkv_w"]
    wq, wk, wv = qkv[:, :DM], qkv[:, DM:2 * DM], qkv[:, 2 * DM:]
    for nm, w in (("wqT", wq), ("wkT", wk), ("wvT", wv)):
        wt = np.ascontiguousarray(w.transpose(0, 2, 1))                              # [12,768,768]
        pad = np.zeros((DEPTH, DM, NH * HP), np.float32)
        for h in range(NH):
            pad[:, :, h * HP:h * HP + HD] = wt[:, :, h * HD:(h + 1) * HD]
        d[nm] = _bf(pad.reshape(DEPTH, 6, 128, NH * HP))
    aot = np.ascontiguousarray(inp["ao_w"].transpose(0, 2, 1))                       # [12,768(dv),768]
    aop = np.zeros((DEPTH, NH * HP, DM), np.float32)
    for h in range(NH):
        aop[:, h * HP:h * HP + HD] = aot[:, h * HD:(h + 1) * HD]
    d["aoT"] = _bf(aop.reshape(DEPTH, 8, 128, DM))
    d["fw1T"] = _bf(np.ascontiguousarray(inp["fw1"].transpose(0, 2, 1)).reshape(DEPTH, 6, 128, 4 * DM))
    d["fw2T"] = _bf(np.ascontiguousarray(inp["fw2"].transpose(0, 2, 1)).reshape(DEPTH, 24, 128, DM))
    d["finadaT"] = _bf(inp["fin_ada_w"].T.reshape(6, 128, 2 * DM))
    d["finT"] = _bf(inp["fin_w"].T.reshape(6, 128, CIN))

    # per-partition scatters (fp32), layout [128, ...]
    d["xb_sc"] = _f(inp["xb"].reshape(6, 128).T)
    d["flb_sc"] = _f(inp["flb"].reshape(6, 128).T)
    d["tb1_r"] = _f(inp["tb1"].reshape(1, DM))
    d["tb2_r"] = _f(inp["tb2"].reshape(1, DM))
    d["fcb1_r"] = _f(inp["fcb1"].reshape(1, DM))
    d["fcb2_r"] = _f(inp["fcb2"].reshape(1, DM))
    d["flpos_sc"] = _f(np.ascontiguousarray(inp["flpos"][0].T).reshape(6, 128, LT).transpose(1, 0, 2))
    d["dtb_sc"] = _f(inp["dt_b"].reshape(DEPTH, 12, 128).transpose(2, 0, 1).reshape(128, -1))
    d["convw_sc"] = _f(inp["conv_w"].reshape(DEPTH, 12, 128, 4).transpose(2, 0, 1, 3).reshape(128, -1))
    d["convb_sc"] = _f(inp["conv_b"].reshape(DEPTH, 12, 128).transpose(2, 0, 1).reshape(128, -1))
    d["Dp_sc"] = _f(inp["Dp"].reshape(DEPTH, 12, 128).transpose(2, 0, 1).reshape(128, -1))
    d["alog_sc"] = _f(inp["A_log"].reshape(DEPTH, 12, 128, NST).transpose(2, 0, 1, 3).reshape(128, -1))
    d["adab_sc"] = _f(inp["ada_b"].reshape(DEPTH, 6, 6, 128).transpose(3, 0, 1, 2).reshape(128, -1))
    qb = inp["qkv_b"]
    for nm, bias in (("bq_sc", qb[:, :DM]), ("bk_sc", qb[:, DM:2 * DM])):
        arr = np.zeros((DEPTH, NH, HP), np.float32)
        arr[:, :, :HD] = np.asarray(bias, np.float32).reshape(DEPTH, NH, HD)
        d[nm] = _f(arr.transpose(2, 0, 1).reshape(128, -1))
    bv = np.zeros((DEPTH, NH, HP), np.float32)
    bv[:, :, :HD] = np.asarray(qb[:, 2 * DM:], np.float32).reshape(DEPTH, NH, HD)
    d["bv_pad"] = _bf(bv.reshape(DEPTH, NH * HP))
    d["aob_sc"] = _f(inp["ao_b"].reshape(DEPTH, 6, 128).transpose(2, 0, 1).reshape(128, -1))
    d["fb1_sc"] = _f(inp["fb1"].reshape(DEPTH, 24, 128).transpose(2, 0, 1).reshape(128, -1))
    d["fb2_sc"] = _f(inp["fb2"].reshape(DEPTH, 6, 128).transpose(2, 0, 1).reshape(128, -1))
    d["finadab_sc"] = _f(inp["fin_ada_b"].reshape(2, 6, 128).transpose(2, 0, 1).reshape(128, 12))
    d["finb_sc"] = _f(inp["fin_b"].reshape(2, 128).T)

    # constants (input independent)
    d["id128"] = _bf(np.eye(128))
    d["id128f"] = _f(np.eye(128))
    d["ones_col"] = _f(np.ones((128, 1)))
    d["ones_colb"] = _bf(np.ones((128, 1)))
    d["ones_row"] = _f(np.ones((1, 128)))
    half = FREQ // 2
    return d


def prep_core(inp, b):
    d = {}
    d["xT"] = _bf(np.asarray(inp["x"][b], np.float32).T.reshape(2, 128, L))
    half = FREQ // 2
    fr = np.exp(-math.log(10000.0) * np.arange(half) / half).reshape(128, 1)
    d["ftp"] = _f(np.concatenate([fr, np.full((128, 1), np.asarray(inp["t"][b], np.float32))], 1))
    d["fc_cols"] = _bf(np.asarray(inp["fc"][b], np.float32).reshape(16, 128).T)
    d["flT"] = _bf(np.asarray(inp["fl"][b], np.float32).T.reshape(16, 128, LT))
    return d


def build_nc(depth=DEPTH):
    nc = bacc.Bacc(None)
    for val in (math.pi / 2, 1e-6, -math.pi):
        t_ = nc.alloc_sbuf_tensor(f"const-f32-{val}", [128, 1], f32)
        nc.gpsimd.memset(t_.ap(), val)
        nc.const_aps.aps[(f32, val)] = t_.ap()
    nc.all_engine_barrier()
    P = nc.declare_dram_parameter

    W = {}
    for nm, shp, dt in [
        ("xwT", [2, 128, DM], bf16), ("tw1T", [2, 128, DM], bf16),
        ("tw2T", [6, 128, DM], bf16), ("fcw1T", [16, 128, DM], bf16),
        ("fcw2T", [6, 128, DM], bf16), ("flwT", [16, 128, DM], bf16),
        ("adaT", [DEPTH, 6, 128, 6 * DM], bf16), ("in_wT", [DEPTH, 6, 128, 2 * DI], bf16),
        ("xprojT", [DEPTH, 12, 128, 96], bf16), ("dt_wT", [DEPTH, 48, DI], bf16),
        ("out_wT", [DEPTH, 12, 128, DM], bf16),
        ("wqT", [DEPTH, 6, 128, NH * HP], bf16), ("wkT", [DEPTH, 6, 128, NH * HP], bf16),
        ("wvT", [DEPTH, 6, 128, NH * HP], bf16), ("aoT", [DEPTH, 8, 128, DM], bf16),
        ("fw1T", [DEPTH, 6, 128, 4 * DM], bf16), ("fw2T", [DEPTH, 24, 128, DM], bf16),
        ("finadaT", [6, 128, 2 * DM], bf16), ("finT", [6, 128, CIN], bf16),
        ("xb_sc", [128, 6], f32), ("flb_sc", [128, 6], f32),
        ("tb1_r", [1, DM], f32), ("tb2_r", [1, DM], f32),
        ("fcb1_r", [1, DM], f32), ("fcb2_r", [1, DM], f32),
        ("flpos_sc", [128, 6, LT], f32),
        ("dtb_sc", [128, DEPTH * 12], f32), ("convw_sc", [128, DEPTH * 48], f32),
        ("convb_sc", [128, DEPTH * 12], f32), ("Dp_sc", [128, DEPTH * 12], f32),
        ("alog_sc", [128, DEPTH * 192], f32), ("adab_sc", [128, DEPTH * 36], f32),
        ("bq_sc", [128, DEPTH * 8], f32), ("bk_sc", [128, DEPTH * 8], f32),
        ("bv_pad", [DEPTH, NH * HP], bf16),
        ("aob_sc", [128, DEPTH * 6], f32), ("fb1_sc", [128, DEPTH * 24], f32),
        ("fb2_sc", [128, DEPTH * 6], f32), ("finadab_sc", [128, 12], f32),
        ("finb_sc", [128, 2], f32),
        ("id128", [128, 128], bf16), ("id128f", [128, 128], f32),
        ("ones_col", [128, 1], f32), ("ones_colb", [128, 1], bf16), ("ones_row", [1, 128], f32),
        ("xT", [2, 128, L], bf16), ("ftp", [128, 2], f32),
        ("fc_cols", [128, 16], bf16), ("flT", [16, 128, LT], bf16),
    ]:
        W[nm] = P(nm, shp, dt, isOutput=False)
    out_d = P("out", [L, CIN], f32, isOutput=True)
    scr_b1 = P("scr_b1", [1, DM], f32, isOutput=True)
    scr_b2 = P("scr_b2", [1, DM], f32, isOutput=True)
    scr_b3 = P("scr_b3", [1, DM], f32, isOutput=True)
    scr_mod = P("scr_mod", [DEPTH, 6 * DM], f32, isOutput=True)
    scr_bc = P("scr_bc", [1, 2 * NST * L], bf16, isOutput=True)
    scr_fm = P("scr_fm", [1, 2 * DM], f32, isOutput=True)

    MM, ACT, DVE, GPS, DMA = nc.tensor, nc.scalar, nc.vector, nc.gpsimd, nc.sync

    def g3(ap, n=GI, w=None):
        return ap.rearrange("p (g t) -> p g t", g=n)

    def load_kpm(dst, srcap, kdim):
        DMA.dma_start(out=dst[:].rearrange("p (k m) -> p k m", k=kdim),
                      in_=srcap.rearrange("k p m -> p k m"))

    with tile.TileContext(nc) as tc:
      from contextlib import ExitStack
      with ExitStack() as top:
        cp = top.enter_context(tc.tile_pool(name="cp", bufs=1))
        stg = top.enter_context(tc.tile_pool(name="stg", bufs=2))
        wb = top.enter_context(tc.tile_pool(name="wb", bufs=3))
        ws = top.enter_context(tc.tile_pool(name="ws", bufs=8))

        # ---- persistent SBUF state ----
        x = cp.tile([128, GM * L], f32)
        u_pad = cp.tile([128, SCANW], bf16)
        uc = cp.tile([128, SCANW], bf16)
        z = cp.tile([128, GI * L], bf16)
        dt_t = cp.tile([128, GI * L], bf16)
        du = cp.tile([128, GI * L], bf16)
        brep = cp.tile([128, NST * L], bf16)
        crep = cp.tile([128, NST * L], bf16)
        dA = [cp.tile([128, SCANW], bf16, name="dA0")] * 2
        dB = [cp.tile([128, SCANW], bf16, name=f"dB{i}") for i in range(2)]
        hh = [cp.tile([128, SCANW], bf16, name=f"hh{i}") for i in range(2)]
        y = cp.tile([128, GI * L], bf16)
        tmp1 = cp.tile([128, GM * L], bf16)
        xn = cp.tile([128, GM * L], bf16)
        hffn = cp.tile([128, 12 * L], bf16)
        q_sb = cp.tile([128, NH, L], bf16)
        k_sb = cp.tile([128, NH, LT], bf16)
        vt_sb = cp.tile([LT, NH * HP], bf16)
        pt_sb = cp.tile([LT, NH * L], bf16)
        p_all = cp.tile([128, NH * 2 * LT], bf16)
        rs_all = cp.tile([128, NH * 2], f32)
        ri_all = cp.tile([128, NH * 2], f32)
        avt_sb = cp.tile([128, NH, L], bf16)
        mod_all = cp.tile([128, DEPTH * 36], f32)
        modx_all = cp.tile([128, DEPTH * 12], f32)
        xdbl_sb = cp.tile([48, L], bf16)
        bc_sb = cp.tile([32, L], bf16)
        bvb = cp.tile([LT, NH * HP], bf16)
        modx = cp.tile([128, 12], f32)
        aneg = cp.tile([128, DEPTH * 192], f32)
        fl_e = cp.tile([128, GM * LT], bf16)
        silu_c = cp.tile([128, 6], bf16)
        stat = cp.tile([1, 2 * L], f32)
        stat2 = cp.tile([1, L], f32)
        small = cp.tile([128, 16], f32)      # ang etc
        smalli = cp.tile([128, 2], mybir.dt.int32)
        temb_c = cp.tile([128, 2], bf16)
        cvec = cp.tile([1, DM], f32)
        fmod_sc = cp.tile([128, 12], f32)

        # consts / biases resident
        C = {}
        for nm in ["dtb_sc", "convw_sc", "convb_sc", "Dp_sc", "adab_sc",
                   "bq_sc", "bk_sc", "aob_sc", "fb1_sc", "fb2_sc", "finadab_sc", "finb_sc",
                   "id128", "id128f", "ones_col", "ones_colb", "ones_row"]:
            C[nm] = cp.tile(list(W[nm].shape), W[nm].dtype, name="c_" + nm)
            DMA.dma_start(out=C[nm][:], in_=W[nm][:])

        # zero the pad columns once; interiors are always written strided
        for tl in dA + dB + [u_pad]:
            GPS.memset(tl[:], 0.0)


        # ---------------- preamble ----------------
        with tc.tile_pool(name="pre", bufs=1) as pre:
            from contextlib import ExitStack as _ES
            _es = _ES()
            psv = _es.enter_context(tc.tile_pool(name="psv", bufs=2, space="PSUM"))
            for nm in ["xb_sc", "flb_sc", "flpos_sc", "ftp", "fc_cols"]:
                C[nm] = pre.tile(list(W[nm].shape), W[nm].dtype, tag="p_" + nm, name="c_" + nm)
                DMA.dma_start(out=C[nm][:], in_=W[nm][:])
            for nm in ["tb1_r", "fcb1_r", "tb2_r", "fcb2_r"]:
                C[nm] = pre.tile(list(W[nm].shape), W[nm].dtype, tag="pvb", name="c_" + nm)
                DMA.dma_start(out=C[nm][:], in_=W[nm][:])
            for i in range(8):
                alg = pre.tile([128, 288], f32, tag="pal", name=f"alg{i}")
                DMA.dma_start(out=alg[:], in_=W["alog_sc"][:, i * 288:(i + 1) * 288])
                ACT.activation(out=aneg[:, i * 288:(i + 1) * 288], in_=alg[:], func=Act.Exp)
            DVE.tensor_scalar_mul(out=aneg[:], in0=aneg[:], scalar1=-1.0)
            # time embedding: ang = t*freqs mod 2pi; temb = [cos ang, sin ang]
            DVE.tensor_tensor(out=small[:, 0:1], in0=C["ftp"][:, 0:1], in1=C["ftp"][:, 1:2], op=Alu.mult)
            TWO_PI = 2 * math.pi
            # cos(ang)=sin(ang+pi/2); reduce each argument into [-pi, pi]
            DVE.tensor_scalar_add(out=small[:, 1:2], in0=small[:, 0:1], scalar1=math.pi / 2)
            for j, col in ((0, 1), (1, 0)):  # j=0: cos arg; j=1: sin arg
                src_c = 1 - col  # small col holding the argument
                a_ = small[:, src_c + 0:src_c + 1] if False else None
            for j, srccol in ((0, 1), (1, 0)):
                arg = small[:, srccol:srccol + 1]
                DVE.tensor_scalar_mul(out=small[:, 4 + j:5 + j], in0=arg, scalar1=1.0 / TWO_PI)
                DVE.tensor_copy(out=smalli[:, j:j + 1], in_=small[:, 4 + j:5 + j])
                DVE.tensor_copy(out=small[:, 6 + j:7 + j], in_=smalli[:, j:j + 1])
                DVE.scalar_tensor_tensor(out=small[:, 8 + j:9 + j], in0=small[:, 6 + j:7 + j],
                                         scalar=-TWO_PI, in1=arg, op0=Alu.mult, op1=Alu.add)
                DVE.tensor_scalar(out=small[:, 10 + j:11 + j], in0=small[:, 8 + j:9 + j],
                                  scalar1=math.pi, scalar2=None, op0=Alu.is_gt)
                DVE.scalar_tensor_tensor(out=small[:, 12 + j:13 + j], in0=small[:, 10 + j:11 + j],
                                         scalar=-TWO_PI, in1=small[:, 8 + j:9 + j],
                                         op0=Alu.mult, op1=Alu.add)
                ACT.activation(out=temb_c[:, j:j + 1], in_=small[:, 12 + j:13 + j], func=Act.Sin)

            tw1 = wb.tile([128, 2 * DM], bf16, tag="wb")
            load_kpm(tw1, W["tw1T"][:], 2)
            h1p = psv.tile([1, DM], f32, tag="vec")
            for k in range(2):
                for lo, hi in ((0, 512), (512, 768)):
                    MM.matmul(out=h1p[:, lo:hi],
                              lhsT=temb_c[:, k:k + 1],
                              rhs=tw1[:, k * DM + lo:k * DM + hi],
                              start=(k == 0), stop=(k == 1))
            h1 = pre.tile([1, DM], f32, tag="pv")
            for lo, hi in ((0, 512), (512, 768)):
                DVE.tensor_tensor(out=h1[:, lo:hi], in0=h1p[:, lo:hi], in1=C["tb1_r"][:, lo:hi], op=Alu.add)
            ACT.activation(out=h1[:], in_=h1[:], func=Act.Silu)
            GPS.dma_start(out=scr_b1[:], in_=h1[:])
            h1f = pre.tile([128, 6], f32, tag="pcf")
            GPS.dma_start(out=h1f[:], in_=scr_b1[0, :].rearrange("(g p) -> p g", g=6))
            h1c = pre.tile([128, 6], bf16, tag="pc")
            ACT.activation(out=h1c[:], in_=h1f[:], func=Act.Copy)

            h2p = psv.tile([1, DM], f32, tag="vec")
            for k in range(16):
                if k % 4 == 0:
                    fcw1c = wb.tile([128, 4 * DM], bf16, tag="wb", name=f"fcw1_{k // 4}")
                    load_kpm(fcw1c, W["fcw1T"][k:k + 4], 4)
                for lo, hi in ((0, 512), (512, 768)):
                    MM.matmul(out=h2p[:, lo:hi],
                              lhsT=C["fc_cols"][:, k:k + 1],
                              rhs=fcw1c[:, (k % 4) * DM + lo:(k % 4) * DM + hi],
                              start=(k == 0), stop=(k == 15))
            h2 = pre.tile([1, DM], f32, tag="pv")
            for lo, hi in ((0, 512), (512, 768)):
                DVE.tensor_tensor(out=h2[:, lo:hi], in0=h2p[:, lo:hi], in1=C["fcb1_r"][:, lo:hi], op=Alu.add)
            ACT.activation(out=h2[:], in_=h2[:], func=Act.Silu)
            GPS.dma_start(out=scr_b2[:], in_=h2[:])
            h2f = pre.tile([128, 6], f32, tag="pcf2")
            GPS.dma_start(out=h2f[:], in_=scr_b2[0, :].rearrange("(g p) -> p g", g=6))
            h2c = pre.tile([128, 6], bf16, tag="pc3")
            ACT.activation(out=h2c[:], in_=h2f[:], func=Act.Copy)

            # c = tw2@h1 + fcw2@h2 + tb2 + fcb2 ; silu; scatter
            cp_ps = psv.tile([1, DM], f32, tag="vec")
            nmm = 0
            for hsrc, wnm in ((h1c, "tw2T"), (h2c, "fcw2T")):
                for k in range(6):
                    if k % 3 == 0:
                        wc = wb.tile([128, 3 * DM], bf16, tag="wb", name=f"cw_{wnm}_{k}")
                        load_kpm(wc, W[wnm][k:k + 3], 3)
                    for lo, hi in ((0, 512), (512, 768)):
                        MM.matmul(out=cp_ps[:, lo:hi],
                                  lhsT=hsrc[:, k:k + 1],
                                  rhs=wc[:, (k % 3) * DM + lo:(k % 3) * DM + hi],
                                  start=(nmm == 0), stop=(nmm == 11))
                    nmm += 1
            for lo, hi in ((0, 512), (512, 768)):
                DVE.tensor_tensor(out=cvec[:, lo:hi], in0=cp_ps[:, lo:hi], in1=C["tb2_r"][:, lo:hi], op=Alu.add)
            DVE.tensor_tensor(out=cvec[:], in0=cvec[:], in1=C["fcb2_r"][:], op=Alu.add)
            ACT.activation(out=cvec[:], in_=cvec[:], func=Act.Silu)
            GPS.dma_start(out=scr_b3[:], in_=cvec[:])
            scf32 = pre.tile([128, 6], f32, tag="pc4")
            GPS.dma_start(out=scf32[:], in_=scr_b3[0, :].rearrange("(g p) -> p g", g=6))
            ACT.activation(out=silu_c[:], in_=scf32[:], func=Act.Copy)

            # fl_e = flw@fl + flb + flpos
            flsb = pre.tile([128, 16, LT], bf16, tag="pfl")
            GPS.dma_start(out=flsb[:], in_=W["flT"][:].rearrange("k p m -> p k m"))
            _es.close()
            _es = _ES()
            psfl = _es.enter_context(tc.tile_pool(name="psfl", bufs=1, space="PSUM"))
            fps = [psfl.tile([128, LT], f32, tag=f"fl{m}", name=f"flp{m}") for m in range(6)]
            for k in range(16):
                if k % 4 == 0:
                    flwc = wb.tile([128, 4 * DM], bf16, tag="wb", name=f"flw_{k // 4}")
                    load_kpm(flwc, W["flwT"][k:k + 4], 4)
                for m in range(6):
                    MM.matmul(out=fps[m][:],
                              lhsT=flwc[:, (k % 4) * DM + m * 128:(k % 4) * DM + (m + 1) * 128],
                              rhs=flsb[:, k, :], start=(k == 0), stop=(k == 15))
            for m in range(6):
                t_ = pre.tile([128, LT], f32, tag="pt2", name=f"fle{m}")
                ACT.activation(out=t_[:], in_=fps[m][:], func=Act.Identity, bias=C["flb_sc"][:, m:m + 1])
                DVE.tensor_tensor(out=fl_e[:, m * LT:(m + 1) * LT], in0=t_[:],
                                  in1=C["flpos_sc"][:, m, :], op=Alu.add)

            # x embedding
            _es.close()
            _es = _ES()
            ps1 = _es.enter_context(tc.tile_pool(name="ps1", bufs=2, space="PSUM"))
            xw = wb.tile([128, 2 * DM], bf16, tag="wb")
            load_kpm(xw, W["xwT"][:], 2)
            xsb = pre.tile([128, 2, L], bf16, tag="pfl2")
            GPS.dma_start(out=xsb[:], in_=W["xT"][:].rearrange("k p m -> p k m"))
            for m in range(6):
                xp = ps1.tile([128, L], f32, tag="mm")
                for k in range(2):
                    MM.matmul(out=xp[:], lhsT=xw[:, k * DM + m * 128:k * DM + (m + 1) * 128],
                              rhs=xsb[:, k, :], start=(k == 0), stop=(k == 1))
                ACT.activation(out=x[:, m * L:(m + 1) * L], in_=xp[:],
                               func=Act.Identity, bias=C["xb_sc"][:, m:m + 1])
            _es.close()

        # ---------------- layers ----------------
        x3 = lambda g: x[:, g * L:(g + 1) * L]
        xn3 = lambda g: xn[:, g * L:(g + 1) * L]
        dt3 = dt_t[:].rearrange("p (g t) -> p g t", g=GI)
        du3 = du[:].rearrange("p (g t) -> p g t", g=GI)
        y3 = y[:].rearrange("p (g t) -> p g t", g=GI)
        uc3i = uc[:].rearrange("p (g s) -> p g s", g=GI)[:, :, 4:SEG]
        up3 = u_pad[:].rearrange("p (g s) -> p g s", g=GI)

        def ln_block(l, psA, psS, psB, scale_col, shift_col):
            """LayerNorm of x -> xn (bf16), optionally modulated."""
            ACT.activation(out=tmp1[:], in_=x[:], func=Act.Square)
            st = psS.tile([1, 512], f32, tag="st", name=f"st{l}")
            for g in range(GM):
                MM.matmul(out=st[:, 0:L], lhsT=C["ones_col"][:], rhs=x3(g),
                          start=(g == 0), stop=(g == GM - 1))
            for g in range(GM):
                MM.matmul(out=st[:, L:2 * L], lhsT=C["ones_colb"][:],
                          rhs=tmp1[:, g * L:(g + 1) * L],
                          start=(g == 0), stop=(g == GM - 1))
            ACT.activation(out=stat[:, 0:L], in_=st[:, 0:L], func=Act.Copy, scale=1.0 / DM)
            ACT.activation(out=stat2[:], in_=stat[:, 0:L], func=Act.Square)
            DVE.scalar_tensor_tensor(out=stat2[:], in0=st[:, L:2 * L], scalar=1.0 / DM,
                                     in1=stat2[:], op0=Alu.mult, op1=Alu.subtract)
            ACT.activation(out=stat2[:], in_=stat2[:], func=Act.Sqrt, bias=1e-6)
            DVE.reciprocal(out=stat[:, L:2 * L], in_=stat2[:])
            bc = psB.tile([128, 512], f32, tag="bc", name=f"bc{l}")
            MM.matmul(out=bc[:], lhsT=C["ones_row"][:], rhs=stat[:, 0:512])
            for g in range(GM):
                DVE.tensor_tensor(out=tmp1[:, g * L:(g + 1) * L], in0=x3(g),
                                  in1=bc[:, 0:L], op=Alu.subtract)
                if scale_col is None:
                    DVE.tensor_tensor(out=xn3(g), in0=tmp1[:, g * L:(g + 1) * L],
                                      in1=bc[:, L:2 * L], op=Alu.mult)
                else:
                    DVE.tensor_tensor(out=tmp1[:, g * L:(g + 1) * L],
                                      in0=tmp1[:, g * L:(g + 1) * L],
                                      in1=bc[:, L:2 * L], op=Alu.mult)
                    DVE.scalar_tensor_tensor(
                        out=xn3(g), in0=tmp1[:, g * L:(g + 1) * L],
                        scalar=scale_col[:, g:g + 1],
                        in1=shift_col[:, g:g + 1].broadcast_to([128, L]),
                        op0=Alu.mult, op1=Alu.add)

        def ada_block(l, psM):
            """adaLN modulation matvecs for layer l -> mod_all/modx_all slices.

            Emitted two layers early so PE/DMA fill the scan window."""
            for r in range(2):
                for si, (lo, wdt) in enumerate(
                        ((0, 512), (512, 512), (1024, 512), (1536, 512), (2048, 256))):
                    ps = psM.tile([1, 512], f32, tag="m", name=f"mps{l}_{r}_{si}")
                    for k in range(6):
                        ah = ws.tile([128, 768], bf16, tag="ws", name=f"ada{l}_{r}_{si}_{k}")
                        DMA.dma_start(out=ah[:, 0:wdt],
                                      in_=W["adaT"][l, k][:, r * 2304 + lo:r * 2304 + lo + wdt])
                        MM.matmul(out=ps[:, 0:wdt], lhsT=silu_c[:, k:k + 1],
                                  rhs=ah[:, 0:wdt],
                                  start=(k == 0), stop=(k == 5))
                    sg = stg.tile([1, 512], f32, tag="stg", name=f"sg{l}_{r}_{si}")
                    ACT.activation(out=sg[:, 0:wdt], in_=ps[:, 0:wdt], func=Act.Copy)
                    GPS.dma_start(out=scr_mod[l:l + 1, r * 2304 + lo:r * 2304 + lo + wdt],
                                  in_=sg[:, 0:wdt])
            mf = mod_all[:, l * 36:(l + 1) * 36]
            GPS.dma_start(out=mf.rearrange("p (bl g) -> p bl g", bl=6),
                          in_=scr_mod[l, :].rearrange("(bl g p) -> p bl g", bl=6, g=6))
            DVE.tensor_tensor(out=mf, in0=mf,
                              in1=C["adab_sc"][:, l * 36:(l + 1) * 36], op=Alu.add)
            DVE.tensor_scalar_add(out=modx_all[:, l * 12:l * 12 + 6],
                                  in0=mod_all[:, l * 36 + 6:l * 36 + 12], scalar1=1.0)
            DVE.tensor_scalar_add(out=modx_all[:, l * 12 + 6:l * 12 + 12],
                                  in0=mod_all[:, l * 36 + 24:l * 36 + 30], scalar1=1.0)

        for l in range(2):
            with tc.tile_pool(name=f"psMp{l}", bufs=2, space="PSUM") as psM:
                ada_block(l, psM)

        for l in range(depth):
            mod_f = mod_all[:, l * 36:(l + 1) * 36]
            modx_l = modx_all[:, l * 12:(l + 1) * 12]

            # ---- mamba ----
            with tc.tile_pool(name=f"psA{l}", bufs=2, space="PSUM") as psA, \
                 tc.tile_pool(name=f"psS{l}", bufs=1, space="PSUM") as psS, \
                 tc.tile_pool(name=f"psB{l}", bufs=1, space="PSUM") as psB, \
                 tc.tile_pool(name=f"psV{l}", bufs=1, space="PSUM") as psV, \
                 tc.tile_pool(name=f"psM{l}", bufs=2, space="PSUM") as psM, \
                 tc.tile_pool(name=f"psX{l}", bufs=1, space="PSUM") as psX:
                ln_block(10 * l, psA, psS, psB, modx_l[:, 0:6], mod_f[:, 0:6])

                for mb in range(4):
                    wts = []
                    for k in range(6):
                        wi = ws.tile([128, 768], bf16, tag="ws", name=f"inw{l}_{mb}_{k}")
                        DMA.dma_start(out=wi[:], in_=W["in_wT"][l, k][:, mb * 768:(mb + 1) * 768])
                        wts.append(wi)
                    for mi in range(6):
                        m = mb * 6 + mi
                        ps = psA.tile([128, L], f32, tag="mm", name=f"ip{l}_{m}")
                        for k in range(6):
                            MM.matmul(out=ps[:], lhsT=wts[k][:, mi * 128:(mi + 1) * 128],
                                      rhs=xn3(k), start=(k == 0), stop=(k == 5))
                        if m < 12:
                            ACT.activation(out=up3[:, m, 4:SEG], in_=ps[:], func=Act.Copy)
                        else:
                            ACT.activation(out=z[:, (m - 12) * L:(m - 11) * L], in_=ps[:], func=Act.Silu)

                # ---- attention K/V: depend only on fl_e, so compute them here
                # so PE has work queued during the scan window ----
                wk_ = []
                for i in range(2):
                    t_ = wb.tile([128, 3 * NH * HP], bf16, tag="wb", name=f"wk{l}_{i}")
                    load_kpm(t_, W["wkT"][l, 3 * i:3 * i + 3], 3)
                    wk_.append(t_)
                for h in range(NH):
                    ps = psA.tile([128, L], f32, tag="mm", name=f"kp{l}_{h}")
                    for k in range(6):
                        MM.matmul(out=ps[:, 0:LT],
                                  lhsT=wk_[k // 3][:, (k % 3) * NH * HP + h * HP:(k % 3) * NH * HP + (h + 1) * HP],
                                  rhs=fl_e[:, k * LT:(k + 1) * LT], start=(k == 0), stop=(k == 5))
                    ACT.activation(out=k_sb[:, h, :], in_=ps[:, 0:LT], func=Act.Identity,
                                   bias=C["bk_sc"][:, l * 8 + h:l * 8 + h + 1])
                wv_ = []
                for i in range(2):
                    t_ = wb.tile([128, 3 * NH * HP], bf16, tag="wb", name=f"wv{l}_{i}")
                    load_kpm(t_, W["wvT"][l, 3 * i:3 * i + 3], 3)
                    wv_.append(t_)
                GPS.dma_start(out=bvb[:], in_=W["bv_pad"][l].partition_broadcast(LT))
                for fo in range(2):
                    vt_ps = psV.tile([LT, 512], f32, tag="vt", name=f"vtp{l}_{fo}")
                    for k in range(6):
                        MM.matmul(out=vt_ps[:],
                                  lhsT=fl_e[:, k * LT:(k + 1) * LT],
                                  rhs=wv_[k // 3][:, (k % 3) * NH * HP + fo * 512:(k % 3) * NH * HP + (fo + 1) * 512],
                                  start=(k == 0), stop=(k == 5))
                    DVE.tensor_tensor(out=vt_sb[:, fo * 512:(fo + 1) * 512],
                                      in0=vt_ps[:],
                                      in1=bvb[:, fo * 512:(fo + 1) * 512], op=Alu.add)

                # causal depthwise conv (acc in y, f32) + silu -> uc
                cw4 = C["convw_sc"][:].rearrange("p (a g k) -> p a g k", a=DEPTH, g=12)
                for k in range(4):
                    wkb = cw4[:, l, :, k].unsqueeze(2).broadcast_to([128, GI, L])
                    sh = up3[:, :, 1 + k:1 + k + L]
                    if k == 0:
                        GPS.tensor_tensor(out=y3, in0=sh, in1=wkb, op=Alu.mult)
                    else:
                        GPS.tensor_tensor(out=hh[0][:].rearrange("p (g s) -> p g s", g=GI)[:, :, 4:SEG],
                                          in0=sh, in1=wkb, op=Alu.mult)
                        GPS.tensor_tensor(out=y3, in0=y3,
                                          in1=hh[0][:].rearrange("p (g s) -> p g s", g=GI)[:, :, 4:SEG],
                                          op=Alu.add)
                for g in range(GI):
                    ACT.activation(out=uc3i[:, g, :], in_=y[:, g * L:(g + 1) * L],
                                   func=Act.Silu, bias=C["convb_sc"][:, l * 12 + g:l * 12 + g + 1])

                # xproj -> xdbl [80, L]
                xpw = wb.tile([128, 1152], bf16, tag="wb", name=f"xpw{l}")
                load_kpm(xpw, W["xprojT"][l], 12)
                xd = psX.tile([96, L], f32, tag="xd", name=f"xd{l}")
                for k in range(GI):
                    MM.matmul(out=xd[:], lhsT=xpw[:, k * 96:(k + 1) * 96],
                              rhs=uc3i[:, k, :], start=(k == 0), stop=(k == GI - 1))
                ACT.activation(out=xdbl_sb[:], in_=xd[0:48, :], func=Act.Copy)
                ACT.activation(out=bc_sb[:], in_=xd[64:96, :], func=Act.Copy)
                GPS.dma_start(out=scr_bc[0, :].rearrange("(a b) -> a b", a=32), in_=bc_sb[:])
                GPS.dma_start(out=brep[:], in_=scr_bc[0, 0:NST * L].partition_broadcast(128))
                GPS.dma_start(out=crep[:], in_=scr_bc[0, NST * L:2 * NST * L].partition_broadcast(128))

                # dt = softplus(dt_w @ xdbl[:48] + dt_b)
                dtw = wb.tile([48, DI], bf16, tag="wb", name=f"dtw{l}")
                DMA.dma_start(out=dtw[:], in_=W["dt_wT"][l])
                for m in range(GI):
                    ps = psA.tile([128, L], f32, tag="mm", name=f"dtp{l}_{m}")
                    MM.matmul(out=ps[:], lhsT=dtw[:, m * 128:(m + 1) * 128], rhs=xdbl_sb[:])
                    sl_ = dt_t[:, m * L:(m + 1) * L]
                    ACT.activation(out=sl_, in_=ps[:], func=Act.Exp,
                                   bias=C["dtb_sc"][:, l * 12 + m:l * 12 + m + 1])
                # softplus tail as two full-width ops: keeps the Exp/Ln table
                # sets from reloading once per m-slice
                DVE.tensor_scalar_add(out=dt_t[:], in0=dt_t[:], scalar1=1.0)
                ACT.activation(out=dt_t[:], in_=dt_t[:], func=Act.Ln)
                DVE.tensor_tensor(out=du3, in0=dt3, in1=uc3i, op=Alu.mult)

                # modulation matvecs for layer l+2: PE+DMA work that overlaps
                # the DVE-bound scan below
                if l + 2 < depth:
                    ada_block(l + 2, psM)

                # selective scan over n (state dim), pad cols reset state
                for n in range(NST):
                    sl = n % 2
                    dAi = dA[sl][:].rearrange("p (g s) -> p g s", g=GI)[:, :, 4:SEG]
                    dBi = dB[sl][:].rearrange("p (g s) -> p g s", g=GI)[:, :, 4:SEG]
                    hi = hh[sl][:].rearrange("p (g s) -> p g s", g=GI)[:, :, 4:SEG]
                    ACT.activation(out=dAi, in_=dt3, func=Act.Exp,
                                   scale=aneg[:, l * 192 + n:l * 192 + n + 1])
                    DVE.tensor_tensor(out=dBi, in0=du3,
                                      in1=brep[:, n * L:(n + 1) * L].unsqueeze(1).broadcast_to([128, GI, L]),
                                      op=Alu.mult)
                    DVE.tensor_tensor_scan(out=hh[sl][:], data0=dA[sl][:], data1=dB[sl][:],
                                           initial=0.0, op0=Alu.mult, op1=Alu.add)
                    DVE.tensor_tensor(out=hi, in0=hi,
                                      in1=crep[:, n * L:(n + 1) * L].unsqueeze(1).broadcast_to([128, GI, L]),
                                      op=Alu.mult)
                    if n == 0:
                        DVE.tensor_copy(out=y3, in_=hi)
                    else:
                        DVE.tensor_tensor(out=y3, in0=y3, in1=hi, op=Alu.add)

                # y = (y + uc*Dp) * silu(z);  out-proj; residual with gm
                for g in range(GI):
                    DVE.scalar_tensor_tensor(out=y[:, g * L:(g + 1) * L], in0=uc3i[:, g, :],
                                             scalar=C["Dp_sc"][:, l * 12 + g:l * 12 + g + 1],
                                             in1=y[:, g * L:(g + 1) * L], op0=Alu.mult, op1=Alu.add)
                DVE.tensor_tensor(out=du[:], in0=y[:], in1=z[:], op=Alu.mult)

                ow = []
                for i in range(2):
                    wi = wb.tile([128, 6 * DM], bf16, tag="wb", name=f"ow{l}_{i}")
                    load_kpm(wi, W["out_wT"][l, 6 * i:6 * i + 6], 6)
                    ow.append(wi)
                for m in range(GM):
                    ps = psA.tile([128, L], f32, tag="mm", name=f"op{l}_{m}")
                    for k in range(GI):
                        MM.matmul(out=ps[:], lhsT=ow[k // 6][:, (k % 6) * DM + m * 128:(k % 6) * DM + (m + 1) * 128],
                                  rhs=du[:, k * L:(k + 1) * L], start=(k == 0), stop=(k == GI - 1))
                    DVE.scalar_tensor_tensor(out=x3(m), in0=ps[:], scalar=mod_f[:, 12 + m:13 + m],
                                             in1=x3(m), op0=Alu.mult, op1=Alu.add)

            # ---- cross attention ----
            with tc.tile_pool(name=f"qsA{l}", bufs=2, space="PSUM") as psA, \
                 tc.tile_pool(name=f"qsS{l}", bufs=1, space="PSUM") as psS, \
                 tc.tile_pool(name=f"qsB{l}", bufs=1, space="PSUM") as psB, \
                 tc.tile_pool(name=f"qsC{l}", bufs=1, space="PSUM") as psC, \
                 tc.tile_pool(name=f"qsP{l}", bufs=1, space="PSUM") as psP:
                ln_block(10 * l + 1, psA, psS, psB, None, None)

                wq = []
                for i in range(2):
                    t_ = wb.tile([128, 3 * NH * HP], bf16, tag="wb", name=f"wq{l}_{i}")
                    load_kpm(t_, W["wqT"][l, 3 * i:3 * i + 3], 3)
                    wq.append(t_)
                for h in range(NH):
                    ps = psA.tile([128, L], f32, tag="mm", name=f"qp{l}_{h}")
                    for k in range(6):
                        MM.matmul(out=ps[:], lhsT=wq[k // 3][:, (k % 3) * NH * HP + h * HP:(k % 3) * NH * HP + (h + 1) * HP],
                                  rhs=xn3(k), start=(k == 0), stop=(k == 5))
                    ACT.activation(out=q_sb[:, h, :], in_=ps[:], func=Act.Identity,
                                   bias=C["bq_sc"][:, l * 8 + h:l * 8 + h + 1])

                for h in range(NH):
                    for tc2 in range(2):
                        idx = h * 2 + tc2
                        sc_ps = psC.tile([128, LT], f32, tag="sc", name=f"scp{l}_{idx}")
                        MM.matmul(out=sc_ps[:], lhsT=q_sb[:, h, tc2 * 128:(tc2 + 1) * 128],
                                  rhs=k_sb[:, h, :])
                        mx = small[:, 2:3]
                        DVE.tensor_reduce(out=mx, in_=sc_ps[:], axis=mybir.AxisListType.X, op=Alu.max)
                        DVE.tensor_scalar_mul(out=small[:, 3:4], in0=mx, scalar1=-SQ)
                        ACT.activation(out=p_all[:, idx * LT:(idx + 1) * LT], in_=sc_ps[:],
                                       func=Act.Exp, scale=SQ, bias=small[:, 3:4],
                                       accum_out=rs_all[:, idx:idx + 1])
                DVE.reciprocal(out=ri_all[:], in_=rs_all[:])
                for h in range(NH):
                    for tc2 in range(2):
                        idx = h * 2 + tc2
                        DVE.tensor_scalar_mul(out=p_all[:, idx * LT:(idx + 1) * LT],
                                              in0=p_all[:, idx * LT:(idx + 1) * LT],
                                              scalar1=ri_all[:, idx:idx + 1])
                        pt_ps = psP.tile([LT, 128], bf16, tag="pt", name=f"ptp{l}_{idx}")
                        MM.transpose(out=pt_ps[:], in_=p_all[:, idx * LT:(idx + 1) * LT], identity=C["id128"][:])
                        ACT.activation(out=pt_sb[:, h * L + tc2 * 128:h * L + (tc2 + 1) * 128],
                                       in_=pt_ps[:], func=Act.Copy)
                for h in range(NH):
                    ps = psA.tile([128, L], f32, tag="mm", name=f"avp{l}_{h}")
                    for tc2 in range(2):
                        MM.matmul(out=ps[:, tc2 * 128:(tc2 + 1) * 128],
                                  lhsT=vt_sb[:, h * HP:(h + 1) * HP],
                                  rhs=pt_sb[:, h * L + tc2 * 128:h * L + (tc2 + 1) * 128])
                    ACT.activation(out=avt_sb[:, h, :], in_=ps[:], func=Act.Copy)

                ao = []
                for i in range(2):
                    t_ = wb.tile([128, 4 * DM], bf16, tag="wb", name=f"ao{l}_{i}")
                    load_kpm(t_, W["aoT"][l, 4 * i:4 * i + 4], 4)
                    ao.append(t_)
                for m in range(GM):
                    ps = psA.tile([128, L], f32, tag="mm", name=f"aop{l}_{m}")
                    for k in range(NH):
                        MM.matmul(out=ps[:], lhsT=ao[k // 4][:, (k % 4) * DM + m * 128:(k % 4) * DM + (m + 1) * 128],
                                  rhs=avt_sb[:, k, :], start=(k == 0), stop=(k == NH - 1))
                    DVE.scalar_tensor_tensor(out=x3(m), in0=ps[:], scalar=C["aob_sc"][:, l * 6 + m:l * 6 + m + 1],
                                             in1=x3(m), op0=Alu.add, op1=Alu.add)

            # ---- FFN ----
            with tc.tile_pool(name=f"fsA{l}", bufs=2, space="PSUM") as psA, \
                 tc.tile_pool(name=f"fsS{l}", bufs=1, space="PSUM") as psS, \
                 tc.tile_pool(name=f"fsB{l}", bufs=1, space="PSUM") as psB:
                ln_block(10 * l + 2, psA, psS, psB, modx_l[:, 6:12], mod_f[:, 18:24])
                for half in range(2):
                    for mb in range(2):
                        wts = []
                        for k in range(6):
                            wi = ws.tile([128, 768], bf16, tag="ws", name=f"f1w{l}_{half}_{mb}_{k}")
                            DMA.dma_start(out=wi[:],
                                          in_=W["fw1T"][l, k][:, (half * 2 + mb) * 768:(half * 2 + mb + 1) * 768])
                            wts.append(wi)
                        for mi in range(6):
                            m = half * 12 + mb * 6 + mi
                            ml = mb * 6 + mi
                            ps = psA.tile([128, L], f32, tag="mm", name=f"f1p{l}_{m}")
                            for k in range(6):
                                MM.matmul(out=ps[:], lhsT=wts[k][:, mi * 128:(mi + 1) * 128],
                                          rhs=xn3(k), start=(k == 0), stop=(k == 5))
                            ACT.activation(out=hffn[:, ml * L:(ml + 1) * L], in_=ps[:], func=Act.Gelu,
                                           bias=C["fb1_sc"][:, l * 24 + m:l * 24 + m + 1])
                    f2 = []
                    for i in range(2):
                        t_ = wb.tile([128, 6 * DM], bf16, tag="wb", name=f"f2{l}_{half}_{i}")
                        load_kpm(t_, W["fw2T"][l, half * 12 + 6 * i:half * 12 + 6 * i + 6], 6)
                        f2.append(t_)
                    for m in range(GM):
                        ps = psA.tile([128, L], f32, tag="mm", name=f"f2p{l}_{half}_{m}")
                        for k in range(12):
                            MM.matmul(out=ps[:], lhsT=f2[k // 6][:, (k % 6) * DM + m * 128:(k % 6) * DM + (m + 1) * 128],
                                      rhs=hffn[:, k * L:(k + 1) * L], start=(k == 0), stop=(k == 11))
                        if half == 0:
                            ACT.activation(out=tmp1[:, m * 256:(m + 1) * 256], in_=ps[:], func=Act.Copy)
                        else:
                            DVE.tensor_tensor(out=tmp1[:, m * 256:(m + 1) * 256],
                                              in0=tmp1[:, m * 256:(m + 1) * 256], in1=ps[:], op=Alu.add)
                            DVE.tensor_scalar(out=tmp1[:, m * 256:(m + 1) * 256],
                                              in0=tmp1[:, m * 256:(m + 1) * 256],
                                              scalar1=C["fb2_sc"][:, l * 6 + m:l * 6 + m + 1],
                                              scalar2=mod_f[:, 30 + m:31 + m], op0=Alu.add, op1=Alu.mult)
                            DVE.tensor_tensor(out=x3(m), in0=x3(m), in1=tmp1[:, m * 256:(m + 1) * 256], op=Alu.add)

        # ---------------- final ----------------
        with tc.tile_pool(name="fin", bufs=1) as fin, \
             tc.tile_pool(name="zsA", bufs=2, space="PSUM") as psA, \
             tc.tile_pool(name="zsS", bufs=1, space="PSUM") as psS, \
             tc.tile_pool(name="zsB", bufs=1, space="PSUM") as psB, \
             tc.tile_pool(name="zsV", bufs=3, space="PSUM") as psV:
            fm_ps = [psV.tile([1, 512], f32, tag="fm5", name=f"fmps{s}") for s in range(3)]
            for k in range(6):
                fad = fin.tile([128, 2 * DM], bf16, tag="fw", name=f"fad{k}")
                load_kpm(fad, W["finadaT"][k:k + 1], 1)
                for s in range(3):
                    MM.matmul(out=fm_ps[s][:],
                              lhsT=silu_c[:, k:k + 1],
                              rhs=fad[:, s * 512:(s + 1) * 512],
                              start=(k == 0), stop=(k == 5))
            for s in range(3):
                sg = stg.tile([1, 512], f32, tag="stg", name=f"fsg{s}")
                ACT.activation(out=sg[:], in_=fm_ps[s][:], func=Act.Copy)
                GPS.dma_start(out=scr_fm[:, s * 512:(s + 1) * 512], in_=sg[:])
            GPS.dma_start(out=fmod_sc[:].rearrange("p (bl g) -> p bl g", bl=2),
                          in_=scr_fm[0, :].rearrange("(bl g p) -> p bl g", bl=2, g=6))
            DVE.tensor_tensor(out=fmod_sc[:], in0=fmod_sc[:], in1=C["finadab_sc"][:], op=Alu.add)
            DVE.tensor_scalar_add(out=modx[:, 0:6], in0=fmod_sc[:, 6:12], scalar1=1.0)
            ln_block(999, psA, psS, psB, modx[:, 0:6], fmod_sc[:, 0:6])

            xo_sb = fin.tile([128, 2, CIN], f32, tag="fxo")
            outT = fin.tile([128, 2 * CIN], f32, tag="fot")
            fw = fin.tile([128, 6 * CIN], bf16, tag="fw2")
            load_kpm(fw, W["finT"][:], 6)
            for m in range(2):
                ps = psA.tile([128, L], f32, tag="mm", name=f"fop{m}")
                for k in range(6):
                    MM.matmul(out=ps[:], lhsT=fw[:, k * CIN + m * 128:k * CIN + (m + 1) * 128],
                              rhs=xn3(k), start=(k == 0), stop=(k == 5))
                ACT.activation(out=xo_sb[:, m, :], in_=ps[:], func=Act.Identity,
                               bias=C["finb_sc"][:, m:m + 1])
            # transpose [ch, t] -> [t, ch] and store
            for tc2 in range(2):
                for m in range(2):
                    tp = psA.tile([128, 128], f32, tag="mm", name=f"tp{tc2}_{m}")
                    MM.transpose(out=tp[:], in_=xo_sb[:, m, tc2 * 128:(tc2 + 1) * 128],
                                 identity=C["id128f"][:])
                    ACT.activation(out=outT[:, tc2 * CIN + m * 128:tc2 * CIN + (m + 1) * 128],
                                   in_=tp[:], func=Act.Copy)
            GPS.dma_start(out=out_d[:].rearrange("(a p) c -> p a c", a=2),
                          in_=outT[:].rearrange("p (a c) -> p a c", a=2))
    nc.finalize()
    # walrus' verifier rejects leftover unused framework registers with
    # reg_id=-1; give each a harmless unique id per engine.
    from collections import defaultdict
    nxt = defaultdict(int)
    for fn in nc.m.functions:
        for a in fn.allocations:
            if getattr(a, "reg_id", None) == -1:
                eng = str(getattr(a, "engine", "?"))
                n = getattr(a, "num_physical_regs", None) or 1
                if n > 1 and nxt[eng] % 2:
                    nxt[eng] += 1
                a.reg_id = nxt[eng]
                nxt[eng] += n
    return nc


_CACHE = {}


def kernel(**inputs):
    depth = DEPTH
    if "nc" not in _CACHE:
        _CACHE["nc"] = build_nc(depth)
    nc = _CACHE["nc"]
    shared = prep_shared(inputs)
    in_maps = []
    for b in range(N_CORES):
        m = dict(shared)
        m.update(prep_core(inputs, b))
        in_maps.append(m)
    res = run_bass_kernel_spmd(nc, in_maps, list(range(N_CORES)))
    out = np.stack([np.asarray(res.results[b]["out"], np.float32) for b in range(N_CORES)])
    return out


def kernel_profiled(**inputs):
    if "nc" not in _CACHE:
        _CACHE["nc"] = build_nc(DEPTH)
    nc = _CACHE["nc"]
    shared = prep_shared(inputs)
    in_maps = []
    for b in range(N_CORES):
        m = dict(shared)
        m.update(prep_core(inputs, b))
        in_maps.append(m)
    res = run_bass_kernel_spmd(nc, in_maps, list(range(N_CORES)), trace=True)
    out = np.stack([np.asarray(res.results[b]["out"], np.float32) for b in range(N_CORES)])
    return out, res.exec_time_ns



# revision 20
# speedup vs baseline: 1.1948x; 1.0154x over previous
import sys

sys.path.insert(0, "/opt/trn_rl_repo")
import math

import numpy as np
import ml_dtypes

from concourse import bass, bacc, mybir
from concourse import tile
from concourse.bass_utils import run_bass_kernel_spmd

BF = ml_dtypes.bfloat16
bf16 = mybir.dt.bfloat16
f32 = mybir.dt.float32
Alu = mybir.AluOpType
Act = mybir.ActivationFunctionType

B, L, CIN, COND, DM, DEPTH = 8, 256, 256, 2048, 768, 12
NST, DCONV, DI, DTR = 16, 4, 1536, 48
NH, HD, LT, FREQ = 8, 96, 35, 256
GM, GI = DM // 128, DI // 128          # 6, 12
SEG = L + 4                            # 260, 4 zero pad cols reset scan state
SCANW = GI * SEG                       # 3120
HP = 128                               # padded head dim
SQ = 1.0 / math.sqrt(HD)
N_CORES = 8


def _bf(a):
    return np.ascontiguousarray(a, dtype=np.float32).astype(BF)


def _f(a):
    return np.ascontiguousarray(a, dtype=np.float32)


def prep_shared(inp):
    """Host-side layout/dtype staging of the weights (shared by all cores)."""
    d = {}
    d["xwT"] = _bf(inp["xw"].T.reshape(2, 128, DM))
    d["tw1T"] = _bf(inp["tw1"].T.reshape(2, 128, DM))
    d["tw2T"] = _bf(inp["tw2"].T.reshape(6, 128, DM))
    d["fcw1T"] = _bf(inp["fcw1"].T.reshape(16, 128, DM))
    d["fcw2T"] = _bf(inp["fcw2"].T.reshape(6, 128, DM))
    d["flwT"] = _bf(inp["flw"].T.reshape(16, 128, DM))
    d["adaT"] = _bf(np.ascontiguousarray(inp["ada_w"].transpose(0, 2, 1)).reshape(DEPTH, 6, 128, 6 * DM))
    d["in_wT"] = _bf(np.ascontiguousarray(inp["in_w"].transpose(0, 2, 1)).reshape(DEPTH, 6, 128, 2 * DI))
    xpt = np.ascontiguousarray(inp["xproj_w"].transpose(0, 2, 1)).astype(np.float32)  # [12,1536,80]
    xpp = np.zeros((DEPTH, DI, 96), np.float32)
    xpp[:, :, 0:48] = xpt[:, :, 0:48]
    xpp[:, :, 64:96] = xpt[:, :, 48:80]
    d["xprojT"] = _bf(xpp.reshape(DEPTH, 12, 128, 96))
    d["dt_wT"] = _bf(np.ascontiguousarray(inp["dt_w"].transpose(0, 2, 1)))          # [12,48,1536]
    d["out_wT"] = _bf(np.ascontiguousarray(inp["out_w"].transpose(0, 2, 1)).reshape(DEPTH, 12, 128, DM))
    qkv = inp["qkv_w"]
    wq, wk, wv = qkv[:, :DM], qkv[:, DM:2 * DM], qkv[:, 2 * DM:]
    for nm, w in (("wqT", wq), ("wkT", wk), ("wvT", wv)):
        wt = np.ascontiguousarray(w.transpose(0, 2, 1))                              # [12,768,768]
        pad = np.zeros((DEPTH, DM, NH * HP), np.float32)
        for h in range(NH):
            pad[:, :, h * HP:h * HP + HD] = wt[:, :, h * HD:(h + 1) * HD]
        d[nm] = _bf(pad.reshape(DEPTH, 6, 128, NH * HP))
    aot = np.ascontiguousarray(inp["ao_w"].transpose(0, 2, 1))                       # [12,768(dv),768]
    aop = np.zeros((DEPTH, NH * HP, DM), np.float32)
    for h in range(NH):
        aop[:, h * HP:h * HP + HD] = aot[:, h * HD:(h + 1) * HD]
    d["aoT"] = _bf(aop.reshape(DEPTH, 8, 128, DM))
    d["fw1T"] = _bf(np.ascontiguousarray(inp["fw1"].transpose(0, 2, 1)).reshape(DEPTH, 6, 128, 4 * DM))
    d["fw2T"] = _bf(np.ascontiguousarray(inp["fw2"].transpose(0, 2, 1)).reshape(DEPTH, 24, 128, DM))
    d["finadaT"] = _bf(inp["fin_ada_w"].T.reshape(6, 128, 2 * DM))
    d["finT"] = _bf(inp["fin_w"].T.reshape(6, 128, CIN))

    # per-partition scatters (fp32), layout [128, ...]
    d["xb_sc"] = _f(inp["xb"].reshape(6, 128).T)
    d["flb_sc"] = _f(inp["flb"].reshape(6, 128).T)
    d["tb1_r"] = _f(inp["tb1"].reshape(1, DM))
    d["tb2_r"] = _f(inp["tb2"].reshape(1, DM))
    d["fcb1_r"] = _f(inp["fcb1"].reshape(1, DM))
    d["fcb2_r"] = _f(inp["fcb2"].reshape(1, DM))
    d["flpos_sc"] = _f(np.ascontiguousarray(inp["flpos"][0].T).reshape(6, 128, LT).transpose(1, 0, 2))
    d["dtb_sc"] = _f(inp["dt_b"].reshape(DEPTH, 12, 128).transpose(2, 0, 1).reshape(128, -1))
    d["convw_sc"] = _f(inp["conv_w"].reshape(DEPTH, 12, 128, 4).transpose(2, 0, 1, 3).reshape(128, -1))
    d["convb_sc"] = _f(inp["conv_b"].reshape(DEPTH, 12, 128).transpose(2, 0, 1).reshape(128, -1))
    d["Dp_sc"] = _f(inp["Dp"].reshape(DEPTH, 12, 128).transpose(2, 0, 1).reshape(128, -1))
    d["alog_sc"] = _f(inp["A_log"].reshape(DEPTH, 12, 128, NST).transpose(2, 0, 1, 3).reshape(128, -1))
    d["adab_sc"] = _f(inp["ada_b"].reshape(DEPTH, 6, 6, 128).transpose(3, 0, 1, 2).reshape(128, -1))
    qb = inp["qkv_b"]
    for nm, bias in (("bq_sc", qb[:, :DM]), ("bk_sc", qb[:, DM:2 * DM])):
        arr = np.zeros((DEPTH, NH, HP), np.float32)
        arr[:, :, :HD] = np.asarray(bias, np.float32).reshape(DEPTH, NH, HD)
        d[nm] = _f(arr.transpose(2, 0, 1).reshape(128, -1))
    bv = np.zeros((DEPTH, NH, HP), np.float32)
    bv[:, :, :HD] = np.asarray(qb[:, 2 * DM:], np.float32).reshape(DEPTH, NH, HD)
    d["bv_pad"] = _bf(bv.reshape(DEPTH, NH * HP))
    d["aob_sc"] = _f(inp["ao_b"].reshape(DEPTH, 6, 128).transpose(2, 0, 1).reshape(128, -1))
    d["fb1_sc"] = _f(inp["fb1"].reshape(DEPTH, 24, 128).transpose(2, 0, 1).reshape(128, -1))
    d["fb2_sc"] = _f(inp["fb2"].reshape(DEPTH, 6, 128).transpose(2, 0, 1).reshape(128, -1))
    d["finadab_sc"] = _f(inp["fin_ada_b"].reshape(2, 6, 128).transpose(2, 0, 1).reshape(128, 12))
    d["finb_sc"] = _f(inp["fin_b"].reshape(2, 128).T)

    # constants (input independent)
    d["id128"] = _bf(np.eye(128))
    d["id128f"] = _f(np.eye(128))
    d["ones_col"] = _f(np.ones((128, 1)))
    d["ones_colb"] = _bf(np.ones((128, 1)))
    d["ones_row"] = _f(np.ones((1, 128)))
    half = FREQ // 2
    return d


def prep_core(inp, b):
    d = {}
    d["xT"] = _bf(np.asarray(inp["x"][b], np.float32).T.reshape(2, 128, L))
    half = FREQ // 2
    fr = np.exp(-math.log(10000.0) * np.arange(half) / half).reshape(128, 1)
    d["ftp"] = _f(np.concatenate([fr, np.full((128, 1), np.asarray(inp["t"][b], np.float32))], 1))
    d["fc_cols"] = _bf(np.asarray(inp["fc"][b], np.float32).reshape(16, 128).T)
    d["flT"] = _bf(np.asarray(inp["fl"][b], np.float32).T.reshape(16, 128, LT))
    return d


def build_nc(depth=DEPTH):
    nc = bacc.Bacc(None)
    for val in (math.pi / 2, 1e-6, -math.pi):
        t_ = nc.alloc_sbuf_tensor(f"const-f32-{val}", [128, 1], f32)
        nc.gpsimd.memset(t_.ap(), val)
        nc.const_aps.aps[(f32, val)] = t_.ap()
    nc.all_engine_barrier()
    P = nc.declare_dram_parameter

    W = {}
    for nm, shp, dt in [
        ("xwT", [2, 128, DM], bf16), ("tw1T", [2, 128, DM], bf16),
        ("tw2T", [6, 128, DM], bf16), ("fcw1T", [16, 128, DM], bf16),
        ("fcw2T", [6, 128, DM], bf16), ("flwT", [16, 128, DM], bf16),
        ("adaT", [DEPTH, 6, 128, 6 * DM], bf16), ("in_wT", [DEPTH, 6, 128, 2 * DI], bf16),
        ("xprojT", [DEPTH, 12, 128, 96], bf16), ("dt_wT", [DEPTH, 48, DI], bf16),
        ("out_wT", [DEPTH, 12, 128, DM], bf16),
        ("wqT", [DEPTH, 6, 128, NH * HP], bf16), ("wkT", [DEPTH, 6, 128, NH * HP], bf16),
        ("wvT", [DEPTH, 6, 128, NH * HP], bf16), ("aoT", [DEPTH, 8, 128, DM], bf16),
        ("fw1T", [DEPTH, 6, 128, 4 * DM], bf16), ("fw2T", [DEPTH, 24, 128, DM], bf16),
        ("finadaT", [6, 128, 2 * DM], bf16), ("finT", [6, 128, CIN], bf16),
        ("xb_sc", [128, 6], f32), ("flb_sc", [128, 6], f32),
        ("tb1_r", [1, DM], f32), ("tb2_r", [1, DM], f32),
        ("fcb1_r", [1, DM], f32), ("fcb2_r", [1, DM], f32),
        ("flpos_sc", [128, 6, LT], f32),
        ("dtb_sc", [128, DEPTH * 12], f32), ("convw_sc", [128, DEPTH * 48], f32),
        ("convb_sc", [128, DEPTH * 12], f32), ("Dp_sc", [128, DEPTH * 12], f32),
        ("alog_sc", [128, DEPTH * 192], f32), ("adab_sc", [128, DEPTH * 36], f32),
        ("bq_sc", [128, DEPTH * 8], f32), ("bk_sc", [128, DEPTH * 8], f32),
        ("bv_pad", [DEPTH, NH * HP], bf16),
        ("aob_sc", [128, DEPTH * 6], f32), ("fb1_sc", [128, DEPTH * 24], f32),
        ("fb2_sc", [128, DEPTH * 6], f32), ("finadab_sc", [128, 12], f32),
        ("finb_sc", [128, 2], f32),
        ("id128", [128, 128], bf16), ("id128f", [128, 128], f32),
        ("ones_col", [128, 1], f32), ("ones_colb", [128, 1], bf16), ("ones_row", [1, 128], f32),
        ("xT", [2, 128, L], bf16), ("ftp", [128, 2], f32),
        ("fc_cols", [128, 16], bf16), ("flT", [16, 128, LT], bf16),
    ]:
        W[nm] = P(nm, shp, dt, isOutput=False)
    out_d = P("out", [L, CIN], f32, isOutput=True)
    scr_b1 = P("scr_b1", [1, DM], f32, isOutput=True)
    scr_b2 = P("scr_b2", [1, DM], f32, isOutput=True)
    scr_b3 = P("scr_b3", [1, DM], f32, isOutput=True)
    scr_mod = P("scr_mod", [DEPTH, 6 * DM], f32, isOutput=True)
    scr_bc = P("scr_bc", [1, 2 * NST * L], bf16, isOutput=True)
    scr_fm = P("scr_fm", [1, 2 * DM], f32, isOutput=True)

    MM, ACT, DVE, GPS, DMA = nc.tensor, nc.scalar, nc.vector, nc.gpsimd, nc.sync

    def g3(ap, n=GI, w=None):
        return ap.rearrange("p (g t) -> p g t", g=n)

    def load_kpm(dst, srcap, kdim):
        DMA.dma_start(out=dst[:].rearrange("p (k m) -> p k m", k=kdim),
                      in_=srcap.rearrange("k p m -> p k m"))

    with tile.TileContext(nc) as tc:
      from contextlib import ExitStack
      with ExitStack() as top:
        cp = top.enter_context(tc.tile_pool(name="cp", bufs=1))
        stg = top.enter_context(tc.tile_pool(name="stg", bufs=2))
        wb = top.enter_context(tc.tile_pool(name="wb", bufs=3))
        ws = top.enter_context(tc.tile_pool(name="ws", bufs=8))

        # ---- persistent SBUF state ----
        x = cp.tile([128, GM * L], f32)
        u_pad = cp.tile([128, SCANW], bf16)
        uc = cp.tile([128, SCANW], bf16)
        z = cp.tile([128, GI * L], bf16)
        dt_t = cp.tile([128, GI * L], bf16)
        du = cp.tile([128, GI * L], bf16)
        brep = cp.tile([128, NST * L], bf16)
        crep = cp.tile([128, NST * L], bf16)
        dA = [cp.tile([128, SCANW], bf16, name="dA0")] * 2
        dB = [cp.tile([128, SCANW], bf16, name=f"dB{i}") for i in range(2)]
        hh = [cp.tile([128, SCANW], bf16, name=f"hh{i}") for i in range(2)]
        y = cp.tile([128, GI * L], bf16)
        tmp1 = cp.tile([128, GM * L], bf16)
        xn = cp.tile([128, GM * L], bf16)
        hffn = cp.tile([128, 12 * L], bf16)
        q_sb = cp.tile([128, NH, L], bf16)
        k_sb = cp.tile([128, NH, LT], bf16)
        vt_sb = cp.tile([LT, NH * HP], bf16)
        pt_sb = cp.tile([LT, NH * L], bf16)
        p_all = cp.tile([128, NH * 2 * LT], bf16)
        rs_all = cp.tile([128, NH * 2], f32)
        ri_all = cp.tile([128, NH * 2], f32)
        avt_sb = cp.tile([128, NH, L], bf16)
        mod_all = cp.tile([128, DEPTH * 36], f32)
        modx_all = cp.tile([128, DEPTH * 12], f32)
        xdbl_sb = cp.tile([48, L], bf16)
        bc_sb = cp.tile([32, L], bf16)
        bvb = cp.tile([LT, NH * HP], bf16)
        modx = cp.tile([128, 12], f32)
        aneg = cp.tile([128, DEPTH * 192], f32)
        fl_e = cp.tile([128, GM * LT], bf16)
        silu_c = cp.tile([128, 6], bf16)
        stat = cp.tile([1, 2 * L], f32)
        stat2 = cp.tile([1, L], f32)
        small = cp.tile([128, 16], f32)      # ang etc
        smalli = cp.tile([128, 2], mybir.dt.int32)
        temb_c = cp.tile([128, 2], bf16)
        cvec = cp.tile([1, DM], f32)
        fmod_sc = cp.tile([128, 12], f32)

        # consts / biases resident
        C = {}
        for nm in ["dtb_sc", "convw_sc", "convb_sc", "Dp_sc", "adab_sc",
                   "bq_sc", "bk_sc", "aob_sc", "fb1_sc", "fb2_sc", "finadab_sc", "finb_sc",
                   "id128", "id128f", "ones_col", "ones_colb", "ones_row"]:
            C[nm] = cp.tile(list(W[nm].shape), W[nm].dtype, name="c_" + nm)
            DMA.dma_start(out=C[nm][:], in_=W[nm][:])

        # zero the pad columns once; interiors are always written strided
        for tl in dA + dB + [u_pad]:
            GPS.memset(tl[:], 0.0)


        # ---------------- preamble ----------------
        with tc.tile_pool(name="pre", bufs=1) as pre:
            from contextlib import ExitStack as _ES
            _es = _ES()
            psv = _es.enter_context(tc.tile_pool(name="psv", bufs=2, space="PSUM"))
            for nm in ["xb_sc", "flb_sc", "flpos_sc", "ftp", "fc_cols"]:
                C[nm] = pre.tile(list(W[nm].shape), W[nm].dtype, tag="p_" + nm, name="c_" + nm)
                DMA.dma_start(out=C[nm][:], in_=W[nm][:])
            for nm in ["tb1_r", "fcb1_r", "tb2_r", "fcb2_r"]:
                C[nm] = pre.tile(list(W[nm].shape), W[nm].dtype, tag="pvb", name="c_" + nm)
                DMA.dma_start(out=C[nm][:], in_=W[nm][:])
            for i in range(8):
                alg = pre.tile([128, 288], f32, tag="pal", name=f"alg{i}")
                DMA.dma_start(out=alg[:], in_=W["alog_sc"][:, i * 288:(i + 1) * 288])
                ACT.activation(out=aneg[:, i * 288:(i + 1) * 288], in_=alg[:], func=Act.Exp)
            DVE.tensor_scalar_mul(out=aneg[:], in0=aneg[:], scalar1=-1.0)
            # time embedding: ang = t*freqs mod 2pi; temb = [cos ang, sin ang]
            DVE.tensor_tensor(out=small[:, 0:1], in0=C["ftp"][:, 0:1], in1=C["ftp"][:, 1:2], op=Alu.mult)
            TWO_PI = 2 * math.pi
            # cos(ang)=sin(ang+pi/2); reduce each argument into [-pi, pi]
            DVE.tensor_scalar_add(out=small[:, 1:2], in0=small[:, 0:1], scalar1=math.pi / 2)
            for j, col in ((0, 1), (1, 0)):  # j=0: cos arg; j=1: sin arg
                src_c = 1 - col  # small col holding the argument
                a_ = small[:, src_c + 0:src_c + 1] if False else None
            for j, srccol in ((0, 1), (1, 0)):
                arg = small[:, srccol:srccol + 1]
                DVE.tensor_scalar_mul(out=small[:, 4 + j:5 + j], in0=arg, scalar1=1.0 / TWO_PI)
                DVE.tensor_copy(out=smalli[:, j:j + 1], in_=small[:, 4 + j:5 + j])
                DVE.tensor_copy(out=small[:, 6 + j:7 + j], in_=smalli[:, j:j + 1])
                DVE.scalar_tensor_tensor(out=small[:, 8 + j:9 + j], in0=small[:, 6 + j:7 + j],
                                         scalar=-TWO_PI, in1=arg, op0=Alu.mult, op1=Alu.add)
                DVE.tensor_scalar(out=small[:, 10 + j:11 + j], in0=small[:, 8 + j:9 + j],
                                  scalar1=math.pi, scalar2=None, op0=Alu.is_gt)
                DVE.scalar_tensor_tensor(out=small[:, 12 + j:13 + j], in0=small[:, 10 + j:11 + j],
                                         scalar=-TWO_PI, in1=small[:, 8 + j:9 + j],
                                         op0=Alu.mult, op1=Alu.add)
                ACT.activation(out=temb_c[:, j:j + 1], in_=small[:, 12 + j:13 + j], func=Act.Sin)

            tw1 = wb.tile([128, 2 * DM], bf16, tag="wb")
            load_kpm(tw1, W["tw1T"][:], 2)
            h1p = psv.tile([1, DM], f32, tag="vec")
            for k in range(2):
                for lo, hi in ((0, 512), (512, 768)):
                    MM.matmul(out=h1p[:, lo:hi],
                              lhsT=temb_c[:, k:k + 1],
                              rhs=tw1[:, k * DM + lo:k * DM + hi],
                              start=(k == 0), stop=(k == 1))
            h1 = pre.tile([1, DM], f32, tag="pv")
            for lo, hi in ((0, 512), (512, 768)):
                DVE.tensor_tensor(out=h1[:, lo:hi], in0=h1p[:, lo:hi], in1=C["tb1_r"][:, lo:hi], op=Alu.add)
            ACT.activation(out=h1[:], in_=h1[:], func=Act.Silu)
            GPS.dma_start(out=scr_b1[:], in_=h1[:])
            h1f = pre.tile([128, 6], f32, tag="pcf")
            GPS.dma_start(out=h1f[:], in_=scr_b1[0, :].rearrange("(g p) -> p g", g=6))
            h1c = pre.tile([128, 6], bf16, tag="pc")
            ACT.activation(out=h1c[:], in_=h1f[:], func=Act.Copy)

            h2p = psv.tile([1, DM], f32, tag="vec")
            for k in range(16):
                if k % 4 == 0:
                    fcw1c = wb.tile([128, 4 * DM], bf16, tag="wb", name=f"fcw1_{k // 4}")
                    load_kpm(fcw1c, W["fcw1T"][k:k + 4], 4)
                for lo, hi in ((0, 512), (512, 768)):
                    MM.matmul(out=h2p[:, lo:hi],
                              lhsT=C["fc_cols"][:, k:k + 1],
                              rhs=fcw1c[:, (k % 4) * DM + lo:(k % 4) * DM + hi],
                              start=(k == 0), stop=(k == 15))
            h2 = pre.tile([1, DM], f32, tag="pv")
            for lo, hi in ((0, 512), (512, 768)):
                DVE.tensor_tensor(out=h2[:, lo:hi], in0=h2p[:, lo:hi], in1=C["fcb1_r"][:, lo:hi], op=Alu.add)
            ACT.activation(out=h2[:], in_=h2[:], func=Act.Silu)
            GPS.dma_start(out=scr_b2[:], in_=h2[:])
            h2f = pre.tile([128, 6], f32, tag="pcf2")
            GPS.dma_start(out=h2f[:], in_=scr_b2[0, :].rearrange("(g p) -> p g", g=6))
            h2c = pre.tile([128, 6], bf16, tag="pc3")
            ACT.activation(out=h2c[:], in_=h2f[:], func=Act.Copy)

            # c = tw2@h1 + fcw2@h2 + tb2 + fcb2 ; silu; scatter
            cp_ps = psv.tile([1, DM], f32, tag="vec")
            nmm = 0
            for hsrc, wnm in ((h1c, "tw2T"), (h2c, "fcw2T")):
                for k in range(6):
                    if k % 3 == 0:
                        wc = wb.tile([128, 3 * DM], bf16, tag="wb", name=f"cw_{wnm}_{k}")
                        load_kpm(wc, W[wnm][k:k + 3], 3)
                    for lo, hi in ((0, 512), (512, 768)):
                        MM.matmul(out=cp_ps[:, lo:hi],
                                  lhsT=hsrc[:, k:k + 1],
                                  rhs=wc[:, (k % 3) * DM + lo:(k % 3) * DM + hi],
                                  start=(nmm == 0), stop=(nmm == 11))
                    nmm += 1
            for lo, hi in ((0, 512), (512, 768)):
                DVE.tensor_tensor(out=cvec[:, lo:hi], in0=cp_ps[:, lo:hi], in1=C["tb2_r"][:, lo:hi], op=Alu.add)
            DVE.tensor_tensor(out=cvec[:], in0=cvec[:], in1=C["fcb2_r"][:], op=Alu.add)
            ACT.activation(out=cvec[:], in_=cvec[:], func=Act.Silu)
            GPS.dma_start(out=scr_b3[:], in_=cvec[:])
            scf32 = pre.tile([128, 6], f32, tag="pc4")
            GPS.dma_start(out=scf32[:], in_=scr_b3[0, :].rearrange("(g p) -> p g", g=6))
            ACT.activation(out=silu_c[:], in_=scf32[:], func=Act.Copy)

            # fl_e = flw@fl + flb + flpos
            flsb = pre.tile([128, 16, LT], bf16, tag="pfl")
            GPS.dma_start(out=flsb[:], in_=W["flT"][:].rearrange("k p m -> p k m"))
            _es.close()
            _es = _ES()
            psfl = _es.enter_context(tc.tile_pool(name="psfl", bufs=1, space="PSUM"))
            fps = [psfl.tile([128, LT], f32, tag=f"fl{m}", name=f"flp{m}") for m in range(6)]
            for k in range(16):
                if k % 4 == 0:
                    flwc = wb.tile([128, 4 * DM], bf16, tag="wb", name=f"flw_{k // 4}")
                    load_kpm(flwc, W["flwT"][k:k + 4], 4)
                for m in range(6):
                    MM.matmul(out=fps[m][:],
                              lhsT=flwc[:, (k % 4) * DM + m * 128:(k % 4) * DM + (m + 1) * 128],
                              rhs=flsb[:, k, :], start=(k == 0), stop=(k == 15))
            for m in range(6):
                t_ = pre.tile([128, LT], f32, tag="pt2", name=f"fle{m}")
                ACT.activation(out=t_[:], in_=fps[m][:], func=Act.Identity, bias=C["flb_sc"][:, m:m + 1])
                DVE.tensor_tensor(out=fl_e[:, m * LT:(m + 1) * LT], in0=t_[:],
                                  in1=C["flpos_sc"][:, m, :], op=Alu.add)

            # x embedding
            _es.close()
            _es = _ES()
            ps1 = _es.enter_context(tc.tile_pool(name="ps1", bufs=2, space="PSUM"))
            xw = wb.tile([128, 2 * DM], bf16, tag="wb")
            load_kpm(xw, W["xwT"][:], 2)
            xsb = pre.tile([128, 2, L], bf16, tag="pfl2")
            GPS.dma_start(out=xsb[:], in_=W["xT"][:].rearrange("k p m -> p k m"))
            for m in range(6):
                xp = ps1.tile([128, L], f32, tag="mm")
                for k in range(2):
                    MM.matmul(out=xp[:], lhsT=xw[:, k * DM + m * 128:k * DM + (m + 1) * 128],
                              rhs=xsb[:, k, :], start=(k == 0), stop=(k == 1))
                ACT.activation(out=x[:, m * L:(m + 1) * L], in_=xp[:],
                               func=Act.Identity, bias=C["xb_sc"][:, m:m + 1])
            _es.close()

        # ---------------- layers ----------------
        x3 = lambda g: x[:, g * L:(g + 1) * L]
        xn3 = lambda g: xn[:, g * L:(g + 1) * L]
        dt3 = dt_t[:].rearrange("p (g t) -> p g t", g=GI)
        du3 = du[:].rearrange("p (g t) -> p g t", g=GI)
        y3 = y[:].rearrange("p (g t) -> p g t", g=GI)
        uc3i = uc[:].rearrange("p (g s) -> p g s", g=GI)[:, :, 4:SEG]
        up3 = u_pad[:].rearrange("p (g s) -> p g s", g=GI)

        def ln_block(l, psA, psS, psB, scale_col, shift_col):
            """LayerNorm of x -> xn (bf16), optionally modulated."""
            ACT.activation(out=tmp1[:], in_=x[:], func=Act.Square)
            st = psS.tile([1, 512], f32, tag="st", name=f"st{l}")
            for g in range(GM):
                MM.matmul(out=st[:, 0:L], lhsT=C["ones_col"][:], rhs=x3(g),
                          start=(g == 0), stop=(g == GM - 1))
            for g in range(GM):
                MM.matmul(out=st[:, L:2 * L], lhsT=C["ones_colb"][:],
                          rhs=tmp1[:, g * L:(g + 1) * L],
                          start=(g == 0), stop=(g == GM - 1))
            ACT.activation(out=stat[:, 0:L], in_=st[:, 0:L], func=Act.Copy, scale=1.0 / DM)
            ACT.activation(out=stat2[:], in_=stat[:, 0:L], func=Act.Square)
            DVE.scalar_tensor_tensor(out=stat2[:], in0=st[:, L:2 * L], scalar=1.0 / DM,
                                     in1=stat2[:], op0=Alu.mult, op1=Alu.subtract)
            ACT.activation(out=stat2[:], in_=stat2[:], func=Act.Sqrt, bias=1e-6)
            DVE.reciprocal(out=stat[:, L:2 * L], in_=stat2[:])
            bc = psB.tile([128, 512], f32, tag="bc", name=f"bc{l}")
            MM.matmul(out=bc[:], lhsT=C["ones_row"][:], rhs=stat[:, 0:512])
            for g in range(GM):
                DVE.tensor_tensor(out=tmp1[:, g * L:(g + 1) * L], in0=x3(g),
                                  in1=bc[:, 0:L], op=Alu.subtract)
                if scale_col is None:
                    DVE.tensor_tensor(out=xn3(g), in0=tmp1[:, g * L:(g + 1) * L],
                                      in1=bc[:, L:2 * L], op=Alu.mult)
                else:
                    DVE.tensor_tensor(out=tmp1[:, g * L:(g + 1) * L],
                                      in0=tmp1[:, g * L:(g + 1) * L],
                                      in1=bc[:, L:2 * L], op=Alu.mult)
                    DVE.scalar_tensor_tensor(
                        out=xn3(g), in0=tmp1[:, g * L:(g + 1) * L],
                        scalar=scale_col[:, g:g + 1],
                        in1=shift_col[:, g:g + 1].broadcast_to([128, L]),
                        op0=Alu.mult, op1=Alu.add)

        def ada_block(l, psM):
            """adaLN modulation matvecs for layer l -> mod_all/modx_all slices.

            Emitted two layers early so PE/DMA fill the scan window."""
            for r in range(2):
                for si, (lo, wdt) in enumerate(
                        ((0, 512), (512, 512), (1024, 512), (1536, 512), (2048, 256))):
                    ps = psM.tile([1, 512], f32, tag="m", name=f"mps{l}_{r}_{si}")
                    for k in range(6):
                        ah = ws.tile([128, 768], bf16, tag="ws", name=f"ada{l}_{r}_{si}_{k}")
                        DMA.dma_start(out=ah[:, 0:wdt],
                                      in_=W["adaT"][l, k][:, r * 2304 + lo:r * 2304 + lo + wdt])
                        MM.matmul(out=ps[:, 0:wdt], lhsT=silu_c[:, k:k + 1],
                                  rhs=ah[:, 0:wdt],
                                  start=(k == 0), stop=(k == 5))
                    sg = stg.tile([1, 512], f32, tag="stg", name=f"sg{l}_{r}_{si}")
                    ACT.activation(out=sg[:, 0:wdt], in_=ps[:, 0:wdt], func=Act.Copy)
                    GPS.dma_start(out=scr_mod[l:l + 1, r * 2304 + lo:r * 2304 + lo + wdt],
                                  in_=sg[:, 0:wdt])
            mf = mod_all[:, l * 36:(l + 1) * 36]
            GPS.dma_start(out=mf.rearrange("p (bl g) -> p bl g", bl=6),
                          in_=scr_mod[l, :].rearrange("(bl g p) -> p bl g", bl=6, g=6))
            DVE.tensor_tensor(out=mf, in0=mf,
                              in1=C["adab_sc"][:, l * 36:(l + 1) * 36], op=Alu.add)
            DVE.tensor_scalar_add(out=modx_all[:, l * 12:l * 12 + 6],
                                  in0=mod_all[:, l * 36 + 6:l * 36 + 12], scalar1=1.0)
            DVE.tensor_scalar_add(out=modx_all[:, l * 12 + 6:l * 12 + 12],
                                  in0=mod_all[:, l * 36 + 24:l * 36 + 30], scalar1=1.0)

        for l in range(2):
            with tc.tile_pool(name=f"psMp{l}", bufs=2, space="PSUM") as psM:
                ada_block(l, psM)

        for l in range(depth):
            mod_f = mod_all[:, l * 36:(l + 1) * 36]
            modx_l = modx_all[:, l * 12:(l + 1) * 12]

            # ---- mamba ----
            with tc.tile_pool(name=f"psA{l}", bufs=2, space="PSUM") as psA, \
                 tc.tile_pool(name=f"psS{l}", bufs=1, space="PSUM") as psS, \
                 tc.tile_pool(name=f"psB{l}", bufs=1, space="PSUM") as psB, \
                 tc.tile_pool(name=f"psV{l}", bufs=1, space="PSUM") as psV, \
                 tc.tile_pool(name=f"psM{l}", bufs=2, space="PSUM") as psM, \
                 tc.tile_pool(name=f"psX{l}", bufs=1, space="PSUM") as psX:
                ln_block(10 * l, psA, psS, psB, modx_l[:, 0:6], mod_f[:, 0:6])

                for mb in range(4):
                    wts = []
                    for k in range(6):
                        wi = ws.tile([128, 768], bf16, tag="ws", name=f"inw{l}_{mb}_{k}")
                        DMA.dma_start(out=wi[:], in_=W["in_wT"][l, k][:, mb * 768:(mb + 1) * 768])
                        wts.append(wi)
                    for mi in range(6):
                        m = mb * 6 + mi
                        ps = psA.tile([128, L], f32, tag="mm", name=f"ip{l}_{m}")
                        for k in range(6):
                            MM.matmul(out=ps[:], lhsT=wts[k][:, mi * 128:(mi + 1) * 128],
                                      rhs=xn3(k), start=(k == 0), stop=(k == 5))
                        if m < 12:
                            ACT.activation(out=up3[:, m, 4:SEG], in_=ps[:], func=Act.Copy)
                        else:
                            ACT.activation(out=z[:, (m - 12) * L:(m - 11) * L], in_=ps[:], func=Act.Silu)

                # ---- attention K/V: depend only on fl_e, so compute them here
                # so PE has work queued during the scan window ----
                wk_ = []
                for i in range(2):
                    t_ = wb.tile([128, 3 * NH * HP], bf16, tag="wb", name=f"wk{l}_{i}")
                    load_kpm(t_, W["wkT"][l, 3 * i:3 * i + 3], 3)
                    wk_.append(t_)
                for h in range(NH):
                    ps = psA.tile([128, L], f32, tag="mm", name=f"kp{l}_{h}")
                    for k in range(6):
                        MM.matmul(out=ps[:, 0:LT],
                                  lhsT=wk_[k // 3][:, (k % 3) * NH * HP + h * HP:(k % 3) * NH * HP + (h + 1) * HP],
                                  rhs=fl_e[:, k * LT:(k + 1) * LT], start=(k == 0), stop=(k == 5))
                    ACT.activation(out=k_sb[:, h, :], in_=ps[:, 0:LT], func=Act.Identity,
                                   bias=C["bk_sc"][:, l * 8 + h:l * 8 + h + 1])
                wv_ = []
                for i in range(2):
                    t_ = wb.tile([128, 3 * NH * HP], bf16, tag="wb", name=f"wv{l}_{i}")
                    load_kpm(t_, W["wvT"][l, 3 * i:3 * i + 3], 3)
                    wv_.append(t_)
                GPS.dma_start(out=bvb[:], in_=W["bv_pad"][l].partition_broadcast(LT))
                for fo in range(2):
                    vt_ps = psV.tile([LT, 512], f32, tag="vt", name=f"vtp{l}_{fo}")
                    for k in range(6):
                        MM.matmul(out=vt_ps[:],
                                  lhsT=fl_e[:, k * LT:(k + 1) * LT],
                                  rhs=wv_[k // 3][:, (k % 3) * NH * HP + fo * 512:(k % 3) * NH * HP + (fo + 1) * 512],
                                  start=(k == 0), stop=(k == 5))
                    DVE.tensor_tensor(out=vt_sb[:, fo * 512:(fo + 1) * 512],
                                      in0=vt_ps[:],
                                      in1=bvb[:, fo * 512:(fo + 1) * 512], op=Alu.add)

                # causal depthwise conv, per channel-group on DVE: each
                # group's 4-tap chain depends only on its own in_proj slice,
                # so Tile overlaps it behind the remaining in_proj matmuls
                cw4 = C["convw_sc"][:].rearrange("p (a g k) -> p a g k", a=DEPTH, g=12)
                for g in range(GI):
                    acc = y[:, g * L:(g + 1) * L]
                    DVE.tensor_scalar_mul(out=acc, in0=up3[:, g, 1:1 + L],
                                          scalar1=cw4[:, l, g, 0:1])
                    for k in range(1, 4):
                        DVE.scalar_tensor_tensor(out=acc, in0=up3[:, g, 1 + k:1 + k + L],
                                                 scalar=cw4[:, l, g, k:k + 1],
                                                 in1=acc, op0=Alu.mult, op1=Alu.add)
                    ACT.activation(out=uc3i[:, g, :], in_=acc,
                                   func=Act.Silu, bias=C["convb_sc"][:, l * 12 + g:l * 12 + g + 1])

                # xproj -> xdbl [80, L]
                xpw = wb.tile([128, 1152], bf16, tag="wb", name=f"xpw{l}")
                load_kpm(xpw, W["xprojT"][l], 12)
                xd = psX.tile([96, L], f32, tag="xd", name=f"xd{l}")
                for k in range(GI):
                    MM.matmul(out=xd[:], lhsT=xpw[:, k * 96:(k + 1) * 96],
                              rhs=uc3i[:, k, :], start=(k == 0), stop=(k == GI - 1))
                ACT.activation(out=xdbl_sb[:], in_=xd[0:48, :], func=Act.Copy)
                ACT.activation(out=bc_sb[:], in_=xd[64:96, :], func=Act.Copy)
                GPS.dma_start(out=scr_bc[0, :].rearrange("(a b) -> a b", a=32), in_=bc_sb[:])
                GPS.dma_start(out=brep[:], in_=scr_bc[0, 0:NST * L].partition_broadcast(128))
                GPS.dma_start(out=crep[:], in_=scr_bc[0, NST * L:2 * NST * L].partition_broadcast(128))

                # dt = softplus(dt_w @ xdbl[:48] + dt_b)
                dtw = wb.tile([48, DI], bf16, tag="wb", name=f"dtw{l}")
                DMA.dma_start(out=dtw[:], in_=W["dt_wT"][l])
                for m in range(GI):
                    ps = psA.tile([128, L], f32, tag="mm", name=f"dtp{l}_{m}")
                    MM.matmul(out=ps[:], lhsT=dtw[:, m * 128:(m + 1) * 128], rhs=xdbl_sb[:])
                    sl_ = dt_t[:, m * L:(m + 1) * L]
                    ACT.activation(out=sl_, in_=ps[:], func=Act.Exp,
                                   bias=C["dtb_sc"][:, l * 12 + m:l * 12 + m + 1])
                # softplus tail as two full-width ops: keeps the Exp/Ln table
                # sets from reloading once per m-slice
                DVE.tensor_scalar_add(out=dt_t[:], in0=dt_t[:], scalar1=1.0)
                ACT.activation(out=dt_t[:], in_=dt_t[:], func=Act.Ln)
                DVE.tensor_tensor(out=du3, in0=dt3, in1=uc3i, op=Alu.mult)

                # modulation matvecs for layer l+2: PE+DMA work that overlaps
                # the DVE-bound scan below
                if l + 2 < depth:
                    ada_block(l + 2, psM)

                # selective scan over n (state dim), pad cols reset state
                for n in range(NST):
                    sl = n % 2
                    dAi = dA[sl][:].rearrange("p (g s) -> p g s", g=GI)[:, :, 4:SEG]
                    dBi = dB[sl][:].rearrange("p (g s) -> p g s", g=GI)[:, :, 4:SEG]
                    hi = hh[sl][:].rearrange("p (g s) -> p g s", g=GI)[:, :, 4:SEG]
                    ACT.activation(out=dAi, in_=dt3, func=Act.Exp,
                                   scale=aneg[:, l * 192 + n:l * 192 + n + 1])
                    DVE.tensor_tensor(out=dBi, in0=du3,
                                      in1=brep[:, n * L:(n + 1) * L].unsqueeze(1).broadcast_to([128, GI, L]),
                                      op=Alu.mult)
                    DVE.tensor_tensor_scan(out=hh[sl][:], data0=dA[sl][:], data1=dB[sl][:],
                                           initial=0.0, op0=Alu.mult, op1=Alu.add)
                    DVE.tensor_tensor(out=hi, in0=hi,
                                      in1=crep[:, n * L:(n + 1) * L].unsqueeze(1).broadcast_to([128, GI, L]),
                                      op=Alu.mult)
                    if n == 0:
                        DVE.tensor_copy(out=y3, in_=hi)
                    else:
                        DVE.tensor_tensor(out=y3, in0=y3, in1=hi, op=Alu.add)

                # y = (y + uc*Dp) * silu(z);  out-proj; residual with gm
                for g in range(GI):
                    DVE.scalar_tensor_tensor(out=y[:, g * L:(g + 1) * L], in0=uc3i[:, g, :],
                                             scalar=C["Dp_sc"][:, l * 12 + g:l * 12 + g + 1],
                                             in1=y[:, g * L:(g + 1) * L], op0=Alu.mult, op1=Alu.add)
                DVE.tensor_tensor(out=du[:], in0=y[:], in1=z[:], op=Alu.mult)

                ow = []
                for i in range(2):
                    wi = wb.tile([128, 6 * DM], bf16, tag="wb", name=f"ow{l}_{i}")
                    load_kpm(wi, W["out_wT"][l, 6 * i:6 * i + 6], 6)
                    ow.append(wi)
                for m in range(GM):
                    ps = psA.tile([128, L], f32, tag="mm", name=f"op{l}_{m}")
                    for k in range(GI):
                        MM.matmul(out=ps[:], lhsT=ow[k // 6][:, (k % 6) * DM + m * 128:(k % 6) * DM + (m + 1) * 128],
                                  rhs=du[:, k * L:(k + 1) * L], start=(k == 0), stop=(k == GI - 1))
                    DVE.scalar_tensor_tensor(out=x3(m), in0=ps[:], scalar=mod_f[:, 12 + m:13 + m],
                                             in1=x3(m), op0=Alu.mult, op1=Alu.add)

            # ---- cross attention ----
            with tc.tile_pool(name=f"qsA{l}", bufs=2, space="PSUM") as psA, \
                 tc.tile_pool(name=f"qsS{l}", bufs=1, space="PSUM") as psS, \
                 tc.tile_pool(name=f"qsB{l}", bufs=1, space="PSUM") as psB, \
                 tc.tile_pool(name=f"qsC{l}", bufs=1, space="PSUM") as psC, \
                 tc.tile_pool(name=f"qsP{l}", bufs=1, space="PSUM") as psP:
                ln_block(10 * l + 1, psA, psS, psB, None, None)

                wq = []
                for i in range(2):
                    t_ = wb.tile([128, 3 * NH * HP], bf16, tag="wb", name=f"wq{l}_{i}")
                    load_kpm(t_, W["wqT"][l, 3 * i:3 * i + 3], 3)
                    wq.append(t_)
                for h in range(NH):
                    ps = psA.tile([128, L], f32, tag="mm", name=f"qp{l}_{h}")
                    for k in range(6):
                        MM.matmul(out=ps[:], lhsT=wq[k // 3][:, (k % 3) * NH * HP + h * HP:(k % 3) * NH * HP + (h + 1) * HP],
                                  rhs=xn3(k), start=(k == 0), stop=(k == 5))
                    ACT.activation(out=q_sb[:, h, :], in_=ps[:], func=Act.Identity,
                                   bias=C["bq_sc"][:, l * 8 + h:l * 8 + h + 1])

                for h in range(NH):
                    for tc2 in range(2):
                        idx = h * 2 + tc2
                        sc_ps = psC.tile([128, LT], f32, tag="sc", name=f"scp{l}_{idx}")
                        MM.matmul(out=sc_ps[:], lhsT=q_sb[:, h, tc2 * 128:(tc2 + 1) * 128],
                                  rhs=k_sb[:, h, :])
                        mx = small[:, 2:3]
                        DVE.tensor_reduce(out=mx, in_=sc_ps[:], axis=mybir.AxisListType.X, op=Alu.max)
                        DVE.tensor_scalar_mul(out=small[:, 3:4], in0=mx, scalar1=-SQ)
                        ACT.activation(out=p_all[:, idx * LT:(idx + 1) * LT], in_=sc_ps[:],
                                       func=Act.Exp, scale=SQ, bias=small[:, 3:4],
                                       accum_out=rs_all[:, idx:idx + 1])
                DVE.reciprocal(out=ri_all[:], in_=rs_all[:])
                for h in range(NH):
                    for tc2 in range(2):
                        idx = h * 2 + tc2
                        DVE.tensor_scalar_mul(out=p_all[:, idx * LT:(idx + 1) * LT],
                                              in0=p_all[:, idx * LT:(idx + 1) * LT],
                                              scalar1=ri_all[:, idx:idx + 1])
                        pt_ps = psP.tile([LT, 128], bf16, tag="pt", name=f"ptp{l}_{idx}")
                        MM.transpose(out=pt_ps[:], in_=p_all[:, idx * LT:(idx + 1) * LT], identity=C["id128"][:])
                        ACT.activation(out=pt_sb[:, h * L + tc2 * 128:h * L + (tc2 + 1) * 128],
                                       in_=pt_ps[:], func=Act.Copy)
                for h in range(NH):
                    ps = psA.tile([128, L], f32, tag="mm", name=f"avp{l}_{h}")
                    for tc2 in range(2):
                        MM.matmul(out=ps[:, tc2 * 128:(tc2 + 1) * 128],
                                  lhsT=vt_sb[:, h * HP:(h + 1) * HP],
                                  rhs=pt_sb[:, h * L + tc2 * 128:h * L + (tc2 + 1) * 128])
                    ACT.activation(out=avt_sb[:, h, :], in_=ps[:], func=Act.Copy)

                ao = []
                for i in range(2):
                    t_ = wb.tile([128, 4 * DM], bf16, tag="wb", name=f"ao{l}_{i}")
                    load_kpm(t_, W["aoT"][l, 4 * i:4 * i + 4], 4)
                    ao.append(t_)
                for m in range(GM):
                    ps = psA.tile([128, L], f32, tag="mm", name=f"aop{l}_{m}")
                    for k in range(NH):
                        MM.matmul(out=ps[:], lhsT=ao[k // 4][:, (k % 4) * DM + m * 128:(k % 4) * DM + (m + 1) * 128],
                                  rhs=avt_sb[:, k, :], start=(k == 0), stop=(k == NH - 1))
                    DVE.scalar_tensor_tensor(out=x3(m), in0=ps[:], scalar=C["aob_sc"][:, l * 6 + m:l * 6 + m + 1],
                                             in1=x3(m), op0=Alu.add, op1=Alu.add)

            # ---- FFN ----
            with tc.tile_pool(name=f"fsA{l}", bufs=2, space="PSUM") as psA, \
                 tc.tile_pool(name=f"fsS{l}", bufs=1, space="PSUM") as psS, \
                 tc.tile_pool(name=f"fsB{l}", bufs=1, space="PSUM") as psB:
                ln_block(10 * l + 2, psA, psS, psB, modx_l[:, 6:12], mod_f[:, 18:24])
                for half in range(2):
                    for mb in range(2):
                        wts = []
                        for k in range(6):
                            wi = ws.tile([128, 768], bf16, tag="ws", name=f"f1w{l}_{half}_{mb}_{k}")
                            DMA.dma_start(out=wi[:],
                                          in_=W["fw1T"][l, k][:, (half * 2 + mb) * 768:(half * 2 + mb + 1) * 768])
                            wts.append(wi)
                        for mi in range(6):
                            m = half * 12 + mb * 6 + mi
                            ml = mb * 6 + mi
                            ps = psA.tile([128, L], f32, tag="mm", name=f"f1p{l}_{m}")
                            for k in range(6):
                                MM.matmul(out=ps[:], lhsT=wts[k][:, mi * 128:(mi + 1) * 128],
                                          rhs=xn3(k), start=(k == 0), stop=(k == 5))
                            ACT.activation(out=hffn[:, ml * L:(ml + 1) * L], in_=ps[:], func=Act.Gelu,
                                           bias=C["fb1_sc"][:, l * 24 + m:l * 24 + m + 1])
                    f2 = []
                    for i in range(2):
                        t_ = wb.tile([128, 6 * DM], bf16, tag="wb", name=f"f2{l}_{half}_{i}")
                        load_kpm(t_, W["fw2T"][l, half * 12 + 6 * i:half * 12 + 6 * i + 6], 6)
                        f2.append(t_)
                    for m in range(GM):
                        ps = psA.tile([128, L], f32, tag="mm", name=f"f2p{l}_{half}_{m}")
                        for k in range(12):
                            MM.matmul(out=ps[:], lhsT=f2[k // 6][:, (k % 6) * DM + m * 128:(k % 6) * DM + (m + 1) * 128],
                                      rhs=hffn[:, k * L:(k + 1) * L], start=(k == 0), stop=(k == 11))
                        if half == 0:
                            ACT.activation(out=tmp1[:, m * 256:(m + 1) * 256], in_=ps[:], func=Act.Copy)
                        else:
                            DVE.tensor_tensor(out=tmp1[:, m * 256:(m + 1) * 256],
                                              in0=tmp1[:, m * 256:(m + 1) * 256], in1=ps[:], op=Alu.add)
                            DVE.tensor_scalar(out=tmp1[:, m * 256:(m + 1) * 256],
                                              in0=tmp1[:, m * 256:(m + 1) * 256],
                                              scalar1=C["fb2_sc"][:, l * 6 + m:l * 6 + m + 1],
                                              scalar2=mod_f[:, 30 + m:31 + m], op0=Alu.add, op1=Alu.mult)
                            DVE.tensor_tensor(out=x3(m), in0=x3(m), in1=tmp1[:, m * 256:(m + 1) * 256], op=Alu.add)

        # ---------------- final ----------------
        with tc.tile_pool(name="fin", bufs=1) as fin, \
             tc.tile_pool(name="zsA", bufs=2, space="PSUM") as psA, \
             tc.tile_pool(name="zsS", bufs=1, space="PSUM") as psS, \
             tc.tile_pool(name="zsB", bufs=1, space="PSUM") as psB, \
             tc.tile_pool(name="zsV", bufs=3, space="PSUM") as psV:
            fm_ps = [psV.tile([1, 512], f32, tag="fm5", name=f"fmps{s}") for s in range(3)]
            for k in range(6):
                fad = fin.tile([128, 2 * DM], bf16, tag="fw", name=f"fad{k}")
                load_kpm(fad, W["finadaT"][k:k + 1], 1)
                for s in range(3):
                    MM.matmul(out=fm_ps[s][:],
                              lhsT=silu_c[:, k:k + 1],
                              rhs=fad[:, s * 512:(s + 1) * 512],
                              start=(k == 0), stop=(k == 5))
            for s in range(3):
                sg = stg.tile([1, 512], f32, tag="stg", name=f"fsg{s}")
                ACT.activation(out=sg[:], in_=fm_ps[s][:], func=Act.Copy)
                GPS.dma_start(out=scr_fm[:, s * 512:(s + 1) * 512], in_=sg[:])
            GPS.dma_start(out=fmod_sc[:].rearrange("p (bl g) -> p bl g", bl=2),
                          in_=scr_fm[0, :].rearrange("(bl g p) -> p bl g", bl=2, g=6))
            DVE.tensor_tensor(out=fmod_sc[:], in0=fmod_sc[:], in1=C["finadab_sc"][:], op=Alu.add)
            DVE.tensor_scalar_add(out=modx[:, 0:6], in0=fmod_sc[:, 6:12], scalar1=1.0)
            ln_block(999, psA, psS, psB, modx[:, 0:6], fmod_sc[:, 0:6])

            xo_sb = fin.tile([128, 2, CIN], f32, tag="fxo")
            outT = fin.tile([128, 2 * CIN], f32, tag="fot")
            fw = fin.tile([128, 6 * CIN], bf16, tag="fw2")
            load_kpm(fw, W["finT"][:], 6)
            for m in range(2):
                ps = psA.tile([128, L], f32, tag="mm", name=f"fop{m}")
                for k in range(6):
                    MM.matmul(out=ps[:], lhsT=fw[:, k * CIN + m * 128:k * CIN + (m + 1) * 128],
                              rhs=xn3(k), start=(k == 0), stop=(k == 5))
                ACT.activation(out=xo_sb[:, m, :], in_=ps[:], func=Act.Identity,
                               bias=C["finb_sc"][:, m:m + 1])
            # transpose [ch, t] -> [t, ch] and store
            for tc2 in range(2):
                for m in range(2):
                    tp = psA.tile([128, 128], f32, tag="mm", name=f"tp{tc2}_{m}")
                    MM.transpose(out=tp[:], in_=xo_sb[:, m, tc2 * 128:(tc2 + 1) * 128],
                                 identity=C["id128f"][:])
                    ACT.activation(out=outT[:, tc2 * CIN + m * 128:tc2 * CIN + (m + 1) * 128],
                                   in_=tp[:], func=Act.Copy)
            GPS.dma_start(out=out_d[:].rearrange("(a p) c -> p a c", a=2),
                          in_=outT[:].rearrange("p (a c) -> p a c", a=2))
    nc.finalize()
    # walrus' verifier rejects leftover unused framework registers with
    # reg_id=-1; give each a harmless unique id per engine.
    from collections import defaultdict
    nxt = defaultdict(int)
    for fn in nc.m.functions:
        for a in fn.allocations:
            if getattr(a, "reg_id", None) == -1:
                eng = str(getattr(a, "engine", "?"))
                n = getattr(a, "num_physical_regs", None) or 1
                if n > 1 and nxt[eng] % 2:
                    nxt[eng] += 1
                a.reg_id = nxt[eng]
                nxt[eng] += n
    return nc


_CACHE = {}


def kernel(**inputs):
    depth = DEPTH
    if "nc" not in _CACHE:
        _CACHE["nc"] = build_nc(depth)
    nc = _CACHE["nc"]
    shared = prep_shared(inputs)
    in_maps = []
    for b in range(N_CORES):
        m = dict(shared)
        m.update(prep_core(inputs, b))
        in_maps.append(m)
    res = run_bass_kernel_spmd(nc, in_maps, list(range(N_CORES)))
    out = np.stack([np.asarray(res.results[b]["out"], np.float32) for b in range(N_CORES)])
    return out


def kernel_profiled(**inputs):
    if "nc" not in _CACHE:
        _CACHE["nc"] = build_nc(DEPTH)
    nc = _CACHE["nc"]
    shared = prep_shared(inputs)
    in_maps = []
    for b in range(N_CORES):
        m = dict(shared)
        m.update(prep_core(inputs, b))
        in_maps.append(m)
    res = run_bass_kernel_spmd(nc, in_maps, list(range(N_CORES)), trace=True)
    out = np.stack([np.asarray(res.results[b]["out"], np.float32) for b in range(N_CORES)])
    return out, res.exec_time_ns

